# revision 14
# baseline (speedup 1.0000x reference)
"""GQA attention Trainium2 kernel (8 NeuronCores, SPMD, no collectives).

Sharding: 2-way data parallel (batch) x 4-way tensor parallel (heads).
Core c handles batch b=c//4 and head-group g=c%4 (8 q heads, 2 kv heads).
Each core produces a partial o_proj output (transposed, [HID, S] bf16);
the host sums the 4 partials per batch (f32) and transposes back.

On-device layout is feature-major ("transposed"): hidden is passed as
hT=[HID,S], projections produce qT/kT/gateT=[dim,S], attention scores are
computed as scoresT=[s_k,s_q] so softmax-exp output feeds the PV matmul
directly (lhsT = natural-layout V with an appended ones column that yields
the softmax denominator in psum row 64).

v2 changes vs baseline:
- rstd via ACT Rsqrt + matmul broadcast (sel2 [2,128] lhsT) instead of
  Sqrt + DVE reciprocal + gpsimd partition_broadcast (reciprocal was
  4.8us/instr, 171us total).
- rope math in bf16 (DVE 2x mode).
- gates staged raw into og_sb, sigmoid batched after phase 1 (avoids
  ACT table-set thrash: rsqrt set resident through phase 1).
- attention processed as head PAIRS (kv0 head rows 0-63, kv1 head rows
  64-127) with interleaved K=64 score matmuls -> concurrent row-tiled
  execution on the PE array (2x score throughput).
- causal trimming at 128-col granularity for scores/exp/PV; boundary
  128x128 strip masked by a DVE tril multiply (replaces gpsimd
  affine_select).
- softmax 1/denom via DVE reciprocal_approx_fast + matmul broadcast.
- o_proj split per q-half and emitted between attention halves so the
  PE has work while ACT grinds exp.
- outT in bf16 (host accumulates partials in f32).
"""

import os
import sys
import numpy as np

for _p in ("/opt/trn_rl_repo", "/root/.axon_site/_ro/trn_rl_repo"):
    if os.path.isdir(_p) and _p not in sys.path:
        sys.path.insert(0, _p)

import ml_dtypes

B, S, HID = 2, 2048, 2048
NH, NKV, HD = 32, 8, 64
ROPE = 32
EPS = 1e-6
SCALE = HD ** -0.5
NCORES = 8
QH = NH // 4      # 8 q heads per core
KVH = NKV // 4    # 2 kv heads per core
QD = QH * HD      # 512 per-core q dim
KD = KVH * HD     # 128 per-core kv dim
KC = HID // 128   # 16 contraction chunks
SB = S // 512     # 4 sequence blocks of 512
BF16 = ml_dtypes.bfloat16

_CACHE = {}


def _build_bass(debug_dump=False):
    import concourse.bass as bass
    from concourse import bacc, mybir, tile

    f32 = mybir.dt.float32
    bf16 = mybir.dt.bfloat16

    nc = bacc.Bacc("TRN2", target_bir_lowering=False, debug=False,
                   enable_asserts=False, num_devices=NCORES)

    hT = nc.dram_tensor("hT", [HID, S], bf16, kind="ExternalInput").ap()
    wqT = nc.dram_tensor("wqT", [HID, QD], bf16, kind="ExternalInput").ap()
    wkT = nc.dram_tensor("wkT", [HID, KD], bf16, kind="ExternalInput").ap()
    wvT = nc.dram_tensor("wvT", [HID, KD], bf16, kind="ExternalInput").ap()
    wgT = nc.dram_tensor("wgT", [HID, QD], bf16, kind="ExternalInput").ap()
    woT = nc.dram_tensor("woT", [QD, HID], bf16, kind="ExternalInput").ap()
    csAq = nc.dram_tensor("csAq", [128, S], bf16, kind="ExternalInput").ap()
    csBq = nc.dram_tensor("csBq", [128, S], bf16, kind="ExternalInput").ap()
    csAk = nc.dram_tensor("csAk", [128, S], bf16, kind="ExternalInput").ap()
    csBk = nc.dram_tensor("csBk", [128, S], bf16, kind="ExternalInput").ap()
    sel2d = nc.dram_tensor("sel2", [2, 128], f32, kind="ExternalInput").ap()
    outT = nc.dram_tensor("outT", [HID, S], bf16, kind="ExternalOutput").ap()
    if debug_dump:
        dbg_q = nc.dram_tensor("dbg_q", [128, 4, S], bf16, kind="ExternalOutput").ap()
        dbg_k = nc.dram_tensor("dbg_k", [128, S], bf16, kind="ExternalOutput").ap()
        dbg_v = nc.dram_tensor("dbg_v", [128, KC, KVH, HD + 1], bf16,
                               kind="ExternalOutput").ap()
        dbg_g = nc.dram_tensor("dbg_g", [128, 4, S], bf16, kind="ExternalOutput").ap()
        dbg_og = nc.dram_tensor("dbg_og", [128, 4, S], bf16,
                                kind="ExternalOutput").ap()

    Exp = mybir.ActivationFunctionType.Exp
    Sigmoid = mybir.ActivationFunctionType.Sigmoid
    Square = mybir.ActivationFunctionType.Square
    Sqrt = mybir.ActivationFunctionType.Sqrt
    Copy = mybir.ActivationFunctionType.Copy
    PSUM = bass.MemorySpace.PSUM

    with tile.TileContext(nc) as tc:
        # ---- persistent sbuf ----
        with tc.tile_pool(name="persist", bufs=1) as pp:
            # head h lives at partition rows (h//4)*64 (matching its kv head's
            # rows so matmul operand bases agree), free-dim chunk h%4
            # triu[p, j] = 1.0 where p <= j (causal keep-mask for the
            # scoresT boundary strip), 0 above
            triu = pp.tile([128, 128], bf16)
            ones64 = pp.tile([1, 64], f32)
            qT_sb = pp.tile([128, 4, S], bf16)        # q (roped+normed)
            kT_sb = pp.tile([128, S], bf16)           # k (roped+normed)
            g_sb = pp.tile([128, 4, S], bf16)         # sigmoid(gate)
            v_sb = pp.tile([128, KC, KVH, HD + 1], bf16)  # natural V + ones col
            wo_sb = pp.tile([128, 4, KC, 128], bf16)
            og_sb = pp.tile([128, 4, S], bf16)        # raw gate, then gated out

            nc.sync.dma_start(out=wo_sb,
                              in_=woT.rearrange("(c p) (mb mm) -> p c mb mm",
                                                p=128, mm=128))
            nc.vector.memset(v_sb[:, :, :, HD:HD + 1], 1.0)

            # ================= phase 1: projections =================
            with tc.tile_pool(name="consts", bufs=1) as cp, \
                 tc.tile_pool(name="wts", bufs=1) as wp, \
                 tc.tile_pool(name="hblk", bufs=2) as hp, \
                 tc.tile_pool(name="work", bufs=3) as wk, \
                 tc.tile_pool(name="smallw", bufs=3) as smp, \
                 tc.tile_pool(name="rbpool", bufs=3) as rbp, \
                 tc.tile_pool(name="rwork", bufs=3) as rwk, \
                 tc.tile_pool(name="pps", bufs=2, space=PSUM) as pps, \
                 tc.tile_pool(name="sqps", bufs=2, space=PSUM) as sqps, \
                 tc.tile_pool(name="rbps", bufs=2, space=PSUM) as rbps, \
                 tc.tile_pool(name="trps", bufs=2, space=PSUM) as trps:

                csA_q = cp.tile([128, S], bf16)
                csB_q = cp.tile([128, S], bf16)
                csA_k = cp.tile([128, S], bf16)
                csB_k = cp.tile([128, S], bf16)
                nc.sync.dma_start(out=csA_q, in_=csAq)
                nc.sync.dma_start(out=csB_q, in_=csBq)
                nc.sync.dma_start(out=csA_k, in_=csAk)
                nc.sync.dma_start(out=csB_k, in_=csBk)
                ident = cp.tile([128, 128], bf16)
                from concourse.masks import make_identity, make_upper_triangular
                make_identity(nc, ident)
                make_upper_triangular(nc, triu, val=1.0, diag=True)
                ones2 = cp.tile([128, 2], bf16)
                nc.vector.memset(ones2, 0.0)
                nc.vector.memset(ones2[0:64, 0:1], 1.0)
                nc.vector.memset(ones2[64:128, 1:2], 1.0)
                # sel2.T broadcast: row p of (sel2.T @ rstd) = rstd[p//64]
                sel2 = cp.tile([2, 128], f32)
                nc.sync.dma_start(out=sel2, in_=sel2d)
                nc.vector.memset(ones64, 1.0)
                eps_t = cp.tile([8, 1], f32)
                nc.vector.memset(eps_t, EPS)

                wq_sb = wp.tile([128, KC, QD], bf16)
                wk_sb = wp.tile([128, KC, KD], bf16)
                wv_sb = wp.tile([128, KC, KD], bf16)
                wg_sb = wp.tile([128, KC, QD], bf16)
                nc.sync.dma_start(out=wq_sb,
                                  in_=wqT.rearrange("(c p) m -> p c m", p=128))
                nc.sync.dma_start(out=wk_sb,
                                  in_=wkT.rearrange("(c p) m -> p c m", p=128))
                nc.sync.dma_start(out=wv_sb,
                                  in_=wvT.rearrange("(c p) m -> p c m", p=128))
                nc.sync.dma_start(out=wg_sb,
                                  in_=wgT.rearrange("(c p) m -> p c m", p=128))

                def rope_norm(ps, csA, csB):
                    """ps: psum [128,512] raw proj. Returns (qa bf16 roped,
                    rb_s bf16 [128,512] rstd broadcast)."""
                    sq_t = wk.tile([128, 512], bf16, tag="sq")
                    nc.scalar.activation(out=sq_t, in_=ps, func=Square)
                    qb = rwk.tile([128, 512], bf16, tag="qb")
                    nc.scalar.activation(out=qb, in_=ps, func=Copy)
                    # sum of squares per 64-row half via matmul
                    sq_ps = sqps.tile([2, 512], f32, tag="sqs")
                    nc.tensor.matmul(sq_ps, ones2, sq_t, start=True, stop=True)
                    sstd = smp.tile([2, 512], f32, tag="sstd")
                    nc.scalar.activation(out=sstd, in_=sq_ps, func=Sqrt,
                                         scale=1.0 / HD, bias=eps_t[0:2])
                    rstd = smp.tile([2, 512], f32, tag="rstd")
                    nc.vector.reciprocal_approx_fast(out=rstd, in_=sstd)
                    # broadcast rstd rows to halves via matmul
                    rb_ps = rbps.tile([128, 512], f32, tag="rb")
                    nc.tensor.matmul(rb_ps, sel2, rstd, start=True, stop=True)
                    rb_s = rbp.tile([128, 512], bf16, tag="rbs")
                    nc.vector.tensor_copy(out=rb_s, in_=rb_ps)
                    # rope: qa = qb*csA + rot(qb)*csB
                    rot = rwk.tile([128, 512], bf16, tag="rot")
                    for hh in (0, 64):
                        # 16-row rotate needs non-32-aligned partition bases:
                        # only DMA can address those
                        nc.gpsimd.dma_start(out=rot[hh + 0:hh + 16],
                                            in_=qb[hh + 16:hh + 32])
                        nc.gpsimd.dma_start(out=rot[hh + 16:hh + 32],
                                            in_=qb[hh + 0:hh + 16])
                        nc.vector.tensor_copy(out=rot[hh + 32:hh + 64],
                                              in_=qb[hh + 32:hh + 64])
                    nc.vector.tensor_mul(rot, rot, csB)
                    qa = rwk.tile([128, 512], bf16, tag="qa")
                    nc.vector.tensor_mul(qa, qb, csA)
                    nc.vector.tensor_add(qa, qa, rot)
                    return qa, rb_s

                for sb in range(SB):
                    s0 = sb * 512
                    hblk = hp.tile([128, KC, 512], bf16)
                    nc.sync.dma_start(
                        out=hblk,
                        in_=hT[:, s0:s0 + 512].rearrange("(c p) s -> p c s",
                                                         p=128))
                    # ---- q projection (4 chunks of 128 rows) ----
                    for m in range(4):
                        ps = pps.tile([128, 512], f32, tag="proj")
                        for kc in range(KC):
                            nc.tensor.matmul(ps, wq_sb[:, kc, m * 128:(m + 1) * 128],
                                             hblk[:, kc, :],
                                             start=(kc == 0), stop=(kc == KC - 1))
                        qa, rb_s = rope_norm(ps, csA_q[:, s0:s0 + 512],
                                             csB_q[:, s0:s0 + 512])
                        # heads 2m, 2m+1 -> row-half r=m//2, chunks 2*(m%2)+{0,1}
                        r = (m // 2) * 64
                        cb = 2 * (m % 2)
                        nc.vector.tensor_mul(
                            qT_sb[r:r + 64, cb, s0:s0 + 512],
                            qa[0:64, :], rb_s[0:64, :])
                        nc.vector.tensor_mul(
                            qT_sb[r:r + 64, cb + 1, s0:s0 + 512],
                            qa[64:128, :], rb_s[64:128, :])
                    # ---- k projection (1 chunk) ----
                    ps = pps.tile([128, 512], f32, tag="proj")
                    for kc in range(KC):
                        nc.tensor.matmul(ps, wk_sb[:, kc, :], hblk[:, kc, :],
                                         start=(kc == 0), stop=(kc == KC - 1))
                    ka, rb_s = rope_norm(ps, csA_k[:, s0:s0 + 512],
                                         csB_k[:, s0:s0 + 512])
                    nc.vector.tensor_mul(kT_sb[:, s0:s0 + 512], ka, rb_s)
                    # ---- v projection + transpose to natural layout ----
                    ps = pps.tile([128, 512], f32, tag="proj")
                    for kc in range(KC):
                        nc.tensor.matmul(ps, wv_sb[:, kc, :], hblk[:, kc, :],
                                         start=(kc == 0), stop=(kc == KC - 1))
                    vt = wk.tile([128, 512], bf16, tag="vt")
                    nc.scalar.activation(out=vt, in_=ps, func=Copy)
                    for ss in range(4):
                        tp = trps.tile([128, 128], bf16, tag="tp")
                        nc.tensor.transpose(tp, vt[:, ss * 128:(ss + 1) * 128],
                                            ident)
                        chunk = sb * 4 + ss
                        nc.vector.tensor_copy(out=v_sb[:, chunk, :, 0:HD],
                                              in_=tp.rearrange("p (kv d) -> p kv d",
                                                               kv=2))
                    # ---- gate projection -> raw staged into og_sb ----
                    for m in range(4):
                        ps = pps.tile([128, 512], f32, tag="proj")
                        for kc in range(KC):
                            nc.tensor.matmul(ps, wg_sb[:, kc, m * 128:(m + 1) * 128],
                                             hblk[:, kc, :],
                                             start=(kc == 0), stop=(kc == KC - 1))
                        nc.scalar.activation(out=og_sb[:, m, s0:s0 + 512],
                                             in_=ps, func=Copy)

            # batched sigmoid: og_sb (raw gate) -> g_sb; one table switch
            for m in range(4):
                nc.scalar.activation(out=g_sb[:, m, :], in_=og_sb[:, m, :],
                                     func=Sigmoid)

            if debug_dump:
                nc.sync.dma_start(out=dbg_g, in_=g_sb)

            # ============ phase 2: attention + interleaved o_proj ============
            with tc.tile_pool(name="probs", bufs=4) as prp, \
                 tc.tile_pool(name="att_sm", bufs=4) as asm, \
                 tc.tile_pool(name="ostg", bufs=4) as ostg, \
                 tc.tile_pool(name="scps", bufs=2, space=PSUM) as scps, \
                 tc.tile_pool(name="avps", bufs=2, space=PSUM) as avps:

                def attn_pair(hp_idx, Q):
                    """Heads hA=hp_idx (kv0, rows 0:64) and hB=hp_idx+4 (kv1,
                    rows 64:128), both free-dim chunk hp_idx; q block Q."""
                    q0 = Q * 1024
                    nkc = 8 * (Q + 1)
                    hc = [hp_idx // 2, hp_idx // 2 + 2]   # og/g chunk per head
                    rr = (hp_idx % 2) * 64                # og/g row half
                    av_a = avps.tile([65, 2, 512], f32, tag="av")
                    av_b = avps.tile([65, 2, 512], f32, tag="av")
                    av = [av_a, av_b]
                    for kc in range(nkc):
                        s_c = max(0, kc * 128 - q0)   # first valid local col
                        pt = prp.tile([128, 2, 1024], bf16, tag="pt")
                        for lb in range(2):
                            lo = lb * 512
                            if s_c >= lo + 512:
                                continue
                            st = max(s_c, lo)
                            sc2 = scps.tile([128, 2, 512], f32, tag="sc")
                            for hh in range(2):  # interleave -> row-tiled pair
                                p0 = hh * 64
                                nc.tensor.matmul(
                                    sc2[:, hh, st - lo:512],
                                    kT_sb[p0:p0 + 64, kc * 128:(kc + 1) * 128],
                                    qT_sb[p0:p0 + 64, hp_idx,
                                          q0 + st:q0 + lo + 512],
                                    start=True, stop=True)
                            nc.scalar.activation(
                                out=pt[:, :, st:lo + 512],
                                in_=sc2[:, :, st - lo:512],
                                func=Exp, scale=SCALE)
                            if s_c >= lo and s_c < lo + 512 and kc * 128 >= q0:
                                # boundary strip: keep k<=q inside cols
                                # [s_c, s_c+128)
                                for hh in range(2):
                                    nc.vector.tensor_mul(
                                        pt[:, hh, s_c:s_c + 128],
                                        pt[:, hh, s_c:s_c + 128], triu)
                            last_kc = 4 * (2 * Q + lb) + 3
                            for hh in range(2):
                                nc.tensor.matmul(
                                    av[hh][:, lb, st - lo:512],
                                    v_sb[:, kc, hh, :],
                                    pt[:, hh, st:lo + 512],
                                    start=(kc == 0), stop=(kc == last_kc))
                    # normalize + gate -> og_sb
                    for hh in range(2):
                        # custom DVE op mishandles base_partition 64: stage the
                        # denominator row to a base-0 sbuf tile first
                        den = asm.tile([1, 1024], f32, tag="den")
                        nc.vector.tensor_copy(
                            out=den,
                            in_=av[hh][64:65, :, :].rearrange("p a b -> p (a b)"))
                        recip = asm.tile([1, 1024], f32, tag="recip")
                        nc.vector.reciprocal_approx_fast(out=recip, in_=den)
                        rbv_ps = scps.tile([64, 2, 512], f32, tag="sc")
                        for lb in range(2):
                            nc.tensor.matmul(rbv_ps[:, lb, :], ones64,
                                             recip[:, lb * 512:(lb + 1) * 512],
                                             start=True, stop=True)
                        for lb in range(2):
                            qsl = slice(q0 + lb * 512, q0 + (lb + 1) * 512)
                            rgv = asm.tile([64, 512], bf16, tag="rgv")
                            nc.vector.tensor_mul(
                                rgv, rbv_ps[:, lb, :],
                                g_sb[rr:rr + 64, hc[hh], qsl])
                            nc.vector.tensor_mul(
                                og_sb[rr:rr + 64, hc[hh], qsl],
                                av[hh][0:64, lb, :], rgv)

                def oproj(nb):
                    for m in range(KC):
                        po = scps.tile([128, 2, 512], f32, tag="sc")
                        for oc in range(4):
                            nc.tensor.matmul(po[:, 0, :], wo_sb[:, oc, m, :],
                                             og_sb[:, oc, nb * 512:(nb + 1) * 512],
                                             start=(oc == 0), stop=(oc == 3))
                        stg = ostg.tile([128, 512], bf16, tag="stg")
                        if m % 2 == 0:
                            nc.scalar.activation(out=stg, in_=po[:, 0, :],
                                                 func=Copy)
                        else:
                            nc.vector.tensor_copy(out=stg, in_=po[:, 0, :])
                        nc.sync.dma_start(
                            out=outT[m * 128:(m + 1) * 128,
                                     nb * 512:(nb + 1) * 512],
                            in_=stg)

                for hp_idx in range(4):
                    attn_pair(hp_idx, 0)
                for nb in (0, 1):
                    oproj(nb)
                for hp_idx in range(4):
                    attn_pair(hp_idx, 1)
                for nb in (2, 3):
                    oproj(nb)

            if debug_dump:
                nc.sync.dma_start(out=dbg_q, in_=qT_sb)
                nc.sync.dma_start(out=dbg_k, in_=kT_sb)
                nc.sync.dma_start(out=dbg_v, in_=v_sb)
                nc.sync.dma_start(out=dbg_og, in_=og_sb)

    nc.compile()
    return nc


def _host_prep(hidden_states, cos, sin, Wq, Wk, Wv, Wg, Wo, q_norm_w, k_norm_w):
    """Build per-core input maps."""
    def cs_tables(cos_b, sin_b, w):
        # csA/csB [128, S]: row p -> head-local dim d = p % 64
        A = np.empty((128, S), np.float32)
        Bt = np.empty((128, S), np.float32)
        cosT = cos_b.T  # [32, S]
        sinT = sin_b.T
        for blk in (0, 64):
            A[blk + 0:blk + 32] = cosT * w[0:32, None]
            A[blk + 32:blk + 64] = w[32:64, None]
            Bt[blk + 0:blk + 16] = -sinT[0:16] * w[16:32, None]
            Bt[blk + 16:blk + 32] = sinT[16:32] * w[0:16, None]
            Bt[blk + 32:blk + 64] = 0.0
        return A.astype(BF16), Bt.astype(BF16)

    sel2_host = np.zeros((2, 128), np.float32)
    sel2_host[0, 0:64] = 1.0
    sel2_host[1, 64:128] = 1.0
    in_maps = []
    for c in range(NCORES):
        b, g = c // 4, c % 4
        qs = slice(g * QD, (g + 1) * QD)
        ks = slice(g * KD, (g + 1) * KD)
        csA_q, csB_q = cs_tables(cos[b], sin[b], np.asarray(q_norm_w))
        csA_k, csB_k = cs_tables(cos[b], sin[b], np.asarray(k_norm_w))
        in_maps.append({
            "hT": np.ascontiguousarray(hidden_states[b].T).astype(BF16),
            "wqT": np.ascontiguousarray(Wq[qs].T).astype(BF16),
            "wkT": np.ascontiguousarray(Wk[ks].T).astype(BF16),
            "wvT": np.ascontiguousarray(Wv[ks].T).astype(BF16),
            "wgT": np.ascontiguousarray(Wg[qs].T).astype(BF16),
            "woT": np.ascontiguousarray(Wo[:, qs].T).astype(BF16),
            "csAq": csA_q, "csBq": csB_q, "csAk": csA_k, "csBk": csB_k,
            "sel2": sel2_host,
        })
    return in_maps


def kernel(hidden_states, cos, sin, Wq, Wk, Wv, Wg, Wo, q_norm_w, k_norm_w):
    from concourse import bass_utils

    if "nc" not in _CACHE:
        _CACHE["nc"] = _build_bass()
    nc = _CACHE["nc"]

    in_maps = _host_prep(hidden_states, cos, sin, Wq, Wk, Wv, Wg, Wo,
                         q_norm_w, k_norm_w)

    trace = bool(int(os.environ.get("KERNEL_TRACE", "0")))
    kwargs = {}
    if trace:
        # the agent image's antenv lacks axon_hooks; recreate it from the
        # boot helper so run_bass_kernel_spmd(trace=True) can NTFF-profile
        try:
            import antenv.axon_hooks  # noqa: F401
        except ImportError:
            import types
            sys.path.insert(0, "/root/.axon_site")
            from trn_agent_boot.trn_boot import _ntff_profile_via_ctypes
            hook = _ntff_profile_via_ctypes("/opt/axon/libaxon_pjrt.so")
            mod = types.ModuleType("antenv.axon_hooks")
            mod.get_axon_ntff_profile_hook = lambda: hook
            sys.modules["antenv.axon_hooks"] = mod
        tmpdir = os.environ.get("KERNEL_TRACE_DIR") or None
        kwargs = dict(trace=True, tmpdir=tmpdir)
    res = bass_utils.run_bass_kernel_spmd(nc, in_maps,
                                          core_ids=list(range(NCORES)),
                                          **kwargs)
    if trace and res.exec_time_ns is not None:
        print(f"HW exec time: {res.exec_time_ns} ns")
        _CACHE["exec_time_ns"] = res.exec_time_ns

    out = np.zeros((B, S, HID), np.float32)
    for c in range(NCORES):
        b = c // 4
        out[b] += res.results[c]["outT"].T.astype(np.float32)
    return out


if __name__ == "__main__":
    rng = np.random.default_rng(0)
    hs = rng.standard_normal((B, S, HID), dtype=np.float32)
    cos = rng.random((B, S, ROPE), dtype=np.float32)
    sin = rng.random((B, S, ROPE), dtype=np.float32)
    out = kernel(hidden_states=hs, cos=cos, sin=sin,
                 Wq=rng.standard_normal((NH * HD, HID), dtype=np.float32) * 0.02,
                 Wk=rng.standard_normal((NKV * HD, HID), dtype=np.float32) * 0.02,
                 Wv=rng.standard_normal((NKV * HD, HID), dtype=np.float32) * 0.02,
                 Wg=rng.standard_normal((NH * HD, HID), dtype=np.float32) * 0.02,
                 Wo=rng.standard_normal((HID, NH * HD), dtype=np.float32) * 0.02,
                 q_norm_w=np.ones(HD, np.float32),
                 k_norm_w=np.ones(HD, np.float32))
    print(out.shape, out.dtype)


# revision 19
# speedup vs baseline: 1.2884x; 1.2884x over previous
"""GQA attention Trainium2 kernel (8 NeuronCores, SPMD, no collectives).

Sharding: 2-way data parallel (batch) x 4-way tensor parallel (heads).
Core c handles batch b=c//4 and head-group g=c%4 (8 q heads, 2 kv heads).
Each core produces a partial o_proj output (transposed, [HID, S] bf16);
the host sums the 4 partials per batch (f32) and transposes back.

On-device layout is feature-major ("transposed"): hidden is passed as
hT=[HID,S], projections produce qT/kT/gateT=[dim,S], attention scores are
computed as scoresT=[s_k,s_q] so softmax-exp output feeds the PV matmul
directly (lhsT = natural-layout V with an appended ones column that yields
the softmax denominator in psum row 64).

v2 changes vs baseline:
- rstd via ACT Rsqrt + matmul broadcast (sel2 [2,128] lhsT) instead of
  Sqrt + DVE reciprocal + gpsimd partition_broadcast (reciprocal was
  4.8us/instr, 171us total).
- rope math in bf16 (DVE 2x mode).
- gates staged raw into og_sb, sigmoid batched after phase 1 (avoids
  ACT table-set thrash: rsqrt set resident through phase 1).
- attention processed as head PAIRS (kv0 head rows 0-63, kv1 head rows
  64-127) with interleaved K=64 score matmuls -> concurrent row-tiled
  execution on the PE array (2x score throughput).
- causal trimming at 128-col granularity for scores/exp/PV; boundary
  128x128 strip masked by a DVE tril multiply (replaces gpsimd
  affine_select).
- softmax 1/denom via DVE reciprocal_approx_fast + matmul broadcast.
- o_proj split per q-half and emitted between attention halves so the
  PE has work while ACT grinds exp.
- outT in bf16 (host accumulates partials in f32).
"""

import os
import sys
import numpy as np

for _p in ("/opt/trn_rl_repo", "/root/.axon_site/_ro/trn_rl_repo"):
    if os.path.isdir(_p) and _p not in sys.path:
        sys.path.insert(0, _p)

import ml_dtypes

B, S, HID = 2, 2048, 2048
NH, NKV, HD = 32, 8, 64
ROPE = 32
EPS = 1e-6
SCALE = HD ** -0.5
NCORES = 8
QH = NH // 4      # 8 q heads per core
KVH = NKV // 4    # 2 kv heads per core
QD = QH * HD      # 512 per-core q dim
KD = KVH * HD     # 128 per-core kv dim
KC = HID // 128   # 16 contraction chunks
SB = S // 512     # 4 sequence blocks of 512
BF16 = ml_dtypes.bfloat16

_CACHE = {}


def _build_bass(debug_dump=False):
    import concourse.bass as bass
    from concourse import bacc, mybir, tile

    f32 = mybir.dt.float32
    bf16 = mybir.dt.bfloat16

    nc = bacc.Bacc("TRN2", target_bir_lowering=False, debug=False,
                   enable_asserts=False, num_devices=NCORES)

    hT = nc.dram_tensor("hT", [HID, S], bf16, kind="ExternalInput").ap()
    wqT = nc.dram_tensor("wqT", [HID, QD], bf16, kind="ExternalInput").ap()
    wkT = nc.dram_tensor("wkT", [HID, KD], bf16, kind="ExternalInput").ap()
    wvT = nc.dram_tensor("wvT", [HID, KD], bf16, kind="ExternalInput").ap()
    wgT = nc.dram_tensor("wgT", [HID, QD], bf16, kind="ExternalInput").ap()
    woT = nc.dram_tensor("woT", [QD, HID], bf16, kind="ExternalInput").ap()
    csAq = nc.dram_tensor("csAq", [128, S], bf16, kind="ExternalInput").ap()
    csBq = nc.dram_tensor("csBq", [128, S], bf16, kind="ExternalInput").ap()
    csAk = nc.dram_tensor("csAk", [128, S], bf16, kind="ExternalInput").ap()
    csBk = nc.dram_tensor("csBk", [128, S], bf16, kind="ExternalInput").ap()
    sel2d = nc.dram_tensor("sel2", [2, 128], f32, kind="ExternalInput").ap()
    outT = nc.dram_tensor("outT", [HID, S], bf16, kind="ExternalOutput").ap()
    if debug_dump:
        dbg_q = nc.dram_tensor("dbg_q", [128, 4, S], bf16, kind="ExternalOutput").ap()
        dbg_k = nc.dram_tensor("dbg_k", [128, S], bf16, kind="ExternalOutput").ap()
        dbg_v = nc.dram_tensor("dbg_v", [128, KC, KVH, HD + 1], bf16,
                               kind="ExternalOutput").ap()
        dbg_g = nc.dram_tensor("dbg_g", [128, 4, S], bf16, kind="ExternalOutput").ap()
        dbg_og = nc.dram_tensor("dbg_og", [128, 4, S], bf16,
                                kind="ExternalOutput").ap()

    Exp = mybir.ActivationFunctionType.Exp
    Sigmoid = mybir.ActivationFunctionType.Sigmoid
    Square = mybir.ActivationFunctionType.Square
    Sqrt = mybir.ActivationFunctionType.Sqrt
    Copy = mybir.ActivationFunctionType.Copy
    PSUM = bass.MemorySpace.PSUM

    with tile.TileContext(nc) as tc:
        # ---- persistent sbuf ----
        with tc.tile_pool(name="persist", bufs=1) as pp:
            # head h lives at partition rows (h//4)*64 (matching its kv head's
            # rows so matmul operand bases agree), free-dim chunk h%4
            # triu[p, j] = 1.0 where p <= j (causal keep-mask for the
            # scoresT boundary strip), 0 above
            triu = pp.tile([128, 128], bf16)
            qT_sb = pp.tile([128, 4, S], bf16)        # q (roped+normed)
            kT_sb = pp.tile([128, S], bf16)           # k (roped+normed)
            g_sb = pp.tile([128, 4, S], bf16)         # sigmoid(gate)
            v_sb = pp.tile([128, KC, KVH, HD + 1], bf16)  # natural V + ones col
            wo_sb = pp.tile([128, 4, KC, 128], bf16)
            og_sb = pp.tile([128, 4, S], bf16)        # raw gate, then gated out

            nc.sync.dma_start(out=wo_sb,
                              in_=woT.rearrange("(c p) (mb mm) -> p c mb mm",
                                                p=128, mm=128))
            nc.vector.memset(v_sb[:, :, :, HD:HD + 1], 1.0)

            # ================= phase 1: projections =================
            with tc.tile_pool(name="consts", bufs=1) as cp, \
                 tc.tile_pool(name="wts", bufs=1) as wp, \
                 tc.tile_pool(name="hblk", bufs=2) as hp, \
                 tc.tile_pool(name="work", bufs=3) as wk, \
                 tc.tile_pool(name="smallw", bufs=3) as smp, \
                 tc.tile_pool(name="rbpool", bufs=3) as rbp, \
                 tc.tile_pool(name="rwork", bufs=3) as rwk, \
                 tc.tile_pool(name="pps", bufs=2, space=PSUM) as pps, \
                 tc.tile_pool(name="sqps", bufs=2, space=PSUM) as sqps, \
                 tc.tile_pool(name="rbps", bufs=2, space=PSUM) as rbps, \
                 tc.tile_pool(name="trps", bufs=2, space=PSUM) as trps:

                csA_q = cp.tile([128, S], bf16)
                csB_q = cp.tile([128, S], bf16)
                csA_k = cp.tile([128, S], bf16)
                csB_k = cp.tile([128, S], bf16)
                nc.sync.dma_start(out=csA_q, in_=csAq)
                nc.sync.dma_start(out=csB_q, in_=csBq)
                nc.sync.dma_start(out=csA_k, in_=csAk)
                nc.sync.dma_start(out=csB_k, in_=csBk)
                ident = cp.tile([128, 128], bf16)
                from concourse.masks import make_identity, make_upper_triangular
                make_identity(nc, ident)
                make_upper_triangular(nc, triu, val=1.0, diag=True)
                ones2 = cp.tile([128, 2], bf16)
                nc.vector.memset(ones2, 0.0)
                nc.vector.memset(ones2[0:64, 0:1], 1.0)
                nc.vector.memset(ones2[64:128, 1:2], 1.0)
                # sel2.T broadcast: row p of (sel2.T @ rstd) = rstd[p//64]
                sel2 = cp.tile([2, 128], f32)
                nc.sync.dma_start(out=sel2, in_=sel2d)
                eps_t = cp.tile([8, 1], f32)
                nc.vector.memset(eps_t, EPS)

                wq_sb = wp.tile([128, KC, QD], bf16)
                wk_sb = wp.tile([128, KC, KD], bf16)
                wv_sb = wp.tile([128, KC, KD], bf16)
                wg_sb = wp.tile([128, KC, QD], bf16)
                nc.sync.dma_start(out=wq_sb,
                                  in_=wqT.rearrange("(c p) m -> p c m", p=128))
                nc.sync.dma_start(out=wk_sb,
                                  in_=wkT.rearrange("(c p) m -> p c m", p=128))
                nc.sync.dma_start(out=wv_sb,
                                  in_=wvT.rearrange("(c p) m -> p c m", p=128))
                nc.sync.dma_start(out=wg_sb,
                                  in_=wgT.rearrange("(c p) m -> p c m", p=128))

                def rope_norm(ps, csA, csB):
                    """ps: psum [128,512] raw proj. Returns (qa bf16 roped,
                    rb_s bf16 [128,512] rstd broadcast)."""
                    sq_t = wk.tile([128, 512], bf16, tag="sq")
                    nc.scalar.activation(out=sq_t, in_=ps, func=Square)
                    qb = rwk.tile([128, 512], bf16, tag="qb")
                    nc.scalar.activation(out=qb, in_=ps, func=Copy)
                    # sum of squares per 64-row half via matmul
                    sq_ps = sqps.tile([2, 512], f32, tag="sqs")
                    nc.tensor.matmul(sq_ps, ones2, sq_t, start=True, stop=True)
                    sstd = smp.tile([2, 512], f32, tag="sstd")
                    nc.scalar.activation(out=sstd, in_=sq_ps, func=Sqrt,
                                         scale=1.0 / HD, bias=eps_t[0:2])
                    rstd = smp.tile([2, 512], f32, tag="rstd")
                    nc.vector.reciprocal_approx_fast(out=rstd, in_=sstd)
                    # broadcast rstd rows to halves via matmul
                    rb_ps = rbps.tile([128, 512], f32, tag="rb")
                    nc.tensor.matmul(rb_ps, sel2, rstd, start=True, stop=True)
                    rb_s = rbp.tile([128, 512], bf16, tag="rbs")
                    nc.vector.tensor_copy(out=rb_s, in_=rb_ps)
                    # rope: qa = qb*csA + rot(qb)*csB
                    rot = rwk.tile([128, 512], bf16, tag="rot")
                    for hh in (0, 64):
                        # 16-row rotate needs non-32-aligned partition bases:
                        # only DMA can address those
                        nc.gpsimd.dma_start(out=rot[hh + 0:hh + 16],
                                            in_=qb[hh + 16:hh + 32])
                        nc.gpsimd.dma_start(out=rot[hh + 16:hh + 32],
                                            in_=qb[hh + 0:hh + 16])
                        nc.vector.tensor_copy(out=rot[hh + 32:hh + 64],
                                              in_=qb[hh + 32:hh + 64])
                    nc.vector.tensor_mul(rot, rot, csB)
                    qa = rwk.tile([128, 512], bf16, tag="qa")
                    nc.vector.tensor_mul(qa, qb, csA)
                    nc.vector.tensor_add(qa, qa, rot)
                    return qa, rb_s

                for sb in range(SB):
                    s0 = sb * 512
                    hblk = hp.tile([128, KC, 512], bf16)
                    nc.sync.dma_start(
                        out=hblk,
                        in_=hT[:, s0:s0 + 512].rearrange("(c p) s -> p c s",
                                                         p=128))
                    # ---- q projection (4 chunks of 128 rows) ----
                    for m in range(4):
                        ps = pps.tile([128, 512], f32, tag="proj")
                        for kc in range(KC):
                            nc.tensor.matmul(ps, wq_sb[:, kc, m * 128:(m + 1) * 128],
                                             hblk[:, kc, :],
                                             start=(kc == 0), stop=(kc == KC - 1))
                        qa, rb_s = rope_norm(ps, csA_q[:, s0:s0 + 512],
                                             csB_q[:, s0:s0 + 512])
                        # heads 2m, 2m+1 -> row-half r=m//2, chunks 2*(m%2)+{0,1}
                        r = (m // 2) * 64
                        cb = 2 * (m % 2)
                        nc.vector.tensor_mul(
                            qT_sb[r:r + 64, cb, s0:s0 + 512],
                            qa[0:64, :], rb_s[0:64, :])
                        nc.vector.tensor_mul(
                            qT_sb[r:r + 64, cb + 1, s0:s0 + 512],
                            qa[64:128, :], rb_s[64:128, :])
                    # ---- k projection (1 chunk) ----
                    ps = pps.tile([128, 512], f32, tag="proj")
                    for kc in range(KC):
                        nc.tensor.matmul(ps, wk_sb[:, kc, :], hblk[:, kc, :],
                                         start=(kc == 0), stop=(kc == KC - 1))
                    ka, rb_s = rope_norm(ps, csA_k[:, s0:s0 + 512],
                                         csB_k[:, s0:s0 + 512])
                    nc.vector.tensor_mul(kT_sb[:, s0:s0 + 512], ka, rb_s)
                    # ---- v projection + transpose to natural layout ----
                    ps = pps.tile([128, 512], f32, tag="proj")
                    for kc in range(KC):
                        nc.tensor.matmul(ps, wv_sb[:, kc, :], hblk[:, kc, :],
                                         start=(kc == 0), stop=(kc == KC - 1))
                    vt = wk.tile([128, 512], bf16, tag="vt")
                    nc.scalar.activation(out=vt, in_=ps, func=Copy)
                    for ss in range(4):
                        tp = trps.tile([128, 128], bf16, tag="tp")
                        nc.tensor.transpose(tp, vt[:, ss * 128:(ss + 1) * 128],
                                            ident)
                        chunk = sb * 4 + ss
                        nc.vector.tensor_copy(out=v_sb[:, chunk, :, 0:HD],
                                              in_=tp.rearrange("p (kv d) -> p kv d",
                                                               kv=2))
                    # ---- gate projection -> raw staged into og_sb ----
                    for m in range(4):
                        ps = pps.tile([128, 512], f32, tag="proj")
                        for kc in range(KC):
                            nc.tensor.matmul(ps, wg_sb[:, kc, m * 128:(m + 1) * 128],
                                             hblk[:, kc, :],
                                             start=(kc == 0), stop=(kc == KC - 1))
                        nc.scalar.activation(out=og_sb[:, m, s0:s0 + 512],
                                             in_=ps, func=Copy)

            # batched sigmoid: og_sb (raw gate) -> g_sb; one table switch
            for m in range(4):
                nc.scalar.activation(out=g_sb[:, m, :], in_=og_sb[:, m, :],
                                     func=Sigmoid)

            if debug_dump:
                nc.sync.dma_start(out=dbg_g, in_=g_sb)

            # ============ phase 2: attention + interleaved o_proj ============
            with tc.tile_pool(name="probs", bufs=6) as prp, \
                 tc.tile_pool(name="att_sm", bufs=4) as asm, \
                 tc.tile_pool(name="ostg", bufs=4) as ostg, \
                 tc.tile_pool(name="scps", bufs=2, space=PSUM) as scps, \
                 tc.tile_pool(name="avps", bufs=2, space=PSUM) as avps:

                def attn_pair(hp_idx, Q):
                    """Heads hA=hp_idx (kv0, rows 0:64) and hB=hp_idx+4 (kv1,
                    rows 64:128), both free-dim chunk hp_idx; q block Q."""
                    q0 = Q * 1024
                    nkc = 8 * (Q + 1)
                    hc = [hp_idx // 2, hp_idx // 2 + 2]   # og/g chunk per head
                    rr = (hp_idx % 2) * 64                # og/g row half
                    av_a = avps.tile([65, 2, 512], f32, tag="av")
                    av_b = avps.tile([65, 2, 512], f32, tag="av")
                    av = [av_a, av_b]
                    for kc in range(nkc):
                        s_c = max(0, kc * 128 - q0)   # first valid local col
                        for lb in range(2):
                            lo = lb * 512
                            if s_c >= lo + 512:
                                continue
                            st = max(s_c, lo)
                            sc2 = scps.tile([128, 2, 512], f32, tag="sc")
                            for hh in range(2):  # interleave -> row-tiled pair
                                p0 = hh * 64
                                nc.tensor.matmul(
                                    sc2[:, hh, st - lo:512],
                                    kT_sb[p0:p0 + 64, kc * 128:(kc + 1) * 128],
                                    qT_sb[p0:p0 + 64, hp_idx,
                                          q0 + st:q0 + lo + 512],
                                    start=True, stop=True)
                            # probs tile per (kc, lb): [*, head, col-in-block]
                            ptl = prp.tile([128, 2, 512], bf16, tag="ptl")
                            if st == lo:
                                # full slot: contiguous 1-region exp
                                nc.scalar.activation(
                                    out=ptl.rearrange("p a b -> p (a b)"),
                                    in_=sc2.rearrange("p a b -> p (a b)"),
                                    func=Exp, scale=SCALE)
                            else:
                                loc = st - lo
                                for hh in range(2):
                                    nc.scalar.activation(
                                        out=ptl[:, hh, loc:512],
                                        in_=sc2[:, hh, loc:512],
                                        func=Exp, scale=SCALE)
                            if lo <= s_c < lo + 512 and kc * 128 >= q0:
                                # boundary strip: keep k<=q inside cols
                                # [s_c, s_c+128) (local [s_c-lo, s_c-lo+128))
                                sl = s_c - lo
                                for hh in range(2):
                                    nc.vector.tensor_mul(
                                        ptl[:, hh, sl:sl + 128],
                                        ptl[:, hh, sl:sl + 128], triu)
                            last_kc = 4 * (2 * Q + lb) + 3
                            for hh in range(2):
                                nc.tensor.matmul(
                                    av[hh][:, lb, st - lo:512],
                                    v_sb[:, kc, hh, :],
                                    ptl[:, hh, st - lo:512],
                                    start=(kc == 0), stop=(kc == last_kc))
                    # normalize + gate -> og_sb
                    for hh in range(2):
                        # custom DVE op mishandles base_partition 64: stage the
                        # denominator row to a base-0 sbuf tile first
                        den = asm.tile([1, 1024], f32, tag="den")
                        nc.vector.tensor_copy(
                            out=den,
                            in_=av[hh][64:65, :, :].rearrange("p a b -> p (a b)"))
                        recip = asm.tile([1, 1024], f32, tag="recip")
                        nc.vector.reciprocal_approx_fast(out=recip, in_=den)
                        rbv = asm.tile([64, 1024], f32, tag="rbv")
                        nc.gpsimd.partition_broadcast(rbv, recip)
                        for lb in range(2):
                            qsl = slice(q0 + lb * 512, q0 + (lb + 1) * 512)
                            dst = og_sb[rr:rr + 64, hc[hh], qsl]
                            nc.vector.tensor_mul(
                                dst, av[hh][0:64, lb, :],
                                rbv[:, lb * 512:(lb + 1) * 512])
                            nc.vector.tensor_mul(
                                dst, dst, g_sb[rr:rr + 64, hc[hh], qsl])

                def oproj(nb, tail):
                    for m in range(KC):
                        po = scps.tile([128, 2, 512], f32, tag="sc")
                        for oc in range(4):
                            nc.tensor.matmul(po[:, 0, :], wo_sb[:, oc, m, :],
                                             og_sb[:, oc, nb * 512:(nb + 1) * 512],
                                             start=(oc == 0), stop=(oc == 3))
                        stg = ostg.tile([128, 512], bf16, tag="stg")
                        if tail and m % 2 == 0:
                            # tail oproj: ACT is idle (exp done) -> share copies
                            nc.scalar.activation(out=stg, in_=po[:, 0, :],
                                                 func=Copy)
                        else:
                            nc.vector.tensor_copy(out=stg, in_=po[:, 0, :])
                        nc.sync.dma_start(
                            out=outT[m * 128:(m + 1) * 128,
                                     nb * 512:(nb + 1) * 512],
                            in_=stg)

                for hp_idx in range(4):
                    attn_pair(hp_idx, 0)
                for hp_idx in range(4):
                    attn_pair(hp_idx, 1)
                # oproj 0/1 deps (all Q0 og) are ready when Q1 starts: the
                # scheduler uses these matmuls to fill Q1's exp-bound cycles
                for nb in (0, 1):
                    oproj(nb, tail=False)
                for nb in (2, 3):
                    oproj(nb, tail=True)

            if debug_dump:
                nc.sync.dma_start(out=dbg_q, in_=qT_sb)
                nc.sync.dma_start(out=dbg_k, in_=kT_sb)
                nc.sync.dma_start(out=dbg_v, in_=v_sb)
                nc.sync.dma_start(out=dbg_og, in_=og_sb)

    nc.compile()
    return nc


def _host_prep(hidden_states, cos, sin, Wq, Wk, Wv, Wg, Wo, q_norm_w, k_norm_w):
    """Build per-core input maps."""
    def cs_tables(cos_b, sin_b, w):
        # csA/csB [128, S]: row p -> head-local dim d = p % 64
        A = np.empty((128, S), np.float32)
        Bt = np.empty((128, S), np.float32)
        cosT = cos_b.T  # [32, S]
        sinT = sin_b.T
        for blk in (0, 64):
            A[blk + 0:blk + 32] = cosT * w[0:32, None]
            A[blk + 32:blk + 64] = w[32:64, None]
            Bt[blk + 0:blk + 16] = -sinT[0:16] * w[16:32, None]
            Bt[blk + 16:blk + 32] = sinT[16:32] * w[0:16, None]
            Bt[blk + 32:blk + 64] = 0.0
        return A.astype(BF16), Bt.astype(BF16)

    sel2_host = np.zeros((2, 128), np.float32)
    sel2_host[0, 0:64] = 1.0
    sel2_host[1, 64:128] = 1.0
    in_maps = []
    for c in range(NCORES):
        b, g = c // 4, c % 4
        qs = slice(g * QD, (g + 1) * QD)
        ks = slice(g * KD, (g + 1) * KD)
        csA_q, csB_q = cs_tables(cos[b], sin[b], np.asarray(q_norm_w))
        csA_k, csB_k = cs_tables(cos[b], sin[b], np.asarray(k_norm_w))
        in_maps.append({
            "hT": np.ascontiguousarray(hidden_states[b].T).astype(BF16),
            "wqT": np.ascontiguousarray(Wq[qs].T).astype(BF16),
            "wkT": np.ascontiguousarray(Wk[ks].T).astype(BF16),
            "wvT": np.ascontiguousarray(Wv[ks].T).astype(BF16),
            "wgT": np.ascontiguousarray(Wg[qs].T).astype(BF16),
            "woT": np.ascontiguousarray(Wo[:, qs].T).astype(BF16),
            "csAq": csA_q, "csBq": csB_q, "csAk": csA_k, "csBk": csB_k,
            "sel2": sel2_host,
        })
    return in_maps


def kernel(hidden_states, cos, sin, Wq, Wk, Wv, Wg, Wo, q_norm_w, k_norm_w):
    from concourse import bass_utils

    if "nc" not in _CACHE:
        _CACHE["nc"] = _build_bass()
    nc = _CACHE["nc"]

    in_maps = _host_prep(hidden_states, cos, sin, Wq, Wk, Wv, Wg, Wo,
                         q_norm_w, k_norm_w)

    trace = bool(int(os.environ.get("KERNEL_TRACE", "0")))
    kwargs = {}
    if trace:
        # the agent image's antenv lacks axon_hooks; recreate it from the
        # boot helper so run_bass_kernel_spmd(trace=True) can NTFF-profile
        try:
            import antenv.axon_hooks  # noqa: F401
        except ImportError:
            import types
            sys.path.insert(0, "/root/.axon_site")
            from trn_agent_boot.trn_boot import _ntff_profile_via_ctypes
            hook = _ntff_profile_via_ctypes("/opt/axon/libaxon_pjrt.so")
            mod = types.ModuleType("antenv.axon_hooks")
            mod.get_axon_ntff_profile_hook = lambda: hook
            sys.modules["antenv.axon_hooks"] = mod
        tmpdir = os.environ.get("KERNEL_TRACE_DIR") or None
        kwargs = dict(trace=True, tmpdir=tmpdir)
    res = bass_utils.run_bass_kernel_spmd(nc, in_maps,
                                          core_ids=list(range(NCORES)),
                                          **kwargs)
    if trace and res.exec_time_ns is not None:
        print(f"HW exec time: {res.exec_time_ns} ns")
        _CACHE["exec_time_ns"] = res.exec_time_ns

    out = np.zeros((B, S, HID), np.float32)
    for c in range(NCORES):
        b = c // 4
        out[b] += res.results[c]["outT"].T.astype(np.float32)
    return out


if __name__ == "__main__":
    rng = np.random.default_rng(0)
    hs = rng.standard_normal((B, S, HID), dtype=np.float32)
    cos = rng.random((B, S, ROPE), dtype=np.float32)
    sin = rng.random((B, S, ROPE), dtype=np.float32)
    out = kernel(hidden_states=hs, cos=cos, sin=sin,
                 Wq=rng.standard_normal((NH * HD, HID), dtype=np.float32) * 0.02,
                 Wk=rng.standard_normal((NKV * HD, HID), dtype=np.float32) * 0.02,
                 Wv=rng.standard_normal((NKV * HD, HID), dtype=np.float32) * 0.02,
                 Wg=rng.standard_normal((NH * HD, HID), dtype=np.float32) * 0.02,
                 Wo=rng.standard_normal((HID, NH * HD), dtype=np.float32) * 0.02,
                 q_norm_w=np.ones(HD, np.float32),
                 k_norm_w=np.ones(HD, np.float32))
    print(out.shape, out.dtype)


# revision 20
# speedup vs baseline: 1.3207x; 1.0251x over previous
"""GQA attention Trainium2 kernel (8 NeuronCores, SPMD, no collectives).

Sharding: 2-way data parallel (batch) x 4-way tensor parallel (heads).
Core c handles batch b=c//4 and head-group g=c%4 (8 q heads, 2 kv heads).
Each core produces a partial o_proj output (transposed, [HID, S] bf16);
the host sums the 4 partials per batch (f32) and transposes back.

On-device layout is feature-major ("transposed"): hidden is passed as
hT=[HID,S], projections produce qT/kT/gateT=[dim,S], attention scores are
computed as scoresT=[s_k,s_q] so softmax-exp output feeds the PV matmul
directly (lhsT = natural-layout V with an appended ones column that yields
the softmax denominator in psum row 64).

v2 changes vs baseline:
- rstd via ACT Rsqrt + matmul broadcast (sel2 [2,128] lhsT) instead of
  Sqrt + DVE reciprocal + gpsimd partition_broadcast (reciprocal was
  4.8us/instr, 171us total).
- rope math in bf16 (DVE 2x mode).
- gates staged raw into og_sb, sigmoid batched after phase 1 (avoids
  ACT table-set thrash: rsqrt set resident through phase 1).
- attention processed as head PAIRS (kv0 head rows 0-63, kv1 head rows
  64-127) with interleaved K=64 score matmuls -> concurrent row-tiled
  execution on the PE array (2x score throughput).
- causal trimming at 128-col granularity for scores/exp/PV; boundary
  128x128 strip masked by a DVE tril multiply (replaces gpsimd
  affine_select).
- softmax 1/denom via DVE reciprocal_approx_fast + matmul broadcast.
- o_proj split per q-half and emitted between attention halves so the
  PE has work while ACT grinds exp.
- outT in bf16 (host accumulates partials in f32).
"""

import os
import sys
import numpy as np

for _p in ("/opt/trn_rl_repo", "/root/.axon_site/_ro/trn_rl_repo"):
    if os.path.isdir(_p) and _p not in sys.path:
        sys.path.insert(0, _p)

import ml_dtypes

B, S, HID = 2, 2048, 2048
NH, NKV, HD = 32, 8, 64
ROPE = 32
EPS = 1e-6
SCALE = HD ** -0.5
NCORES = 8
QH = NH // 4      # 8 q heads per core
KVH = NKV // 4    # 2 kv heads per core
QD = QH * HD      # 512 per-core q dim
KD = KVH * HD     # 128 per-core kv dim
KC = HID // 128   # 16 contraction chunks
SB = S // 512     # 4 sequence blocks of 512
BF16 = ml_dtypes.bfloat16

_CACHE = {}


def _build_bass(debug_dump=False):
    import concourse.bass as bass
    from concourse import bacc, mybir, tile

    f32 = mybir.dt.float32
    bf16 = mybir.dt.bfloat16

    nc = bacc.Bacc("TRN2", target_bir_lowering=False, debug=False,
                   enable_asserts=False, num_devices=NCORES)

    hT = nc.dram_tensor("hT", [HID, S], bf16, kind="ExternalInput").ap()
    wqT = nc.dram_tensor("wqT", [HID, QD], bf16, kind="ExternalInput").ap()
    wkT = nc.dram_tensor("wkT", [HID, KD], bf16, kind="ExternalInput").ap()
    wvT = nc.dram_tensor("wvT", [HID, KD], bf16, kind="ExternalInput").ap()
    wgT = nc.dram_tensor("wgT", [HID, QD], bf16, kind="ExternalInput").ap()
    woT = nc.dram_tensor("woT", [QD, HID], bf16, kind="ExternalInput").ap()
    csAq = nc.dram_tensor("csAq", [128, S], bf16, kind="ExternalInput").ap()
    csBq = nc.dram_tensor("csBq", [128, S], bf16, kind="ExternalInput").ap()
    csAk = nc.dram_tensor("csAk", [128, S], bf16, kind="ExternalInput").ap()
    csBk = nc.dram_tensor("csBk", [128, S], bf16, kind="ExternalInput").ap()
    sel2d = nc.dram_tensor("sel2", [2, 128], f32, kind="ExternalInput").ap()
    outT = nc.dram_tensor("outT", [HID, S], bf16, kind="ExternalOutput").ap()
    if debug_dump:
        dbg_q = nc.dram_tensor("dbg_q", [128, 4, S], bf16, kind="ExternalOutput").ap()
        dbg_k = nc.dram_tensor("dbg_k", [128, S], bf16, kind="ExternalOutput").ap()
        dbg_v = nc.dram_tensor("dbg_v", [128, KC, KVH, HD + 1], bf16,
                               kind="ExternalOutput").ap()
        dbg_g = nc.dram_tensor("dbg_g", [128, 4, S], bf16, kind="ExternalOutput").ap()
        dbg_og = nc.dram_tensor("dbg_og", [128, 4, S], bf16,
                                kind="ExternalOutput").ap()

    Exp = mybir.ActivationFunctionType.Exp
    Sigmoid = mybir.ActivationFunctionType.Sigmoid
    Square = mybir.ActivationFunctionType.Square
    Sqrt = mybir.ActivationFunctionType.Sqrt
    Copy = mybir.ActivationFunctionType.Copy
    PSUM = bass.MemorySpace.PSUM

    with tile.TileContext(nc) as tc:
        # ---- persistent sbuf ----
        with tc.tile_pool(name="persist", bufs=1) as pp:
            # head h lives at partition rows (h//4)*64 (matching its kv head's
            # rows so matmul operand bases agree), free-dim chunk h%4
            # triu[p, j] = 1.0 where p <= j (causal keep-mask for the
            # scoresT boundary strip), 0 above
            triu = pp.tile([128, 128], bf16)
            qT_sb = pp.tile([128, 4, S], bf16)        # q (roped+normed)
            kT_sb = pp.tile([128, S], bf16)           # k (roped+normed)
            g_sb = pp.tile([128, 4, S], bf16)         # sigmoid(gate)
            v_sb = pp.tile([128, KC, KVH, HD + 1], bf16)  # natural V + ones col
            wo_sb = pp.tile([128, 4, KC, 128], bf16)
            og_sb = pp.tile([128, 4, S], bf16)        # raw gate, then gated out

            nc.sync.dma_start(out=wo_sb,
                              in_=woT.rearrange("(c p) (mb mm) -> p c mb mm",
                                                p=128, mm=128))
            nc.vector.memset(v_sb[:, :, :, HD:HD + 1], 1.0)

            # ================= phase 1: projections =================
            with tc.tile_pool(name="consts", bufs=1) as cp, \
                 tc.tile_pool(name="wts", bufs=1) as wp, \
                 tc.tile_pool(name="hblk", bufs=2) as hp, \
                 tc.tile_pool(name="work", bufs=3) as wk, \
                 tc.tile_pool(name="smallw", bufs=3) as smp, \
                 tc.tile_pool(name="rbpool", bufs=3) as rbp, \
                 tc.tile_pool(name="rwork", bufs=3) as rwk, \
                 tc.tile_pool(name="pps", bufs=2, space=PSUM) as pps, \
                 tc.tile_pool(name="sqps", bufs=2, space=PSUM) as sqps, \
                 tc.tile_pool(name="rbps", bufs=2, space=PSUM) as rbps, \
                 tc.tile_pool(name="trps", bufs=2, space=PSUM) as trps:

                csA_q = cp.tile([128, S], bf16)
                csB_q = cp.tile([128, S], bf16)
                csA_k = cp.tile([128, S], bf16)
                csB_k = cp.tile([128, S], bf16)
                nc.sync.dma_start(out=csA_q, in_=csAq)
                nc.sync.dma_start(out=csB_q, in_=csBq)
                nc.sync.dma_start(out=csA_k, in_=csAk)
                nc.sync.dma_start(out=csB_k, in_=csBk)
                ident = cp.tile([128, 128], bf16)
                from concourse.masks import make_identity, make_upper_triangular
                make_identity(nc, ident)
                make_upper_triangular(nc, triu, val=1.0, diag=True)
                ones2 = cp.tile([128, 2], bf16)
                nc.vector.memset(ones2, 0.0)
                nc.vector.memset(ones2[0:64, 0:1], 1.0)
                nc.vector.memset(ones2[64:128, 1:2], 1.0)
                # sel2.T broadcast: row p of (sel2.T @ rstd) = rstd[p//64]
                sel2 = cp.tile([2, 128], f32)
                nc.sync.dma_start(out=sel2, in_=sel2d)
                eps_t = cp.tile([8, 1], f32)
                nc.vector.memset(eps_t, EPS)

                wq_sb = wp.tile([128, KC, QD], bf16)
                wk_sb = wp.tile([128, KC, KD], bf16)
                wv_sb = wp.tile([128, KC, KD], bf16)
                wg_sb = wp.tile([128, KC, QD], bf16)
                nc.sync.dma_start(out=wq_sb,
                                  in_=wqT.rearrange("(c p) m -> p c m", p=128))
                nc.sync.dma_start(out=wk_sb,
                                  in_=wkT.rearrange("(c p) m -> p c m", p=128))
                nc.sync.dma_start(out=wv_sb,
                                  in_=wvT.rearrange("(c p) m -> p c m", p=128))
                nc.sync.dma_start(out=wg_sb,
                                  in_=wgT.rearrange("(c p) m -> p c m", p=128))

                def rope_norm(ps, csA, csB):
                    """ps: psum [128,512] raw proj. Returns (qa bf16 roped,
                    rb_s bf16 [128,512] rstd broadcast)."""
                    sq_t = wk.tile([128, 512], bf16, tag="sq")
                    nc.scalar.activation(out=sq_t, in_=ps, func=Square)
                    qb = rwk.tile([128, 512], bf16, tag="qb")
                    nc.scalar.activation(out=qb, in_=ps, func=Copy)
                    # sum of squares per 64-row half via matmul
                    sq_ps = sqps.tile([2, 512], f32, tag="sqs")
                    nc.tensor.matmul(sq_ps, ones2, sq_t, start=True, stop=True)
                    sstd = smp.tile([2, 512], f32, tag="sstd")
                    nc.scalar.activation(out=sstd, in_=sq_ps, func=Sqrt,
                                         scale=1.0 / HD, bias=eps_t[0:2])
                    rstd = smp.tile([2, 512], f32, tag="rstd")
                    nc.vector.reciprocal_approx_fast(out=rstd, in_=sstd)
                    # broadcast rstd rows to halves via matmul
                    rb_ps = rbps.tile([128, 512], f32, tag="rb")
                    nc.tensor.matmul(rb_ps, sel2, rstd, start=True, stop=True)
                    rb_s = rbp.tile([128, 512], bf16, tag="rbs")
                    nc.vector.tensor_copy(out=rb_s, in_=rb_ps)
                    # rope: qa = qb*csA + rot(qb)*csB
                    rot = rwk.tile([128, 512], bf16, tag="rot")
                    for hh in (0, 64):
                        # 16-row rotate needs non-32-aligned partition bases:
                        # only DMA can address those
                        nc.gpsimd.dma_start(out=rot[hh + 0:hh + 16],
                                            in_=qb[hh + 16:hh + 32])
                        nc.gpsimd.dma_start(out=rot[hh + 16:hh + 32],
                                            in_=qb[hh + 0:hh + 16])
                        nc.vector.tensor_copy(out=rot[hh + 32:hh + 64],
                                              in_=qb[hh + 32:hh + 64])
                    nc.vector.tensor_mul(rot, rot, csB)
                    qa = rwk.tile([128, 512], bf16, tag="qa")
                    nc.vector.tensor_mul(qa, qb, csA)
                    nc.vector.tensor_add(qa, qa, rot)
                    return qa, rb_s

                for sb in range(SB):
                    s0 = sb * 512
                    hblk = hp.tile([128, KC, 512], bf16)
                    nc.sync.dma_start(
                        out=hblk,
                        in_=hT[:, s0:s0 + 512].rearrange("(c p) s -> p c s",
                                                         p=128))
                    # ---- q projection (4 chunks of 128 rows) ----
                    for m in range(4):
                        ps = pps.tile([128, 512], f32, tag="proj")
                        for kc in range(KC):
                            nc.tensor.matmul(ps, wq_sb[:, kc, m * 128:(m + 1) * 128],
                                             hblk[:, kc, :],
                                             start=(kc == 0), stop=(kc == KC - 1))
                        qa, rb_s = rope_norm(ps, csA_q[:, s0:s0 + 512],
                                             csB_q[:, s0:s0 + 512])
                        # heads 2m, 2m+1 -> row-half r=m//2, chunks 2*(m%2)+{0,1}
                        r = (m // 2) * 64
                        cb = 2 * (m % 2)
                        nc.vector.tensor_mul(
                            qT_sb[r:r + 64, cb, s0:s0 + 512],
                            qa[0:64, :], rb_s[0:64, :])
                        nc.vector.tensor_mul(
                            qT_sb[r:r + 64, cb + 1, s0:s0 + 512],
                            qa[64:128, :], rb_s[64:128, :])
                    # ---- k projection (1 chunk) ----
                    ps = pps.tile([128, 512], f32, tag="proj")
                    for kc in range(KC):
                        nc.tensor.matmul(ps, wk_sb[:, kc, :], hblk[:, kc, :],
                                         start=(kc == 0), stop=(kc == KC - 1))
                    ka, rb_s = rope_norm(ps, csA_k[:, s0:s0 + 512],
                                         csB_k[:, s0:s0 + 512])
                    nc.vector.tensor_mul(kT_sb[:, s0:s0 + 512], ka, rb_s)
                    # ---- v projection + transpose to natural layout ----
                    ps = pps.tile([128, 512], f32, tag="proj")
                    for kc in range(KC):
                        nc.tensor.matmul(ps, wv_sb[:, kc, :], hblk[:, kc, :],
                                         start=(kc == 0), stop=(kc == KC - 1))
                    vt = wk.tile([128, 512], bf16, tag="vt")
                    nc.scalar.activation(out=vt, in_=ps, func=Copy)
                    for ss in range(4):
                        tp = trps.tile([128, 128], bf16, tag="tp")
                        nc.tensor.transpose(tp, vt[:, ss * 128:(ss + 1) * 128],
                                            ident)
                        chunk = sb * 4 + ss
                        nc.vector.tensor_copy(out=v_sb[:, chunk, :, 0:HD],
                                              in_=tp.rearrange("p (kv d) -> p kv d",
                                                               kv=2))
                    # ---- gate projection -> raw staged into og_sb ----
                    for m in range(4):
                        ps = pps.tile([128, 512], f32, tag="proj")
                        for kc in range(KC):
                            nc.tensor.matmul(ps, wg_sb[:, kc, m * 128:(m + 1) * 128],
                                             hblk[:, kc, :],
                                             start=(kc == 0), stop=(kc == KC - 1))
                        nc.scalar.activation(out=og_sb[:, m, s0:s0 + 512],
                                             in_=ps, func=Copy)

            # batched sigmoid: og_sb (raw gate) -> g_sb; one table switch
            for m in range(4):
                nc.scalar.activation(out=g_sb[:, m, :], in_=og_sb[:, m, :],
                                     func=Sigmoid)

            if debug_dump:
                nc.sync.dma_start(out=dbg_g, in_=g_sb)

            # ============ phase 2: attention + interleaved o_proj ============
            with tc.tile_pool(name="probs", bufs=6) as prp, \
                 tc.tile_pool(name="att_sm", bufs=4) as asm, \
                 tc.tile_pool(name="ostg", bufs=4) as ostg, \
                 tc.tile_pool(name="scps", bufs=2, space=PSUM) as scps, \
                 tc.tile_pool(name="avps", bufs=2, space=PSUM) as avps:

                def attn_pair(hp_idx, Q):
                    """Heads hA=hp_idx (kv0, rows 0:64) and hB=hp_idx+4 (kv1,
                    rows 64:128), both free-dim chunk hp_idx; q block Q."""
                    q0 = Q * 1024
                    nkc = 8 * (Q + 1)
                    hc = [hp_idx // 2, hp_idx // 2 + 2]   # og/g chunk per head
                    rr = (hp_idx % 2) * 64                # og/g row half
                    av_a = avps.tile([65, 2, 512], f32, tag="av")
                    av_b = avps.tile([65, 2, 512], f32, tag="av")
                    av = [av_a, av_b]
                    for kc in range(nkc):
                        s_c = max(0, kc * 128 - q0)   # first valid local col
                        for lb in range(2):
                            lo = lb * 512
                            if s_c >= lo + 512:
                                continue
                            st = max(s_c, lo)
                            sc2 = scps.tile([128, 2, 512], f32, tag="sc")
                            for hh in range(2):  # interleave -> row-tiled pair
                                p0 = hh * 64
                                nc.tensor.matmul(
                                    sc2[:, hh, st - lo:512],
                                    kT_sb[p0:p0 + 64, kc * 128:(kc + 1) * 128],
                                    qT_sb[p0:p0 + 64, hp_idx,
                                          q0 + st:q0 + lo + 512],
                                    start=True, stop=True)
                            # probs tile per (kc, lb): [*, head, col-in-block]
                            ptl = prp.tile([128, 2, 512], bf16, tag="ptl")
                            if st == lo:
                                # full slot: contiguous 1-region exp
                                nc.scalar.activation(
                                    out=ptl.rearrange("p a b -> p (a b)"),
                                    in_=sc2.rearrange("p a b -> p (a b)"),
                                    func=Exp, scale=SCALE)
                            else:
                                loc = st - lo
                                for hh in range(2):
                                    nc.scalar.activation(
                                        out=ptl[:, hh, loc:512],
                                        in_=sc2[:, hh, loc:512],
                                        func=Exp, scale=SCALE)
                            if lo <= s_c < lo + 512 and kc * 128 >= q0:
                                # boundary strip: keep k<=q inside cols
                                # [s_c, s_c+128) (local [s_c-lo, s_c-lo+128))
                                sl = s_c - lo
                                for hh in range(2):
                                    nc.vector.tensor_mul(
                                        ptl[:, hh, sl:sl + 128],
                                        ptl[:, hh, sl:sl + 128], triu)
                            last_kc = 4 * (2 * Q + lb) + 3
                            for hh in range(2):
                                nc.tensor.matmul(
                                    av[hh][:, lb, st - lo:512],
                                    v_sb[:, kc, hh, :],
                                    ptl[:, hh, st - lo:512],
                                    start=(kc == 0), stop=(kc == last_kc))
                    # normalize + gate -> og_sb. First flush av psum -> sbuf
                    # (bf16) so the psum slots free immediately and the next
                    # pair's PV can start; the whole tail chain then runs off
                    # sbuf concurrently with the next pair.
                    for hh in range(2):
                        av_sb = asm.tile([65, 1024], bf16, tag="avs")
                        nc.vector.tensor_copy(
                            out=av_sb,
                            in_=av[hh][:, :, :].rearrange("p a b -> p (a b)"))
                        # custom DVE op mishandles base_partition 64 and needs
                        # f32 input: stage the denominator row to base 0
                        den = asm.tile([1, 1024], f32, tag="den")
                        nc.vector.tensor_copy(out=den, in_=av_sb[64:65, :])
                        recip = asm.tile([1, 1024], f32, tag="recip")
                        nc.vector.reciprocal_approx_fast(out=recip, in_=den)
                        rbv = asm.tile([64, 1024], f32, tag="rbv")
                        nc.gpsimd.partition_broadcast(rbv, recip)
                        for lb in range(2):
                            qsl = slice(q0 + lb * 512, q0 + (lb + 1) * 512)
                            dst = og_sb[rr:rr + 64, hc[hh], qsl]
                            nc.vector.tensor_mul(
                                dst, av_sb[0:64, lb * 512:(lb + 1) * 512],
                                rbv[:, lb * 512:(lb + 1) * 512])
                            nc.vector.tensor_mul(
                                dst, dst, g_sb[rr:rr + 64, hc[hh], qsl])

                def oproj(nb, tail):
                    for m in range(KC):
                        po = scps.tile([128, 2, 512], f32, tag="sc")
                        for oc in range(4):
                            nc.tensor.matmul(po[:, 0, :], wo_sb[:, oc, m, :],
                                             og_sb[:, oc, nb * 512:(nb + 1) * 512],
                                             start=(oc == 0), stop=(oc == 3))
                        stg = ostg.tile([128, 512], bf16, tag="stg")
                        if tail and m % 2 == 0:
                            # tail oproj: ACT is idle (exp done) -> share copies
                            nc.scalar.activation(out=stg, in_=po[:, 0, :],
                                                 func=Copy)
                        else:
                            nc.vector.tensor_copy(out=stg, in_=po[:, 0, :])
                        nc.sync.dma_start(
                            out=outT[m * 128:(m + 1) * 128,
                                     nb * 512:(nb + 1) * 512],
                            in_=stg)

                for hp_idx in range(4):
                    attn_pair(hp_idx, 0)
                for hp_idx in range(4):
                    attn_pair(hp_idx, 1)
                # oproj 0/1 deps (all Q0 og) are ready when Q1 starts: the
                # scheduler uses these matmuls to fill Q1's exp-bound cycles
                for nb in (0, 1):
                    oproj(nb, tail=False)
                for nb in (2, 3):
                    oproj(nb, tail=True)

            if debug_dump:
                nc.sync.dma_start(out=dbg_q, in_=qT_sb)
                nc.sync.dma_start(out=dbg_k, in_=kT_sb)
                nc.sync.dma_start(out=dbg_v, in_=v_sb)
                nc.sync.dma_start(out=dbg_og, in_=og_sb)

    nc.compile()
    return nc


def _host_prep(hidden_states, cos, sin, Wq, Wk, Wv, Wg, Wo, q_norm_w, k_norm_w):
    """Build per-core input maps."""
    def cs_tables(cos_b, sin_b, w):
        # csA/csB [128, S]: row p -> head-local dim d = p % 64
        A = np.empty((128, S), np.float32)
        Bt = np.empty((128, S), np.float32)
        cosT = cos_b.T  # [32, S]
        sinT = sin_b.T
        for blk in (0, 64):
            A[blk + 0:blk + 32] = cosT * w[0:32, None]
            A[blk + 32:blk + 64] = w[32:64, None]
            Bt[blk + 0:blk + 16] = -sinT[0:16] * w[16:32, None]
            Bt[blk + 16:blk + 32] = sinT[16:32] * w[0:16, None]
            Bt[blk + 32:blk + 64] = 0.0
        return A.astype(BF16), Bt.astype(BF16)

    sel2_host = np.zeros((2, 128), np.float32)
    sel2_host[0, 0:64] = 1.0
    sel2_host[1, 64:128] = 1.0
    in_maps = []
    for c in range(NCORES):
        b, g = c // 4, c % 4
        qs = slice(g * QD, (g + 1) * QD)
        ks = slice(g * KD, (g + 1) * KD)
        csA_q, csB_q = cs_tables(cos[b], sin[b], np.asarray(q_norm_w))
        csA_k, csB_k = cs_tables(cos[b], sin[b], np.asarray(k_norm_w))
        in_maps.append({
            "hT": np.ascontiguousarray(hidden_states[b].T).astype(BF16),
            "wqT": np.ascontiguousarray(Wq[qs].T).astype(BF16),
            "wkT": np.ascontiguousarray(Wk[ks].T).astype(BF16),
            "wvT": np.ascontiguousarray(Wv[ks].T).astype(BF16),
            "wgT": np.ascontiguousarray(Wg[qs].T).astype(BF16),
            "woT": np.ascontiguousarray(Wo[:, qs].T).astype(BF16),
            "csAq": csA_q, "csBq": csB_q, "csAk": csA_k, "csBk": csB_k,
            "sel2": sel2_host,
        })
    return in_maps


def kernel(hidden_states, cos, sin, Wq, Wk, Wv, Wg, Wo, q_norm_w, k_norm_w):
    from concourse import bass_utils

    if "nc" not in _CACHE:
        _CACHE["nc"] = _build_bass()
    nc = _CACHE["nc"]

    in_maps = _host_prep(hidden_states, cos, sin, Wq, Wk, Wv, Wg, Wo,
                         q_norm_w, k_norm_w)

    trace = bool(int(os.environ.get("KERNEL_TRACE", "0")))
    kwargs = {}
    if trace:
        # the agent image's antenv lacks axon_hooks; recreate it from the
        # boot helper so run_bass_kernel_spmd(trace=True) can NTFF-profile
        try:
            import antenv.axon_hooks  # noqa: F401
        except ImportError:
            import types
            sys.path.insert(0, "/root/.axon_site")
            from trn_agent_boot.trn_boot import _ntff_profile_via_ctypes
            hook = _ntff_profile_via_ctypes("/opt/axon/libaxon_pjrt.so")
            mod = types.ModuleType("antenv.axon_hooks")
            mod.get_axon_ntff_profile_hook = lambda: hook
            sys.modules["antenv.axon_hooks"] = mod
        tmpdir = os.environ.get("KERNEL_TRACE_DIR") or None
        kwargs = dict(trace=True, tmpdir=tmpdir)
    res = bass_utils.run_bass_kernel_spmd(nc, in_maps,
                                          core_ids=list(range(NCORES)),
                                          **kwargs)
    if trace and res.exec_time_ns is not None:
        print(f"HW exec time: {res.exec_time_ns} ns")
        _CACHE["exec_time_ns"] = res.exec_time_ns

    out = np.zeros((B, S, HID), np.float32)
    for c in range(NCORES):
        b = c // 4
        out[b] += res.results[c]["outT"].T.astype(np.float32)
    return out


if __name__ == "__main__":
    rng = np.random.default_rng(0)
    hs = rng.standard_normal((B, S, HID), dtype=np.float32)
    cos = rng.random((B, S, ROPE), dtype=np.float32)
    sin = rng.random((B, S, ROPE), dtype=np.float32)
    out = kernel(hidden_states=hs, cos=cos, sin=sin,
                 Wq=rng.standard_normal((NH * HD, HID), dtype=np.float32) * 0.02,
                 Wk=rng.standard_normal((NKV * HD, HID), dtype=np.float32) * 0.02,
                 Wv=rng.standard_normal((NKV * HD, HID), dtype=np.float32) * 0.02,
                 Wg=rng.standard_normal((NH * HD, HID), dtype=np.float32) * 0.02,
                 Wo=rng.standard_normal((HID, NH * HD), dtype=np.float32) * 0.02,
                 q_norm_w=np.ones(HD, np.float32),
                 k_norm_w=np.ones(HD, np.float32))
    print(out.shape, out.dtype)


# revision 22
# speedup vs baseline: 1.3358x; 1.0114x over previous
"""GQA attention Trainium2 kernel (8 NeuronCores, SPMD, no collectives).

Sharding: 2-way data parallel (batch) x 4-way tensor parallel (heads).
Core c handles batch b=c//4 and head-group g=c%4 (8 q heads, 2 kv heads).
Each core produces a partial o_proj output (transposed, [HID, S] bf16);
the host sums the 4 partials per batch (f32) and transposes back.

On-device layout is feature-major ("transposed"): hidden is passed as
hT=[HID,S], projections produce qT/kT/gateT=[dim,S], attention scores are
computed as scoresT=[s_k,s_q] so softmax-exp output feeds the PV matmul
directly (lhsT = natural-layout V with an appended ones column that yields
the softmax denominator in psum row 64).

v2 changes vs baseline:
- rstd via ACT Rsqrt + matmul broadcast (sel2 [2,128] lhsT) instead of
  Sqrt + DVE reciprocal + gpsimd partition_broadcast (reciprocal was
  4.8us/instr, 171us total).
- rope math in bf16 (DVE 2x mode).
- gates staged raw into og_sb, sigmoid batched after phase 1 (avoids
  ACT table-set thrash: rsqrt set resident through phase 1).
- attention processed as head PAIRS (kv0 head rows 0-63, kv1 head rows
  64-127) with interleaved K=64 score matmuls -> concurrent row-tiled
  execution on the PE array (2x score throughput).
- causal trimming at 128-col granularity for scores/exp/PV; boundary
  128x128 strip masked by a DVE tril multiply (replaces gpsimd
  affine_select).
- softmax 1/denom via DVE reciprocal_approx_fast + matmul broadcast.
- o_proj split per q-half and emitted between attention halves so the
  PE has work while ACT grinds exp.
- outT in bf16 (host accumulates partials in f32).
"""

import os
import sys
import numpy as np

for _p in ("/opt/trn_rl_repo", "/root/.axon_site/_ro/trn_rl_repo"):
    if os.path.isdir(_p) and _p not in sys.path:
        sys.path.insert(0, _p)

import ml_dtypes

B, S, HID = 2, 2048, 2048
NH, NKV, HD = 32, 8, 64
ROPE = 32
EPS = 1e-6
SCALE = HD ** -0.5
NCORES = 8
QH = NH // 4      # 8 q heads per core
KVH = NKV // 4    # 2 kv heads per core
QD = QH * HD      # 512 per-core q dim
KD = KVH * HD     # 128 per-core kv dim
KC = HID // 128   # 16 contraction chunks
SB = S // 512     # 4 sequence blocks of 512
BF16 = ml_dtypes.bfloat16

_CACHE = {}


def _build_bass(debug_dump=False):
    import concourse.bass as bass
    from concourse import bacc, mybir, tile

    f32 = mybir.dt.float32
    bf16 = mybir.dt.bfloat16

    nc = bacc.Bacc("TRN2", target_bir_lowering=False, debug=False,
                   enable_asserts=False, num_devices=NCORES)

    hT = nc.dram_tensor("hT", [HID, S], bf16, kind="ExternalInput").ap()
    wqT = nc.dram_tensor("wqT", [HID, QD], bf16, kind="ExternalInput").ap()
    wkT = nc.dram_tensor("wkT", [HID, KD], bf16, kind="ExternalInput").ap()
    wvT = nc.dram_tensor("wvT", [HID, KD], bf16, kind="ExternalInput").ap()
    wgT = nc.dram_tensor("wgT", [HID, QD], bf16, kind="ExternalInput").ap()
    woT = nc.dram_tensor("woT", [QD, HID], bf16, kind="ExternalInput").ap()
    csAq = nc.dram_tensor("csAq", [128, S], bf16, kind="ExternalInput").ap()
    csBq = nc.dram_tensor("csBq", [128, S], bf16, kind="ExternalInput").ap()
    csAk = nc.dram_tensor("csAk", [128, S], bf16, kind="ExternalInput").ap()
    csBk = nc.dram_tensor("csBk", [128, S], bf16, kind="ExternalInput").ap()
    sel2d = nc.dram_tensor("sel2", [2, 128], f32, kind="ExternalInput").ap()
    outT = nc.dram_tensor("outT", [HID, S], bf16, kind="ExternalOutput").ap()
    if debug_dump:
        dbg_q = nc.dram_tensor("dbg_q", [128, 4, S], bf16, kind="ExternalOutput").ap()
        dbg_k = nc.dram_tensor("dbg_k", [128, S], bf16, kind="ExternalOutput").ap()
        dbg_v = nc.dram_tensor("dbg_v", [128, KC, KVH, HD + 1], bf16,
                               kind="ExternalOutput").ap()
        dbg_g = nc.dram_tensor("dbg_g", [128, 4, S], bf16, kind="ExternalOutput").ap()
        dbg_og = nc.dram_tensor("dbg_og", [128, 4, S], bf16,
                                kind="ExternalOutput").ap()

    Exp = mybir.ActivationFunctionType.Exp
    Sigmoid = mybir.ActivationFunctionType.Sigmoid
    Square = mybir.ActivationFunctionType.Square
    Sqrt = mybir.ActivationFunctionType.Sqrt
    Copy = mybir.ActivationFunctionType.Copy
    PSUM = bass.MemorySpace.PSUM

    with tile.TileContext(nc) as tc:
        # ---- persistent sbuf ----
        with tc.tile_pool(name="persist", bufs=1) as pp:
            # head h lives at partition rows (h//4)*64 (matching its kv head's
            # rows so matmul operand bases agree), free-dim chunk h%4
            # triu[p, j] = 1.0 where p <= j (causal keep-mask for the
            # scoresT boundary strip), 0 above
            triu = pp.tile([128, 128], bf16)
            qT_sb = pp.tile([128, 4, S], bf16)        # q (roped+normed)
            kT_sb = pp.tile([128, S], bf16)           # k (roped+normed)
            g_sb = pp.tile([128, 4, S], bf16)         # sigmoid(gate)
            v_sb = pp.tile([128, KC, KVH, HD + 1], bf16)  # natural V + ones col
            wo_sb = pp.tile([128, 4, KC, 128], bf16)
            og_sb = pp.tile([128, 4, S], bf16)        # raw gate, then gated out

            nc.sync.dma_start(out=wo_sb,
                              in_=woT.rearrange("(c p) (mb mm) -> p c mb mm",
                                                p=128, mm=128))
            nc.vector.memset(v_sb[:, :, :, HD:HD + 1], 1.0)

            # ================= phase 1: projections =================
            with tc.tile_pool(name="consts", bufs=1) as cp, \
                 tc.tile_pool(name="wts", bufs=1) as wp, \
                 tc.tile_pool(name="hblk", bufs=2) as hp, \
                 tc.tile_pool(name="work", bufs=3) as wk, \
                 tc.tile_pool(name="smallw", bufs=3) as smp, \
                 tc.tile_pool(name="rbpool", bufs=3) as rbp, \
                 tc.tile_pool(name="rwork", bufs=3) as rwk, \
                 tc.tile_pool(name="pps", bufs=2, space=PSUM) as pps, \
                 tc.tile_pool(name="sqps", bufs=2, space=PSUM) as sqps, \
                 tc.tile_pool(name="rbps", bufs=2, space=PSUM) as rbps, \
                 tc.tile_pool(name="trps", bufs=2, space=PSUM) as trps:

                csA_q = cp.tile([128, S], bf16)
                csB_q = cp.tile([128, S], bf16)
                csA_k = cp.tile([128, S], bf16)
                csB_k = cp.tile([128, S], bf16)
                nc.sync.dma_start(out=csA_q, in_=csAq)
                nc.sync.dma_start(out=csB_q, in_=csBq)
                nc.sync.dma_start(out=csA_k, in_=csAk)
                nc.sync.dma_start(out=csB_k, in_=csBk)
                ident = cp.tile([128, 128], bf16)
                from concourse.masks import make_identity, make_upper_triangular
                make_identity(nc, ident)
                make_upper_triangular(nc, triu, val=1.0, diag=True)
                ones2 = cp.tile([128, 2], bf16)
                nc.vector.memset(ones2, 0.0)
                nc.vector.memset(ones2[0:64, 0:1], 1.0)
                nc.vector.memset(ones2[64:128, 1:2], 1.0)
                # sel2.T broadcast: row p of (sel2.T @ rstd) = rstd[p//64]
                sel2 = cp.tile([2, 128], f32)
                nc.sync.dma_start(out=sel2, in_=sel2d)
                eps_t = cp.tile([8, 1], f32)
                nc.vector.memset(eps_t, EPS)

                wq_sb = wp.tile([128, KC, QD], bf16)
                wk_sb = wp.tile([128, KC, KD], bf16)
                wv_sb = wp.tile([128, KC, KD], bf16)
                wg_sb = wp.tile([128, KC, QD], bf16)
                nc.sync.dma_start(out=wq_sb,
                                  in_=wqT.rearrange("(c p) m -> p c m", p=128))
                nc.sync.dma_start(out=wk_sb,
                                  in_=wkT.rearrange("(c p) m -> p c m", p=128))
                nc.sync.dma_start(out=wv_sb,
                                  in_=wvT.rearrange("(c p) m -> p c m", p=128))
                nc.sync.dma_start(out=wg_sb,
                                  in_=wgT.rearrange("(c p) m -> p c m", p=128))

                def rope_norm(ps, csA, csB):
                    """ps: psum [128,512] raw proj. Returns (qa bf16 roped,
                    rb_s bf16 [128,512] rstd broadcast)."""
                    sq_t = wk.tile([128, 512], bf16, tag="sq")
                    nc.scalar.activation(out=sq_t, in_=ps, func=Square)
                    qb = rwk.tile([128, 512], bf16, tag="qb")
                    nc.scalar.activation(out=qb, in_=ps, func=Copy)
                    # sum of squares per 64-row half via matmul
                    sq_ps = sqps.tile([2, 512], f32, tag="sqs")
                    nc.tensor.matmul(sq_ps, ones2, sq_t, start=True, stop=True)
                    sstd = smp.tile([2, 512], f32, tag="sstd")
                    nc.scalar.activation(out=sstd, in_=sq_ps, func=Sqrt,
                                         scale=1.0 / HD, bias=eps_t[0:2])
                    rstd = smp.tile([2, 512], f32, tag="rstd")
                    nc.vector.reciprocal_approx_fast(out=rstd, in_=sstd)
                    # broadcast rstd rows to halves via matmul
                    rb_ps = rbps.tile([128, 512], f32, tag="rb")
                    nc.tensor.matmul(rb_ps, sel2, rstd, start=True, stop=True)
                    rb_s = rbp.tile([128, 512], bf16, tag="rbs")
                    nc.vector.tensor_copy(out=rb_s, in_=rb_ps)
                    # rope: qa = qb*csA + rot(qb)*csB
                    rot = rwk.tile([128, 512], bf16, tag="rot")
                    for hh in (0, 64):
                        # 16-row rotate needs non-32-aligned partition bases:
                        # only DMA can address those
                        nc.gpsimd.dma_start(out=rot[hh + 0:hh + 16],
                                            in_=qb[hh + 16:hh + 32])
                        nc.gpsimd.dma_start(out=rot[hh + 16:hh + 32],
                                            in_=qb[hh + 0:hh + 16])
                        nc.vector.tensor_copy(out=rot[hh + 32:hh + 64],
                                              in_=qb[hh + 32:hh + 64])
                    nc.vector.tensor_mul(rot, rot, csB)
                    qa = rwk.tile([128, 512], bf16, tag="qa")
                    nc.vector.tensor_mul(qa, qb, csA)
                    nc.vector.tensor_add(qa, qa, rot)
                    return qa, rb_s

                for sb in range(SB):
                    s0 = sb * 512
                    hblk = hp.tile([128, KC, 512], bf16)
                    nc.sync.dma_start(
                        out=hblk,
                        in_=hT[:, s0:s0 + 512].rearrange("(c p) s -> p c s",
                                                         p=128))
                    # ---- q projection (4 chunks of 128 rows) ----
                    for m in range(4):
                        ps = pps.tile([128, 512], f32, tag="proj")
                        for kc in range(KC):
                            nc.tensor.matmul(ps, wq_sb[:, kc, m * 128:(m + 1) * 128],
                                             hblk[:, kc, :],
                                             start=(kc == 0), stop=(kc == KC - 1))
                        qa, rb_s = rope_norm(ps, csA_q[:, s0:s0 + 512],
                                             csB_q[:, s0:s0 + 512])
                        # heads 2m, 2m+1 -> row-half r=m//2, chunks 2*(m%2)+{0,1}
                        r = (m // 2) * 64
                        cb = 2 * (m % 2)
                        nc.vector.tensor_mul(
                            qT_sb[r:r + 64, cb, s0:s0 + 512],
                            qa[0:64, :], rb_s[0:64, :])
                        nc.vector.tensor_mul(
                            qT_sb[r:r + 64, cb + 1, s0:s0 + 512],
                            qa[64:128, :], rb_s[64:128, :])
                    # ---- k projection (1 chunk) ----
                    ps = pps.tile([128, 512], f32, tag="proj")
                    for kc in range(KC):
                        nc.tensor.matmul(ps, wk_sb[:, kc, :], hblk[:, kc, :],
                                         start=(kc == 0), stop=(kc == KC - 1))
                    ka, rb_s = rope_norm(ps, csA_k[:, s0:s0 + 512],
                                         csB_k[:, s0:s0 + 512])
                    nc.vector.tensor_mul(kT_sb[:, s0:s0 + 512], ka, rb_s)
                    # ---- v projection + transpose to natural layout ----
                    ps = pps.tile([128, 512], f32, tag="proj")
                    for kc in range(KC):
                        nc.tensor.matmul(ps, wv_sb[:, kc, :], hblk[:, kc, :],
                                         start=(kc == 0), stop=(kc == KC - 1))
                    vt = wk.tile([128, 512], bf16, tag="vt")
                    nc.scalar.activation(out=vt, in_=ps, func=Copy)
                    for ss in range(4):
                        tp = trps.tile([128, 128], bf16, tag="tp")
                        nc.tensor.transpose(tp, vt[:, ss * 128:(ss + 1) * 128],
                                            ident)
                        chunk = sb * 4 + ss
                        nc.vector.tensor_copy(out=v_sb[:, chunk, :, 0:HD],
                                              in_=tp.rearrange("p (kv d) -> p kv d",
                                                               kv=2))
                    # ---- gate projection -> raw staged into og_sb ----
                    for m in range(4):
                        ps = pps.tile([128, 512], f32, tag="proj")
                        for kc in range(KC):
                            nc.tensor.matmul(ps, wg_sb[:, kc, m * 128:(m + 1) * 128],
                                             hblk[:, kc, :],
                                             start=(kc == 0), stop=(kc == KC - 1))
                        nc.scalar.activation(out=og_sb[:, m, s0:s0 + 512],
                                             in_=ps, func=Copy)

            # batched sigmoid: og_sb (raw gate) -> g_sb; one table switch
            for m in range(4):
                nc.scalar.activation(out=g_sb[:, m, :], in_=og_sb[:, m, :],
                                     func=Sigmoid)

            if debug_dump:
                nc.sync.dma_start(out=dbg_g, in_=g_sb)

            # ============ phase 2: attention + interleaved o_proj ============
            with tc.tile_pool(name="probs", bufs=6) as prp, \
                 tc.tile_pool(name="att_sm", bufs=4) as asm, \
                 tc.tile_pool(name="ostg", bufs=4) as ostg, \
                 tc.tile_pool(name="scps", bufs=2, space=PSUM) as scps, \
                 tc.tile_pool(name="avps", bufs=2, space=PSUM) as avps:

                def attn_pair(hp_idx, Q):
                    """Heads hA=hp_idx (kv0, rows 0:64) and hB=hp_idx+4 (kv1,
                    rows 64:128), both free-dim chunk hp_idx; q block Q."""
                    q0 = Q * 1024
                    nkc = 8 * (Q + 1)
                    hc = [hp_idx // 2, hp_idx // 2 + 2]   # og/g chunk per head
                    rr = (hp_idx % 2) * 64                # og/g row half
                    av_a = avps.tile([65, 2, 512], f32, tag="av")
                    av_b = avps.tile([65, 2, 512], f32, tag="av")
                    av = [av_a, av_b]
                    for kc in range(nkc):
                        s_c = max(0, kc * 128 - q0)   # first valid local col
                        for lb in range(2):
                            lo = lb * 512
                            if s_c >= lo + 512:
                                continue
                            st = max(s_c, lo)
                            sc2 = scps.tile([128, 2, 512], f32, tag="sc")
                            for hh in range(2):  # interleave -> row-tiled pair
                                p0 = hh * 64
                                nc.tensor.matmul(
                                    sc2[:, hh, st - lo:512],
                                    kT_sb[p0:p0 + 64, kc * 128:(kc + 1) * 128],
                                    qT_sb[p0:p0 + 64, hp_idx,
                                          q0 + st:q0 + lo + 512],
                                    start=True, stop=True)
                            # probs tile per (kc, lb): [*, head, col-in-block]
                            ptl = prp.tile([128, 2, 512], bf16, tag="ptl")
                            if st == lo:
                                # full slot: contiguous 1-region exp
                                nc.scalar.activation(
                                    out=ptl.rearrange("p a b -> p (a b)"),
                                    in_=sc2.rearrange("p a b -> p (a b)"),
                                    func=Exp, scale=SCALE)
                            else:
                                loc = st - lo
                                for hh in range(2):
                                    nc.scalar.activation(
                                        out=ptl[:, hh, loc:512],
                                        in_=sc2[:, hh, loc:512],
                                        func=Exp, scale=SCALE)
                            if lo <= s_c < lo + 512 and kc * 128 >= q0:
                                # boundary strip: keep k<=q inside cols
                                # [s_c, s_c+128) (local [s_c-lo, s_c-lo+128))
                                sl = s_c - lo
                                for hh in range(2):
                                    nc.vector.tensor_mul(
                                        ptl[:, hh, sl:sl + 128],
                                        ptl[:, hh, sl:sl + 128], triu)
                            last_kc = 4 * (2 * Q + lb) + 3
                            for hh in range(2):
                                nc.tensor.matmul(
                                    av[hh][:, lb, st - lo:512],
                                    v_sb[:, kc, hh, :],
                                    ptl[:, hh, st - lo:512],
                                    start=(kc == 0), stop=(kc == last_kc))
                    # flush av psum -> sbuf (bf16) so the psum slots free
                    # immediately; the normalize/gate tail is emitted LATER
                    # (after the next pair's kc loop) so Tile's position-keyed
                    # slot releases don't serialize pairs on the tail chain.
                    avs = []
                    for hh in range(2):
                        av_sb = asm.tile([65, 1024], bf16, tag="avs")
                        nc.vector.tensor_copy(
                            out=av_sb,
                            in_=av[hh][:, :, :].rearrange("p a b -> p (a b)"))
                        avs.append(av_sb)
                    return (avs, hp_idx, Q)

                def attn_tail(ctx):
                    avs, hp_idx, Q = ctx
                    q0 = Q * 1024
                    hc = [hp_idx // 2, hp_idx // 2 + 2]
                    rr = (hp_idx % 2) * 64
                    for hh in range(2):
                        av_sb = avs[hh]
                        # custom DVE op mishandles base_partition 64 and needs
                        # f32 input: stage the denominator row to base 0
                        den = asm.tile([1, 1024], f32, tag="den")
                        nc.vector.tensor_copy(out=den, in_=av_sb[64:65, :])
                        recip = asm.tile([1, 1024], f32, tag="recip")
                        nc.vector.reciprocal_approx_fast(out=recip, in_=den)
                        rbv = asm.tile([64, 1024], f32, tag="rbv")
                        nc.gpsimd.partition_broadcast(rbv, recip)
                        for lb in range(2):
                            qsl = slice(q0 + lb * 512, q0 + (lb + 1) * 512)
                            dst = og_sb[rr:rr + 64, hc[hh], qsl]
                            nc.vector.tensor_mul(
                                dst, av_sb[0:64, lb * 512:(lb + 1) * 512],
                                rbv[:, lb * 512:(lb + 1) * 512])
                            nc.vector.tensor_mul(
                                dst, dst, g_sb[rr:rr + 64, hc[hh], qsl])

                def oproj(nb, tail):
                    for m in range(KC):
                        po = scps.tile([128, 2, 512], f32, tag="sc")
                        for oc in range(4):
                            nc.tensor.matmul(po[:, 0, :], wo_sb[:, oc, m, :],
                                             og_sb[:, oc, nb * 512:(nb + 1) * 512],
                                             start=(oc == 0), stop=(oc == 3))
                        stg = ostg.tile([128, 512], bf16, tag="stg")
                        if tail and m % 2 == 0:
                            # tail oproj: ACT is idle (exp done) -> share copies
                            nc.scalar.activation(out=stg, in_=po[:, 0, :],
                                                 func=Copy)
                        else:
                            nc.vector.tensor_copy(out=stg, in_=po[:, 0, :])
                        nc.sync.dma_start(
                            out=outT[m * 128:(m + 1) * 128,
                                     nb * 512:(nb + 1) * 512],
                            in_=stg)

                # software-pipeline: pair p's normalize tail is emitted after
                # pair p+1's kc loop
                pending = None
                for Q in (0, 1):
                    for hp_idx in range(4):
                        ctx = attn_pair(hp_idx, Q)
                        if pending is not None:
                            attn_tail(pending)
                        pending = ctx
                attn_tail(pending)
                for nb in (0, 1):
                    oproj(nb, tail=False)
                for nb in (2, 3):
                    oproj(nb, tail=True)

            if debug_dump:
                nc.sync.dma_start(out=dbg_q, in_=qT_sb)
                nc.sync.dma_start(out=dbg_k, in_=kT_sb)
                nc.sync.dma_start(out=dbg_v, in_=v_sb)
                nc.sync.dma_start(out=dbg_og, in_=og_sb)

    nc.compile()
    return nc


def _host_prep(hidden_states, cos, sin, Wq, Wk, Wv, Wg, Wo, q_norm_w, k_norm_w):
    """Build per-core input maps."""
    def cs_tables(cos_b, sin_b, w):
        # csA/csB [128, S]: row p -> head-local dim d = p % 64
        A = np.empty((128, S), np.float32)
        Bt = np.empty((128, S), np.float32)
        cosT = cos_b.T  # [32, S]
        sinT = sin_b.T
        for blk in (0, 64):
            A[blk + 0:blk + 32] = cosT * w[0:32, None]
            A[blk + 32:blk + 64] = w[32:64, None]
            Bt[blk + 0:blk + 16] = -sinT[0:16] * w[16:32, None]
            Bt[blk + 16:blk + 32] = sinT[16:32] * w[0:16, None]
            Bt[blk + 32:blk + 64] = 0.0
        return A.astype(BF16), Bt.astype(BF16)

    sel2_host = np.zeros((2, 128), np.float32)
    sel2_host[0, 0:64] = 1.0
    sel2_host[1, 64:128] = 1.0
    in_maps = []
    for c in range(NCORES):
        b, g = c // 4, c % 4
        qs = slice(g * QD, (g + 1) * QD)
        ks = slice(g * KD, (g + 1) * KD)
        csA_q, csB_q = cs_tables(cos[b], sin[b], np.asarray(q_norm_w))
        csA_k, csB_k = cs_tables(cos[b], sin[b], np.asarray(k_norm_w))
        in_maps.append({
            "hT": np.ascontiguousarray(hidden_states[b].T).astype(BF16),
            "wqT": np.ascontiguousarray(Wq[qs].T).astype(BF16),
            "wkT": np.ascontiguousarray(Wk[ks].T).astype(BF16),
            "wvT": np.ascontiguousarray(Wv[ks].T).astype(BF16),
            "wgT": np.ascontiguousarray(Wg[qs].T).astype(BF16),
            "woT": np.ascontiguousarray(Wo[:, qs].T).astype(BF16),
            "csAq": csA_q, "csBq": csB_q, "csAk": csA_k, "csBk": csB_k,
            "sel2": sel2_host,
        })
    return in_maps


def kernel(hidden_states, cos, sin, Wq, Wk, Wv, Wg, Wo, q_norm_w, k_norm_w):
    from concourse import bass_utils

    if "nc" not in _CACHE:
        _CACHE["nc"] = _build_bass()
    nc = _CACHE["nc"]

    in_maps = _host_prep(hidden_states, cos, sin, Wq, Wk, Wv, Wg, Wo,
                         q_norm_w, k_norm_w)

    trace = bool(int(os.environ.get("KERNEL_TRACE", "0")))
    kwargs = {}
    if trace:
        # the agent image's antenv lacks axon_hooks; recreate it from the
        # boot helper so run_bass_kernel_spmd(trace=True) can NTFF-profile
        try:
            import antenv.axon_hooks  # noqa: F401
        except ImportError:
            import types
            sys.path.insert(0, "/root/.axon_site")
            from trn_agent_boot.trn_boot import _ntff_profile_via_ctypes
            hook = _ntff_profile_via_ctypes("/opt/axon/libaxon_pjrt.so")
            mod = types.ModuleType("antenv.axon_hooks")
            mod.get_axon_ntff_profile_hook = lambda: hook
            sys.modules["antenv.axon_hooks"] = mod
        tmpdir = os.environ.get("KERNEL_TRACE_DIR") or None
        kwargs = dict(trace=True, tmpdir=tmpdir)
    res = bass_utils.run_bass_kernel_spmd(nc, in_maps,
                                          core_ids=list(range(NCORES)),
                                          **kwargs)
    if trace and res.exec_time_ns is not None:
        print(f"HW exec time: {res.exec_time_ns} ns")
        _CACHE["exec_time_ns"] = res.exec_time_ns

    out = np.zeros((B, S, HID), np.float32)
    for c in range(NCORES):
        b = c // 4
        out[b] += res.results[c]["outT"].T.astype(np.float32)
    return out


if __name__ == "__main__":
    rng = np.random.default_rng(0)
    hs = rng.standard_normal((B, S, HID), dtype=np.float32)
    cos = rng.random((B, S, ROPE), dtype=np.float32)
    sin = rng.random((B, S, ROPE), dtype=np.float32)
    out = kernel(hidden_states=hs, cos=cos, sin=sin,
                 Wq=rng.standard_normal((NH * HD, HID), dtype=np.float32) * 0.02,
                 Wk=rng.standard_normal((NKV * HD, HID), dtype=np.float32) * 0.02,
                 Wv=rng.standard_normal((NKV * HD, HID), dtype=np.float32) * 0.02,
                 Wg=rng.standard_normal((NH * HD, HID), dtype=np.float32) * 0.02,
                 Wo=rng.standard_normal((HID, NH * HD), dtype=np.float32) * 0.02,
                 q_norm_w=np.ones(HD, np.float32),
                 k_norm_w=np.ones(HD, np.float32))
    print(out.shape, out.dtype)


# revision 25
# speedup vs baseline: 1.3496x; 1.0103x over previous
"""GQA attention Trainium2 kernel (8 NeuronCores, SPMD, no collectives).

Sharding: 2-way data parallel (batch) x 4-way tensor parallel (heads).
Core c handles batch b=c//4 and head-group g=c%4 (8 q heads, 2 kv heads).
Each core produces a partial o_proj output (transposed, [HID, S] bf16);
the host sums the 4 partials per batch (f32) and transposes back.

On-device layout is feature-major ("transposed"): hidden is passed as
hT=[HID,S], projections produce qT/kT/gateT=[dim,S], attention scores are
computed as scoresT=[s_k,s_q] so softmax-exp output feeds the PV matmul
directly (lhsT = natural-layout V with an appended ones column that yields
the softmax denominator in psum row 64).

v2 changes vs baseline:
- rstd via ACT Rsqrt + matmul broadcast (sel2 [2,128] lhsT) instead of
  Sqrt + DVE reciprocal + gpsimd partition_broadcast (reciprocal was
  4.8us/instr, 171us total).
- rope math in bf16 (DVE 2x mode).
- gates staged raw into og_sb, sigmoid batched after phase 1 (avoids
  ACT table-set thrash: rsqrt set resident through phase 1).
- attention processed as head PAIRS (kv0 head rows 0-63, kv1 head rows
  64-127) with interleaved K=64 score matmuls -> concurrent row-tiled
  execution on the PE array (2x score throughput).
- causal trimming at 128-col granularity for scores/exp/PV; boundary
  128x128 strip masked by a DVE tril multiply (replaces gpsimd
  affine_select).
- softmax 1/denom via DVE reciprocal_approx_fast + matmul broadcast.
- o_proj split per q-half and emitted between attention halves so the
  PE has work while ACT grinds exp.
- outT in bf16 (host accumulates partials in f32).
"""

import os
import sys
import numpy as np

for _p in ("/opt/trn_rl_repo", "/root/.axon_site/_ro/trn_rl_repo"):
    if os.path.isdir(_p) and _p not in sys.path:
        sys.path.insert(0, _p)

import ml_dtypes

B, S, HID = 2, 2048, 2048
NH, NKV, HD = 32, 8, 64
ROPE = 32
EPS = 1e-6
SCALE = HD ** -0.5
NCORES = 8
QH = NH // 4      # 8 q heads per core
KVH = NKV // 4    # 2 kv heads per core
QD = QH * HD      # 512 per-core q dim
KD = KVH * HD     # 128 per-core kv dim
KC = HID // 128   # 16 contraction chunks
SB = S // 512     # 4 sequence blocks of 512
BF16 = ml_dtypes.bfloat16

_CACHE = {}


def _build_bass(debug_dump=False):
    import concourse.bass as bass
    from concourse import bacc, mybir, tile

    f32 = mybir.dt.float32
    bf16 = mybir.dt.bfloat16

    nc = bacc.Bacc("TRN2", target_bir_lowering=False, debug=False,
                   enable_asserts=False, num_devices=NCORES)

    hT = nc.dram_tensor("hT", [HID, S], bf16, kind="ExternalInput").ap()
    wqT = nc.dram_tensor("wqT", [HID, QD], bf16, kind="ExternalInput").ap()
    wkT = nc.dram_tensor("wkT", [HID, KD], bf16, kind="ExternalInput").ap()
    wvT = nc.dram_tensor("wvT", [HID, KD], bf16, kind="ExternalInput").ap()
    wgT = nc.dram_tensor("wgT", [HID, QD], bf16, kind="ExternalInput").ap()
    woT = nc.dram_tensor("woT", [QD, HID], bf16, kind="ExternalInput").ap()
    csAq = nc.dram_tensor("csAq", [128, S], bf16, kind="ExternalInput").ap()
    csBq = nc.dram_tensor("csBq", [128, S], bf16, kind="ExternalInput").ap()
    csAk = nc.dram_tensor("csAk", [128, S], bf16, kind="ExternalInput").ap()
    csBk = nc.dram_tensor("csBk", [128, S], bf16, kind="ExternalInput").ap()
    sel2d = nc.dram_tensor("sel2", [2, 128], f32, kind="ExternalInput").ap()
    outT = nc.dram_tensor("outT", [HID, S], bf16, kind="ExternalOutput").ap()
    if debug_dump:
        dbg_q = nc.dram_tensor("dbg_q", [128, 4, S], bf16, kind="ExternalOutput").ap()
        dbg_k = nc.dram_tensor("dbg_k", [128, S], bf16, kind="ExternalOutput").ap()
        dbg_v = nc.dram_tensor("dbg_v", [128, KC, KVH, HD + 1], bf16,
                               kind="ExternalOutput").ap()
        dbg_g = nc.dram_tensor("dbg_g", [128, 4, S], bf16, kind="ExternalOutput").ap()
        dbg_og = nc.dram_tensor("dbg_og", [128, 4, S], bf16,
                                kind="ExternalOutput").ap()

    Exp = mybir.ActivationFunctionType.Exp
    Sigmoid = mybir.ActivationFunctionType.Sigmoid
    Square = mybir.ActivationFunctionType.Square
    Sqrt = mybir.ActivationFunctionType.Sqrt
    Copy = mybir.ActivationFunctionType.Copy
    PSUM = bass.MemorySpace.PSUM

    with tile.TileContext(nc) as tc:
        # ---- persistent sbuf ----
        with tc.tile_pool(name="persist", bufs=1) as pp:
            # head h lives at partition rows (h//4)*64 (matching its kv head's
            # rows so matmul operand bases agree), free-dim chunk h%4
            # triu[p, j] = 1.0 where p <= j (causal keep-mask for the
            # scoresT boundary strip), 0 above
            triu = pp.tile([128, 128], bf16)
            qT_sb = pp.tile([128, 4, S], bf16)        # q (roped+normed)
            kT_sb = pp.tile([128, S], bf16)           # k (roped+normed)
            g_sb = pp.tile([128, 4, S], bf16)         # sigmoid(gate)
            v_sb = pp.tile([128, KC, KVH, HD + 1], bf16)  # natural V + ones col
            wo_sb = pp.tile([128, 4, KC, 128], bf16)
            og_sb = pp.tile([128, 4, S], bf16)        # raw gate, then gated out

            nc.sync.dma_start(out=wo_sb,
                              in_=woT.rearrange("(c p) (mb mm) -> p c mb mm",
                                                p=128, mm=128))
            nc.vector.memset(v_sb[:, :, :, HD:HD + 1], 1.0)

            # ================= phase 1: projections =================
            with tc.tile_pool(name="consts", bufs=1) as cp, \
                 tc.tile_pool(name="wts", bufs=1) as wp, \
                 tc.tile_pool(name="hblk", bufs=2) as hp, \
                 tc.tile_pool(name="work", bufs=3) as wk, \
                 tc.tile_pool(name="smallw", bufs=3) as smp, \
                 tc.tile_pool(name="rbpool", bufs=3) as rbp, \
                 tc.tile_pool(name="rwork", bufs=3) as rwk, \
                 tc.tile_pool(name="pps", bufs=2, space=PSUM) as pps, \
                 tc.tile_pool(name="sqps", bufs=2, space=PSUM) as sqps, \
                 tc.tile_pool(name="rbps", bufs=2, space=PSUM) as rbps, \
                 tc.tile_pool(name="trps", bufs=2, space=PSUM) as trps:

                csA_q = cp.tile([128, S], bf16)
                csB_q = cp.tile([128, S], bf16)
                csA_k = cp.tile([128, S], bf16)
                csB_k = cp.tile([128, S], bf16)
                nc.sync.dma_start(out=csA_q, in_=csAq)
                nc.sync.dma_start(out=csB_q, in_=csBq)
                nc.sync.dma_start(out=csA_k, in_=csAk)
                nc.sync.dma_start(out=csB_k, in_=csBk)
                ident = cp.tile([128, 128], bf16)
                from concourse.masks import make_identity, make_upper_triangular
                make_identity(nc, ident)
                make_upper_triangular(nc, triu, val=1.0, diag=True)
                ones2 = cp.tile([128, 2], bf16)
                nc.vector.memset(ones2, 0.0)
                nc.vector.memset(ones2[0:64, 0:1], 1.0)
                nc.vector.memset(ones2[64:128, 1:2], 1.0)
                # sel2.T broadcast: row p of (sel2.T @ rstd) = rstd[p//64]
                sel2 = cp.tile([2, 128], f32)
                nc.sync.dma_start(out=sel2, in_=sel2d)
                eps_t = cp.tile([8, 1], f32)
                nc.vector.memset(eps_t, EPS)

                wq_sb = wp.tile([128, KC, QD], bf16)
                wk_sb = wp.tile([128, KC, KD], bf16)
                wv_sb = wp.tile([128, KC, KD], bf16)
                wg_sb = wp.tile([128, KC, QD], bf16)
                nc.sync.dma_start(out=wq_sb,
                                  in_=wqT.rearrange("(c p) m -> p c m", p=128))
                nc.sync.dma_start(out=wk_sb,
                                  in_=wkT.rearrange("(c p) m -> p c m", p=128))
                nc.sync.dma_start(out=wv_sb,
                                  in_=wvT.rearrange("(c p) m -> p c m", p=128))
                nc.sync.dma_start(out=wg_sb,
                                  in_=wgT.rearrange("(c p) m -> p c m", p=128))

                def rope_norm(ps, csA, csB):
                    """ps: psum [128,512] raw proj. Returns (qa bf16 roped,
                    rb_s bf16 [128,512] rstd broadcast)."""
                    sq_t = wk.tile([128, 512], bf16, tag="sq")
                    nc.scalar.activation(out=sq_t, in_=ps, func=Square)
                    qb = rwk.tile([128, 512], bf16, tag="qb")
                    nc.scalar.activation(out=qb, in_=ps, func=Copy)
                    # sum of squares per 64-row half via matmul
                    sq_ps = sqps.tile([2, 512], f32, tag="sqs")
                    nc.tensor.matmul(sq_ps, ones2, sq_t, start=True, stop=True)
                    sstd = smp.tile([2, 512], f32, tag="sstd")
                    nc.scalar.activation(out=sstd, in_=sq_ps, func=Sqrt,
                                         scale=1.0 / HD, bias=eps_t[0:2])
                    rstd = smp.tile([2, 512], f32, tag="rstd")
                    nc.vector.reciprocal_approx_fast(out=rstd, in_=sstd)
                    # broadcast rstd rows to halves via matmul
                    rb_ps = rbps.tile([128, 512], f32, tag="rb")
                    nc.tensor.matmul(rb_ps, sel2, rstd, start=True, stop=True)
                    rb_s = rbp.tile([128, 512], bf16, tag="rbs")
                    nc.vector.tensor_copy(out=rb_s, in_=rb_ps)
                    # rope: qa = qb*csA + rot(qb)*csB
                    rot = rwk.tile([128, 512], bf16, tag="rot")
                    for hh in (0, 64):
                        # 16-row rotate needs non-32-aligned partition bases:
                        # only DMA can address those
                        nc.gpsimd.dma_start(out=rot[hh + 0:hh + 16],
                                            in_=qb[hh + 16:hh + 32])
                        nc.gpsimd.dma_start(out=rot[hh + 16:hh + 32],
                                            in_=qb[hh + 0:hh + 16])
                        nc.vector.tensor_copy(out=rot[hh + 32:hh + 64],
                                              in_=qb[hh + 32:hh + 64])
                    nc.vector.tensor_mul(rot, rot, csB)
                    qa = rwk.tile([128, 512], bf16, tag="qa")
                    nc.vector.tensor_mul(qa, qb, csA)
                    nc.vector.tensor_add(qa, qa, rot)
                    return qa, rb_s

                for sb in range(SB):
                    s0 = sb * 512
                    hblk = hp.tile([128, KC, 512], bf16)
                    nc.sync.dma_start(
                        out=hblk,
                        in_=hT[:, s0:s0 + 512].rearrange("(c p) s -> p c s",
                                                         p=128))
                    # ---- q projection (4 chunks of 128 rows) ----
                    for m in range(4):
                        ps = pps.tile([128, 512], f32, tag="proj")
                        for kc in range(KC):
                            nc.tensor.matmul(ps, wq_sb[:, kc, m * 128:(m + 1) * 128],
                                             hblk[:, kc, :],
                                             start=(kc == 0), stop=(kc == KC - 1))
                        qa, rb_s = rope_norm(ps, csA_q[:, s0:s0 + 512],
                                             csB_q[:, s0:s0 + 512])
                        # heads 2m, 2m+1 -> row-half r=m//2, chunks 2*(m%2)+{0,1}
                        r = (m // 2) * 64
                        cb = 2 * (m % 2)
                        nc.vector.tensor_mul(
                            qT_sb[r:r + 64, cb, s0:s0 + 512],
                            qa[0:64, :], rb_s[0:64, :])
                        nc.vector.tensor_mul(
                            qT_sb[r:r + 64, cb + 1, s0:s0 + 512],
                            qa[64:128, :], rb_s[64:128, :])
                    # ---- k projection (1 chunk) ----
                    ps = pps.tile([128, 512], f32, tag="proj")
                    for kc in range(KC):
                        nc.tensor.matmul(ps, wk_sb[:, kc, :], hblk[:, kc, :],
                                         start=(kc == 0), stop=(kc == KC - 1))
                    ka, rb_s = rope_norm(ps, csA_k[:, s0:s0 + 512],
                                         csB_k[:, s0:s0 + 512])
                    nc.vector.tensor_mul(kT_sb[:, s0:s0 + 512], ka, rb_s)
                    # ---- v projection + transpose to natural layout ----
                    ps = pps.tile([128, 512], f32, tag="proj")
                    for kc in range(KC):
                        nc.tensor.matmul(ps, wv_sb[:, kc, :], hblk[:, kc, :],
                                         start=(kc == 0), stop=(kc == KC - 1))
                    vt = wk.tile([128, 512], bf16, tag="vt")
                    nc.scalar.activation(out=vt, in_=ps, func=Copy)
                    for ss in range(4):
                        tp = trps.tile([128, 128], bf16, tag="tp")
                        nc.tensor.transpose(tp, vt[:, ss * 128:(ss + 1) * 128],
                                            ident)
                        chunk = sb * 4 + ss
                        nc.vector.tensor_copy(out=v_sb[:, chunk, :, 0:HD],
                                              in_=tp.rearrange("p (kv d) -> p kv d",
                                                               kv=2))
                    # ---- gate projection -> raw staged into og_sb ----
                    for m in range(4):
                        ps = pps.tile([128, 512], f32, tag="proj")
                        for kc in range(KC):
                            nc.tensor.matmul(ps, wg_sb[:, kc, m * 128:(m + 1) * 128],
                                             hblk[:, kc, :],
                                             start=(kc == 0), stop=(kc == KC - 1))
                        nc.scalar.activation(out=og_sb[:, m, s0:s0 + 512],
                                             in_=ps, func=Copy)

            # batched sigmoid: og_sb (raw gate) -> g_sb; one table switch
            for m in range(4):
                nc.scalar.activation(out=g_sb[:, m, :], in_=og_sb[:, m, :],
                                     func=Sigmoid)

            if debug_dump:
                nc.sync.dma_start(out=dbg_g, in_=g_sb)

            # ============ phase 2: attention + interleaved o_proj ============
            with tc.tile_pool(name="probs", bufs=6) as prp, \
                 tc.tile_pool(name="att_sm", bufs=4) as asm, \
                 tc.tile_pool(name="ostg", bufs=4) as ostg, \
                 tc.tile_pool(name="scps", bufs=2, space=PSUM) as scps, \
                 tc.tile_pool(name="avps", bufs=2, space=PSUM) as avps:

                def emit_pv(av, kc, lb, loc, ptl, Q):
                    last_kc = 4 * (2 * Q + lb) + 3
                    for hh in range(2):
                        nc.tensor.matmul(
                            av[hh][:, lb, loc:512],
                            v_sb[:, kc, hh, :],
                            ptl[:, hh, loc:512],
                            start=(kc == 0), stop=(kc == last_kc))

                def attn_pair(hp_idx, Q):
                    """Heads hA=hp_idx (kv0, rows 0:64) and hB=hp_idx+4 (kv1,
                    rows 64:128), both free-dim chunk hp_idx; q block Q."""
                    q0 = Q * 1024
                    nkc = 8 * (Q + 1)
                    hc = [hp_idx // 2, hp_idx // 2 + 2]   # og/g chunk per head
                    rr = (hp_idx % 2) * 64                # og/g row half
                    av_a = avps.tile([65, 2, 512], f32, tag="av")
                    av_b = avps.tile([65, 2, 512], f32, tag="av")
                    av = [av_a, av_b]
                    # PV emission is deferred one slot: slot s+1's score MMs
                    # enter the PE FIFO before slot s's PV, so the PE is never
                    # head-of-line blocked on exp(s).
                    pv_pend = None
                    for kc in range(nkc):
                        s_c = max(0, kc * 128 - q0)   # first valid local col
                        for lb in range(2):
                            lo = lb * 512
                            if s_c >= lo + 512:
                                continue
                            st = max(s_c, lo)
                            sc2 = scps.tile([128, 2, 512], f32, tag="sc")
                            for hh in range(2):  # interleave -> row-tiled pair
                                p0 = hh * 64
                                nc.tensor.matmul(
                                    sc2[:, hh, st - lo:512],
                                    kT_sb[p0:p0 + 64, kc * 128:(kc + 1) * 128],
                                    qT_sb[p0:p0 + 64, hp_idx,
                                          q0 + st:q0 + lo + 512],
                                    start=True, stop=True)
                            # probs tile per (kc, lb): [*, head, col-in-block]
                            ptl = prp.tile([128, 2, 512], bf16, tag="ptl")
                            if st == lo:
                                # full slot: contiguous 1-region exp
                                nc.scalar.activation(
                                    out=ptl.rearrange("p a b -> p (a b)"),
                                    in_=sc2.rearrange("p a b -> p (a b)"),
                                    func=Exp, scale=SCALE)
                            else:
                                loc = st - lo
                                for hh in range(2):
                                    nc.scalar.activation(
                                        out=ptl[:, hh, loc:512],
                                        in_=sc2[:, hh, loc:512],
                                        func=Exp, scale=SCALE)
                            if lo <= s_c < lo + 512 and kc * 128 >= q0:
                                # boundary strip: keep k<=q inside cols
                                # [s_c, s_c+128) (local [s_c-lo, s_c-lo+128))
                                sl = s_c - lo
                                for hh in range(2):
                                    nc.vector.tensor_mul(
                                        ptl[:, hh, sl:sl + 128],
                                        ptl[:, hh, sl:sl + 128], triu)
                            if pv_pend is not None:
                                emit_pv(*pv_pend)
                            pv_pend = (av, kc, lb, st - lo, ptl, Q)
                    emit_pv(*pv_pend)
                    # flush av psum -> sbuf (bf16) so the psum slots free
                    # immediately; the normalize/gate tail is emitted LATER
                    # (after the next pair's kc loop) so Tile's position-keyed
                    # slot releases don't serialize pairs on the tail chain.
                    avs = []
                    for hh in range(2):
                        av_sb = asm.tile([65, 1024], bf16, tag="avs")
                        nc.vector.tensor_copy(
                            out=av_sb,
                            in_=av[hh][:, :, :].rearrange("p a b -> p (a b)"))
                        avs.append(av_sb)
                    return (avs, hp_idx, Q)

                def attn_tail(ctx):
                    avs, hp_idx, Q = ctx
                    q0 = Q * 1024
                    hc = [hp_idx // 2, hp_idx // 2 + 2]
                    rr = (hp_idx % 2) * 64
                    for hh in range(2):
                        av_sb = avs[hh]
                        # custom DVE op mishandles base_partition 64 and needs
                        # f32 input: stage the denominator row to base 0
                        den = asm.tile([1, 1024], f32, tag="den")
                        nc.vector.tensor_copy(out=den, in_=av_sb[64:65, :])
                        recip = asm.tile([1, 1024], f32, tag="recip")
                        nc.vector.reciprocal_approx_fast(out=recip, in_=den)
                        rbv = asm.tile([64, 1024], f32, tag="rbv")
                        nc.gpsimd.partition_broadcast(rbv, recip)
                        for lb in range(2):
                            qsl = slice(q0 + lb * 512, q0 + (lb + 1) * 512)
                            dst = og_sb[rr:rr + 64, hc[hh], qsl]
                            nc.vector.tensor_mul(
                                dst, av_sb[0:64, lb * 512:(lb + 1) * 512],
                                rbv[:, lb * 512:(lb + 1) * 512])
                            nc.vector.tensor_mul(
                                dst, dst, g_sb[rr:rr + 64, hc[hh], qsl])

                def oproj_m(m, nbs, use_act):
                    """o_proj chunk m for the two q blocks in nbs."""
                    po = scps.tile([128, 2, 512], f32, tag="sc")
                    for oc in range(4):
                        for j, nb in enumerate(nbs):
                            nc.tensor.matmul(
                                po[:, j, :], wo_sb[:, oc, m, :],
                                og_sb[:, oc, nb * 512:(nb + 1) * 512],
                                start=(oc == 0), stop=(oc == 3))
                    for j, nb in enumerate(nbs):
                        stg = ostg.tile([128, 512], bf16, tag="stg")
                        if use_act and j == 0:
                            # tail oproj: ACT is idle (exp done) -> share copies
                            nc.scalar.activation(out=stg, in_=po[:, j, :],
                                                 func=Copy)
                        else:
                            nc.vector.tensor_copy(out=stg, in_=po[:, j, :])
                        nc.sync.dma_start(
                            out=outT[m * 128:(m + 1) * 128,
                                     nb * 512:(nb + 1) * 512],
                            in_=stg)

                # software-pipeline: pair p's normalize tail is emitted after
                # pair p+1's kc loop; oproj for q blocks 0/1 is interleaved
                # between Q1 pairs so its matmuls fill exp-bound cycles
                pending = None
                for Q in (0, 1):
                    for hp_idx in range(4):
                        ctx = attn_pair(hp_idx, Q)
                        if pending is not None:
                            attn_tail(pending)
                        pending = ctx
                        if Q == 1:
                            for m in range(hp_idx * 4, hp_idx * 4 + 4):
                                oproj_m(m, (0, 1), use_act=False)
                attn_tail(pending)
                for m in range(KC):
                    oproj_m(m, (2, 3), use_act=True)

            if debug_dump:
                nc.sync.dma_start(out=dbg_q, in_=qT_sb)
                nc.sync.dma_start(out=dbg_k, in_=kT_sb)
                nc.sync.dma_start(out=dbg_v, in_=v_sb)
                nc.sync.dma_start(out=dbg_og, in_=og_sb)

    nc.compile()
    return nc


def _host_prep(hidden_states, cos, sin, Wq, Wk, Wv, Wg, Wo, q_norm_w, k_norm_w):
    """Build per-core input maps."""
    def cs_tables(cos_b, sin_b, w):
        # csA/csB [128, S]: row p -> head-local dim d = p % 64
        A = np.empty((128, S), np.float32)
        Bt = np.empty((128, S), np.float32)
        cosT = cos_b.T  # [32, S]
        sinT = sin_b.T
        for blk in (0, 64):
            A[blk + 0:blk + 32] = cosT * w[0:32, None]
            A[blk + 32:blk + 64] = w[32:64, None]
            Bt[blk + 0:blk + 16] = -sinT[0:16] * w[16:32, None]
            Bt[blk + 16:blk + 32] = sinT[16:32] * w[0:16, None]
            Bt[blk + 32:blk + 64] = 0.0
        return A.astype(BF16), Bt.astype(BF16)

    sel2_host = np.zeros((2, 128), np.float32)
    sel2_host[0, 0:64] = 1.0
    sel2_host[1, 64:128] = 1.0
    in_maps = []
    for c in range(NCORES):
        b, g = c // 4, c % 4
        qs = slice(g * QD, (g + 1) * QD)
        ks = slice(g * KD, (g + 1) * KD)
        csA_q, csB_q = cs_tables(cos[b], sin[b], np.asarray(q_norm_w))
        csA_k, csB_k = cs_tables(cos[b], sin[b], np.asarray(k_norm_w))
        in_maps.append({
            "hT": np.ascontiguousarray(hidden_states[b].T).astype(BF16),
            "wqT": np.ascontiguousarray(Wq[qs].T).astype(BF16),
            "wkT": np.ascontiguousarray(Wk[ks].T).astype(BF16),
            "wvT": np.ascontiguousarray(Wv[ks].T).astype(BF16),
            "wgT": np.ascontiguousarray(Wg[qs].T).astype(BF16),
            "woT": np.ascontiguousarray(Wo[:, qs].T).astype(BF16),
            "csAq": csA_q, "csBq": csB_q, "csAk": csA_k, "csBk": csB_k,
            "sel2": sel2_host,
        })
    return in_maps


def kernel(hidden_states, cos, sin, Wq, Wk, Wv, Wg, Wo, q_norm_w, k_norm_w):
    from concourse import bass_utils

    if "nc" not in _CACHE:
        _CACHE["nc"] = _build_bass()
    nc = _CACHE["nc"]

    in_maps = _host_prep(hidden_states, cos, sin, Wq, Wk, Wv, Wg, Wo,
                         q_norm_w, k_norm_w)

    trace = bool(int(os.environ.get("KERNEL_TRACE", "0")))
    kwargs = {}
    if trace:
        # the agent image's antenv lacks axon_hooks; recreate it from the
        # boot helper so run_bass_kernel_spmd(trace=True) can NTFF-profile
        try:
            import antenv.axon_hooks  # noqa: F401
        except ImportError:
            import types
            sys.path.insert(0, "/root/.axon_site")
            from trn_agent_boot.trn_boot import _ntff_profile_via_ctypes
            hook = _ntff_profile_via_ctypes("/opt/axon/libaxon_pjrt.so")
            mod = types.ModuleType("antenv.axon_hooks")
            mod.get_axon_ntff_profile_hook = lambda: hook
            sys.modules["antenv.axon_hooks"] = mod
        tmpdir = os.environ.get("KERNEL_TRACE_DIR") or None
        kwargs = dict(trace=True, tmpdir=tmpdir)
    res = bass_utils.run_bass_kernel_spmd(nc, in_maps,
                                          core_ids=list(range(NCORES)),
                                          **kwargs)
    if trace and res.exec_time_ns is not None:
        print(f"HW exec time: {res.exec_time_ns} ns")
        _CACHE["exec_time_ns"] = res.exec_time_ns

    out = np.zeros((B, S, HID), np.float32)
    for c in range(NCORES):
        b = c // 4
        out[b] += res.results[c]["outT"].T.astype(np.float32)
    return out


if __name__ == "__main__":
    rng = np.random.default_rng(0)
    hs = rng.standard_normal((B, S, HID), dtype=np.float32)
    cos = rng.random((B, S, ROPE), dtype=np.float32)
    sin = rng.random((B, S, ROPE), dtype=np.float32)
    out = kernel(hidden_states=hs, cos=cos, sin=sin,
                 Wq=rng.standard_normal((NH * HD, HID), dtype=np.float32) * 0.02,
                 Wk=rng.standard_normal((NKV * HD, HID), dtype=np.float32) * 0.02,
                 Wv=rng.standard_normal((NKV * HD, HID), dtype=np.float32) * 0.02,
                 Wg=rng.standard_normal((NH * HD, HID), dtype=np.float32) * 0.02,
                 Wo=rng.standard_normal((HID, NH * HD), dtype=np.float32) * 0.02,
                 q_norm_w=np.ones(HD, np.float32),
                 k_norm_w=np.ones(HD, np.float32))
    print(out.shape, out.dtype)


# revision 35
# speedup vs baseline: 1.4227x; 1.0542x over previous
"""GQA attention Trainium2 kernel (8 NeuronCores, SPMD, no collectives).

Sharding: 2-way data parallel (batch) x 4-way tensor parallel (heads).
Core c handles batch b=c//4 and head-group g=c%4 (8 q heads, 2 kv heads).
Each core produces a partial o_proj output (transposed, [HID, S] bf16);
the host sums the 4 partials per batch (f32) and transposes back.

On-device layout is feature-major ("transposed"): hidden is passed as
hT=[HID,S], projections produce qT/kT/gateT=[dim,S], attention scores are
computed as scoresT=[s_k,s_q] so softmax-exp output feeds the PV matmul
directly (lhsT = natural-layout V with an appended ones column that yields
the softmax denominator in psum row 64).

v2 changes vs baseline:
- rstd via ACT Rsqrt + matmul broadcast (sel2 [2,128] lhsT) instead of
  Sqrt + DVE reciprocal + gpsimd partition_broadcast (reciprocal was
  4.8us/instr, 171us total).
- rope math in bf16 (DVE 2x mode).
- gates staged raw into og_sb, sigmoid batched after phase 1 (avoids
  ACT table-set thrash: rsqrt set resident through phase 1).
- attention processed as head PAIRS (kv0 head rows 0-63, kv1 head rows
  64-127) with interleaved K=64 score matmuls -> concurrent row-tiled
  execution on the PE array (2x score throughput).
- causal trimming at 128-col granularity for scores/exp/PV; boundary
  128x128 strip masked by a DVE tril multiply (replaces gpsimd
  affine_select).
- softmax 1/denom via DVE reciprocal_approx_fast + matmul broadcast.
- o_proj split per q-half and emitted between attention halves so the
  PE has work while ACT grinds exp.
- outT in bf16 (host accumulates partials in f32).
"""

import os
import sys
import numpy as np

for _p in ("/opt/trn_rl_repo", "/root/.axon_site/_ro/trn_rl_repo"):
    if os.path.isdir(_p) and _p not in sys.path:
        sys.path.insert(0, _p)

import ml_dtypes

B, S, HID = 2, 2048, 2048
NH, NKV, HD = 32, 8, 64
ROPE = 32
EPS = 1e-6
SCALE = HD ** -0.5
NCORES = 8
QH = NH // 4      # 8 q heads per core
KVH = NKV // 4    # 2 kv heads per core
QD = QH * HD      # 512 per-core q dim
KD = KVH * HD     # 128 per-core kv dim
KC = HID // 128   # 16 contraction chunks
SB = S // 512     # 4 sequence blocks of 512
BF16 = ml_dtypes.bfloat16

_CACHE = {}


def _build_bass(debug_dump=False):
    import concourse.bass as bass
    from concourse import bacc, mybir, tile

    f32 = mybir.dt.float32
    bf16 = mybir.dt.bfloat16

    nc = bacc.Bacc("TRN2", target_bir_lowering=False, debug=False,
                   enable_asserts=False, num_devices=NCORES)

    hT = nc.dram_tensor("hT", [HID, S], bf16, kind="ExternalInput").ap()
    wqT = nc.dram_tensor("wqT", [HID, QD], bf16, kind="ExternalInput").ap()
    wkT = nc.dram_tensor("wkT", [HID, KD], bf16, kind="ExternalInput").ap()
    wvT = nc.dram_tensor("wvT", [HID, KD], bf16, kind="ExternalInput").ap()
    wgT = nc.dram_tensor("wgT", [HID, QD], bf16, kind="ExternalInput").ap()
    woT = nc.dram_tensor("woT", [QD, HID], bf16, kind="ExternalInput").ap()
    csAq = nc.dram_tensor("csAq", [128, S], bf16, kind="ExternalInput").ap()
    csBq = nc.dram_tensor("csBq", [128, S], bf16, kind="ExternalInput").ap()
    csAk = nc.dram_tensor("csAk", [128, S], bf16, kind="ExternalInput").ap()
    csBk = nc.dram_tensor("csBk", [128, S], bf16, kind="ExternalInput").ap()
    sel2d = nc.dram_tensor("sel2", [2, 128], f32, kind="ExternalInput").ap()
    outT = nc.dram_tensor("outT", [HID, S], bf16, kind="ExternalOutput").ap()
    if debug_dump:
        dbg_q = nc.dram_tensor("dbg_q", [128, 4, S], bf16, kind="ExternalOutput").ap()
        dbg_k = nc.dram_tensor("dbg_k", [128, S], bf16, kind="ExternalOutput").ap()
        dbg_v = nc.dram_tensor("dbg_v", [128, KC, KVH, HD + 1], bf16,
                               kind="ExternalOutput").ap()
        dbg_g = nc.dram_tensor("dbg_g", [128, 4, S], bf16, kind="ExternalOutput").ap()
        dbg_og = nc.dram_tensor("dbg_og", [128, 4, S], bf16,
                                kind="ExternalOutput").ap()

    Exp = mybir.ActivationFunctionType.Exp
    Sigmoid = mybir.ActivationFunctionType.Sigmoid
    Square = mybir.ActivationFunctionType.Square
    Sqrt = mybir.ActivationFunctionType.Sqrt
    Copy = mybir.ActivationFunctionType.Copy
    PSUM = bass.MemorySpace.PSUM

    with tile.TileContext(nc) as tc:
        # ---- persistent sbuf ----
        with tc.tile_pool(name="persist", bufs=1) as pp:
            # head h lives at partition rows (h//4)*64 (matching its kv head's
            # rows so matmul operand bases agree), free-dim chunk h%4
            # triu[p, j] = 1.0 where p <= j (causal keep-mask for the
            # scoresT boundary strip), 0 above
            triu = pp.tile([128, 128], bf16)
            qT_sb = pp.tile([128, 4, S], bf16)        # q (roped+normed)
            kT_sb = pp.tile([128, S], bf16)           # k (roped+normed)
            g_sb = pp.tile([128, 4, S], bf16)         # sigmoid(gate)
            v_sb = pp.tile([128, KC, KVH, HD + 1], bf16)  # natural V + ones col
            og_sb = pp.tile([128, 4, S], bf16)        # raw gate, then gated out

            nc.vector.memset(v_sb[:, :, :, HD:HD + 1], 1.0)

            # ================= phase 1: projections =================
            with tc.tile_pool(name="consts", bufs=1) as cp, \
                 tc.tile_pool(name="wts", bufs=1) as wp, \
                 tc.tile_pool(name="hblk", bufs=2) as hp, \
                 tc.tile_pool(name="work", bufs=3) as wk, \
                 tc.tile_pool(name="smallw", bufs=3) as smp, \
                 tc.tile_pool(name="rbpool", bufs=3) as rbp, \
                 tc.tile_pool(name="rwork", bufs=3) as rwk, \
                 tc.tile_pool(name="pps", bufs=2, space=PSUM) as pps, \
                 tc.tile_pool(name="sqps", bufs=2, space=PSUM) as sqps, \
                 tc.tile_pool(name="rbps", bufs=2, space=PSUM) as rbps, \
                 tc.tile_pool(name="trps", bufs=2, space=PSUM) as trps:

                # DMA order matters for the kernel preamble: wq + first hidden
                # block gate the first matmul, so they go first; wo is only
                # needed by o_proj and loads inside the attention scope.
                wq_sb = wp.tile([128, KC, QD], bf16)
                wk_sb = wp.tile([128, KC, KD], bf16)
                wv_sb = wp.tile([128, KC, KD], bf16)
                wg_sb = wp.tile([128, KC, QD], bf16)
                nc.sync.dma_start(out=wq_sb,
                                  in_=wqT.rearrange("(c p) m -> p c m", p=128))
                hblk0 = hp.tile([128, KC, 512], bf16, tag="hblk")
                nc.sync.dma_start(
                    out=hblk0,
                    in_=hT[:, 0:512].rearrange("(c p) s -> p c s", p=128))
                csA_q = cp.tile([128, S], bf16)
                csB_q = cp.tile([128, S], bf16)
                csA_k = cp.tile([128, S], bf16)
                csB_k = cp.tile([128, S], bf16)
                nc.sync.dma_start(out=csA_q, in_=csAq)
                nc.sync.dma_start(out=csB_q, in_=csBq)
                nc.sync.dma_start(out=csA_k, in_=csAk)
                nc.sync.dma_start(out=csB_k, in_=csBk)
                ident = cp.tile([128, 128], bf16)
                from concourse.masks import make_identity, make_upper_triangular
                make_identity(nc, ident)
                make_upper_triangular(nc, triu, val=1.0, diag=True)
                ones2 = cp.tile([128, 2], bf16)
                nc.vector.memset(ones2, 0.0)
                nc.vector.memset(ones2[0:64, 0:1], 1.0)
                nc.vector.memset(ones2[64:128, 1:2], 1.0)
                # sel2.T broadcast: row p of (sel2.T @ rstd) = rstd[p//64]
                sel2 = cp.tile([2, 128], f32)
                nc.sync.dma_start(out=sel2, in_=sel2d)
                eps_t = cp.tile([8, 1], f32)
                nc.vector.memset(eps_t, EPS)

                nc.sync.dma_start(out=wk_sb,
                                  in_=wkT.rearrange("(c p) m -> p c m", p=128))
                nc.sync.dma_start(out=wv_sb,
                                  in_=wvT.rearrange("(c p) m -> p c m", p=128))
                nc.sync.dma_start(out=wg_sb,
                                  in_=wgT.rearrange("(c p) m -> p c m", p=128))
                def rope_norm(ps, csA, csB):
                    """ps: psum [128,512] raw proj. Returns (qa bf16 roped,
                    rb_s bf16 [128,512] rstd broadcast)."""
                    sq_t = wk.tile([128, 512], bf16, tag="sq")
                    nc.scalar.activation(out=sq_t, in_=ps, func=Square)
                    qb = rwk.tile([128, 512], bf16, tag="qb")
                    nc.scalar.activation(out=qb, in_=ps, func=Copy)
                    # sum of squares per 64-row half via matmul
                    sq_ps = sqps.tile([2, 512], f32, tag="sqs")
                    nc.tensor.matmul(sq_ps, ones2, sq_t, start=True, stop=True)
                    sstd = smp.tile([2, 512], f32, tag="sstd")
                    nc.scalar.activation(out=sstd, in_=sq_ps, func=Sqrt,
                                         scale=1.0 / HD, bias=eps_t[0:2])
                    rstd = smp.tile([2, 512], f32, tag="rstd")
                    nc.vector.reciprocal_approx_fast(out=rstd, in_=sstd)
                    # broadcast rstd rows to halves via matmul
                    rb_ps = rbps.tile([128, 512], f32, tag="rb")
                    nc.tensor.matmul(rb_ps, sel2, rstd, start=True, stop=True)
                    rb_s = rbp.tile([128, 512], bf16, tag="rbs")
                    nc.vector.tensor_copy(out=rb_s, in_=rb_ps)
                    # rope: qa = qb*csA + rot(qb)*csB
                    rot = rwk.tile([128, 512], bf16, tag="rot")
                    for hh in (0, 64):
                        # 16-row rotate needs non-32-aligned partition bases:
                        # only DMA can address those
                        nc.gpsimd.dma_start(out=rot[hh + 0:hh + 16],
                                            in_=qb[hh + 16:hh + 32])
                        nc.gpsimd.dma_start(out=rot[hh + 16:hh + 32],
                                            in_=qb[hh + 0:hh + 16])
                        nc.vector.tensor_copy(out=rot[hh + 32:hh + 64],
                                              in_=qb[hh + 32:hh + 64])
                    nc.vector.tensor_mul(rot, rot, csB)
                    qa = rwk.tile([128, 512], bf16, tag="qa")
                    nc.vector.tensor_mul(qa, qb, csA)
                    nc.vector.tensor_add(qa, qa, rot)
                    return qa, rb_s

                for sb in range(SB):
                    s0 = sb * 512
                    if sb == 0:
                        hblk = hblk0
                    else:
                        hblk = hp.tile([128, KC, 512], bf16, tag="hblk")
                        nc.sync.dma_start(
                            out=hblk,
                            in_=hT[:, s0:s0 + 512].rearrange("(c p) s -> p c s",
                                                             p=128))
                    # ---- q projection (4 chunks of 128 rows) ----
                    for m in range(4):
                        ps = pps.tile([128, 512], f32, tag="proj")
                        for kc in range(KC):
                            nc.tensor.matmul(ps, wq_sb[:, kc, m * 128:(m + 1) * 128],
                                             hblk[:, kc, :],
                                             start=(kc == 0), stop=(kc == KC - 1))
                        qa, rb_s = rope_norm(ps, csA_q[:, s0:s0 + 512],
                                             csB_q[:, s0:s0 + 512])
                        # heads 2m, 2m+1 -> row-half r=m//2, chunks 2*(m%2)+{0,1}
                        r = (m // 2) * 64
                        cb = 2 * (m % 2)
                        nc.vector.tensor_mul(
                            qT_sb[r:r + 64, cb, s0:s0 + 512],
                            qa[0:64, :], rb_s[0:64, :])
                        nc.vector.tensor_mul(
                            qT_sb[r:r + 64, cb + 1, s0:s0 + 512],
                            qa[64:128, :], rb_s[64:128, :])
                    # ---- k projection (1 chunk) ----
                    ps = pps.tile([128, 512], f32, tag="proj")
                    for kc in range(KC):
                        nc.tensor.matmul(ps, wk_sb[:, kc, :], hblk[:, kc, :],
                                         start=(kc == 0), stop=(kc == KC - 1))
                    ka, rb_s = rope_norm(ps, csA_k[:, s0:s0 + 512],
                                         csB_k[:, s0:s0 + 512])
                    nc.vector.tensor_mul(kT_sb[:, s0:s0 + 512], ka, rb_s)
                    # ---- v projection + transpose to natural layout ----
                    ps = pps.tile([128, 512], f32, tag="proj")
                    for kc in range(KC):
                        nc.tensor.matmul(ps, wv_sb[:, kc, :], hblk[:, kc, :],
                                         start=(kc == 0), stop=(kc == KC - 1))
                    vt = wk.tile([128, 512], bf16, tag="vt")
                    nc.scalar.activation(out=vt, in_=ps, func=Copy)
                    for ss in range(4):
                        tp = trps.tile([128, 128], bf16, tag="tp")
                        nc.tensor.transpose(tp, vt[:, ss * 128:(ss + 1) * 128],
                                            ident)
                        chunk = sb * 4 + ss
                        nc.vector.tensor_copy(out=v_sb[:, chunk, :, 0:HD],
                                              in_=tp.rearrange("p (kv d) -> p kv d",
                                                               kv=2))
                    # ---- gate projection -> raw staged into og_sb ----
                    for m in range(4):
                        ps = pps.tile([128, 512], f32, tag="proj")
                        for kc in range(KC):
                            nc.tensor.matmul(ps, wg_sb[:, kc, m * 128:(m + 1) * 128],
                                             hblk[:, kc, :],
                                             start=(kc == 0), stop=(kc == KC - 1))
                        nc.scalar.activation(out=og_sb[:, m, s0:s0 + 512],
                                             in_=ps, func=Copy)

                # pre-warm the gpsimd broadcast ext-isa lib (~7us load) after
                # the last gpsimd dma_start, so attention's first
                # partition_broadcast doesn't pay it on the critical path
                warm_b = cp.tile([64, 16], f32)
                warm_s = cp.tile([1, 16], f32)
                nc.vector.memset(warm_s, 1.0)
                nc.gpsimd.partition_broadcast(warm_b, warm_s)

            # batched sigmoid: og_sb (raw gate) -> g_sb; one table switch
            for m in range(4):
                nc.scalar.activation(out=g_sb[:, m, :], in_=og_sb[:, m, :],
                                     func=Sigmoid)

            if debug_dump:
                nc.sync.dma_start(out=dbg_g, in_=g_sb)

            # ============ phase 2: attention + interleaved o_proj ============
            with tc.tile_pool(name="probs", bufs=6) as prp, \
                 tc.tile_pool(name="att_sm", bufs=4) as asm, \
                 tc.tile_pool(name="ostg", bufs=4) as ostg, \
                 tc.tile_pool(name="p2w", bufs=1) as p2w, \
                 tc.tile_pool(name="scps", bufs=2, space=PSUM) as scps, \
                 tc.tile_pool(name="avps", bufs=2, space=PSUM) as avps:

                # o_proj weights load in the background during attention Q0
                wo_sb = p2w.tile([128, 4, KC, 128], bf16)
                nc.sync.dma_start(out=wo_sb,
                                  in_=woT.rearrange("(c p) (mb mm) -> p c mb mm",
                                                    p=128, mm=128))

                def emit_pv(av, kc, lb, loc, ptl, Q):
                    last_kc = 4 * (2 * Q + lb) + 3
                    for hh in range(2):
                        nc.tensor.matmul(
                            av[hh][:, lb, loc:512],
                            v_sb[:, kc, hh, :],
                            ptl[:, hh, loc:512],
                            start=(kc == 0), stop=(kc == last_kc))

                def attn_pair(hp_idx, Q):
                    """Heads hA=hp_idx (kv0, rows 0:64) and hB=hp_idx+4 (kv1,
                    rows 64:128), both free-dim chunk hp_idx; q block Q."""
                    q0 = Q * 1024
                    nkc = 8 * (Q + 1)
                    hc = [hp_idx // 2, hp_idx // 2 + 2]   # og/g chunk per head
                    rr = (hp_idx % 2) * 64                # og/g row half
                    av_a = avps.tile([65, 2, 512], f32, tag="av")
                    av_b = avps.tile([65, 2, 512], f32, tag="av")
                    av = [av_a, av_b]
                    # PV emission is deferred one slot: slot s+1's score MMs
                    # enter the PE FIFO before slot s's PV, so the PE is never
                    # head-of-line blocked on exp(s).
                    pv_pend = None
                    for kc in range(nkc):
                        s_c = max(0, kc * 128 - q0)   # first valid local col
                        for lb in range(2):
                            lo = lb * 512
                            if s_c >= lo + 512:
                                continue
                            st = max(s_c, lo)
                            sc2 = scps.tile([128, 2, 512], f32, tag="sc")
                            for hh in range(2):  # interleave -> row-tiled pair
                                p0 = hh * 64
                                nc.tensor.matmul(
                                    sc2[:, hh, st - lo:512],
                                    kT_sb[p0:p0 + 64, kc * 128:(kc + 1) * 128],
                                    qT_sb[p0:p0 + 64, hp_idx,
                                          q0 + st:q0 + lo + 512],
                                    start=True, stop=True)
                            # probs tile per (kc, lb): [*, head, col-in-block]
                            ptl = prp.tile([128, 2, 512], bf16, tag="ptl")
                            if st == lo:
                                # full slot: contiguous 1-region exp
                                nc.scalar.activation(
                                    out=ptl.rearrange("p a b -> p (a b)"),
                                    in_=sc2.rearrange("p a b -> p (a b)"),
                                    func=Exp, scale=SCALE)
                            else:
                                loc = st - lo
                                for hh in range(2):
                                    nc.scalar.activation(
                                        out=ptl[:, hh, loc:512],
                                        in_=sc2[:, hh, loc:512],
                                        func=Exp, scale=SCALE)
                            if lo <= s_c < lo + 512 and kc * 128 >= q0:
                                # boundary strip: keep k<=q inside cols
                                # [s_c, s_c+128) (local [s_c-lo, s_c-lo+128))
                                sl = s_c - lo
                                for hh in range(2):
                                    nc.vector.tensor_mul(
                                        ptl[:, hh, sl:sl + 128],
                                        ptl[:, hh, sl:sl + 128], triu)
                            if pv_pend is not None:
                                emit_pv(*pv_pend)
                            pv_pend = (av, kc, lb, st - lo, ptl, Q)
                    emit_pv(*pv_pend)
                    # flush av psum -> sbuf (bf16) so the psum slots free
                    # immediately; the normalize/gate tail is emitted LATER
                    # (after the next pair's kc loop) so Tile's position-keyed
                    # slot releases don't serialize pairs on the tail chain.
                    avs = []
                    for hh in range(2):
                        av_sb = asm.tile([65, 1024], bf16, tag="avs")
                        nc.vector.tensor_copy(
                            out=av_sb,
                            in_=av[hh][:, :, :].rearrange("p a b -> p (a b)"))
                        avs.append(av_sb)
                    return (avs, hp_idx, Q)

                def attn_tail(ctx):
                    avs, hp_idx, Q = ctx
                    q0 = Q * 1024
                    hc = [hp_idx // 2, hp_idx // 2 + 2]
                    rr = (hp_idx % 2) * 64
                    for hh in range(2):
                        av_sb = avs[hh]
                        # custom DVE op mishandles base_partition 64 and needs
                        # f32 input: stage the denominator row to base 0
                        den = asm.tile([1, 1024], f32, tag="den")
                        nc.vector.tensor_copy(out=den, in_=av_sb[64:65, :])
                        recip = asm.tile([1, 1024], f32, tag="recip")
                        nc.vector.reciprocal_approx_fast(out=recip, in_=den)
                        rbv = asm.tile([64, 1024], f32, tag="rbv")
                        nc.gpsimd.partition_broadcast(rbv, recip)
                        for lb in range(2):
                            qsl = slice(q0 + lb * 512, q0 + (lb + 1) * 512)
                            dst = og_sb[rr:rr + 64, hc[hh], qsl]
                            nc.vector.tensor_mul(
                                dst, av_sb[0:64, lb * 512:(lb + 1) * 512],
                                rbv[:, lb * 512:(lb + 1) * 512])
                            nc.vector.tensor_mul(
                                dst, dst, g_sb[rr:rr + 64, hc[hh], qsl])

                def oproj_m(m, nbs, use_act):
                    """o_proj chunk m for the two q blocks in nbs."""
                    po = scps.tile([128, 2, 512], f32, tag="sc")
                    for oc in range(4):
                        for j, nb in enumerate(nbs):
                            nc.tensor.matmul(
                                po[:, j, :], wo_sb[:, oc, m, :],
                                og_sb[:, oc, nb * 512:(nb + 1) * 512],
                                start=(oc == 0), stop=(oc == 3))
                    for j, nb in enumerate(nbs):
                        stg = ostg.tile([128, 512], bf16, tag="stg")
                        if use_act and j == 0:
                            # tail oproj: ACT is idle (exp done) -> share copies
                            nc.scalar.activation(out=stg, in_=po[:, j, :],
                                                 func=Copy)
                        else:
                            nc.vector.tensor_copy(out=stg, in_=po[:, j, :])
                        nc.sync.dma_start(
                            out=outT[m * 128:(m + 1) * 128,
                                     nb * 512:(nb + 1) * 512],
                            in_=stg)

                # software-pipeline: pair p's normalize tail is emitted after
                # pair p+1's kc loop; oproj for q blocks 0/1 is interleaved
                # between Q1 pairs so its matmuls fill exp-bound cycles
                pending = None
                for Q in (0, 1):
                    for hp_idx in range(4):
                        ctx = attn_pair(hp_idx, Q)
                        if pending is not None:
                            attn_tail(pending)
                        pending = ctx
                        if Q == 1:
                            for m in range(hp_idx * 2, hp_idx * 2 + 2):
                                oproj_m(m, (0, 1), use_act=False)
                attn_tail(pending)
                for m in range(8, KC):
                    oproj_m(m, (0, 1), use_act=False)
                for m in range(KC):
                    oproj_m(m, (2, 3), use_act=True)

            if debug_dump:
                nc.sync.dma_start(out=dbg_q, in_=qT_sb)
                nc.sync.dma_start(out=dbg_k, in_=kT_sb)
                nc.sync.dma_start(out=dbg_v, in_=v_sb)
                nc.sync.dma_start(out=dbg_og, in_=og_sb)

    nc.compile()
    return nc


def _host_prep(hidden_states, cos, sin, Wq, Wk, Wv, Wg, Wo, q_norm_w, k_norm_w):
    """Build per-core input maps."""
    def cs_tables(cos_b, sin_b, w):
        # csA/csB [128, S]: row p -> head-local dim d = p % 64
        A = np.empty((128, S), np.float32)
        Bt = np.empty((128, S), np.float32)
        cosT = cos_b.T  # [32, S]
        sinT = sin_b.T
        for blk in (0, 64):
            A[blk + 0:blk + 32] = cosT * w[0:32, None]
            A[blk + 32:blk + 64] = w[32:64, None]
            Bt[blk + 0:blk + 16] = -sinT[0:16] * w[16:32, None]
            Bt[blk + 16:blk + 32] = sinT[16:32] * w[0:16, None]
            Bt[blk + 32:blk + 64] = 0.0
        return A.astype(BF16), Bt.astype(BF16)

    sel2_host = np.zeros((2, 128), np.float32)
    sel2_host[0, 0:64] = 1.0
    sel2_host[1, 64:128] = 1.0
    in_maps = []
    for c in range(NCORES):
        b, g = c // 4, c % 4
        qs = slice(g * QD, (g + 1) * QD)
        ks = slice(g * KD, (g + 1) * KD)
        csA_q, csB_q = cs_tables(cos[b], sin[b], np.asarray(q_norm_w))
        csA_k, csB_k = cs_tables(cos[b], sin[b], np.asarray(k_norm_w))
        in_maps.append({
            "hT": np.ascontiguousarray(hidden_states[b].T).astype(BF16),
            "wqT": np.ascontiguousarray(Wq[qs].T).astype(BF16),
            "wkT": np.ascontiguousarray(Wk[ks].T).astype(BF16),
            "wvT": np.ascontiguousarray(Wv[ks].T).astype(BF16),
            "wgT": np.ascontiguousarray(Wg[qs].T).astype(BF16),
            "woT": np.ascontiguousarray(Wo[:, qs].T).astype(BF16),
            "csAq": csA_q, "csBq": csB_q, "csAk": csA_k, "csBk": csB_k,
            "sel2": sel2_host,
        })
    return in_maps


def kernel(hidden_states, cos, sin, Wq, Wk, Wv, Wg, Wo, q_norm_w, k_norm_w):
    from concourse import bass_utils

    if "nc" not in _CACHE:
        _CACHE["nc"] = _build_bass()
    nc = _CACHE["nc"]

    in_maps = _host_prep(hidden_states, cos, sin, Wq, Wk, Wv, Wg, Wo,
                         q_norm_w, k_norm_w)

    trace = bool(int(os.environ.get("KERNEL_TRACE", "0")))
    kwargs = {}
    if trace:
        # the agent image's antenv lacks axon_hooks; recreate it from the
        # boot helper so run_bass_kernel_spmd(trace=True) can NTFF-profile
        try:
            import antenv.axon_hooks  # noqa: F401
        except ImportError:
            import types
            sys.path.insert(0, "/root/.axon_site")
            from trn_agent_boot.trn_boot import _ntff_profile_via_ctypes
            hook = _ntff_profile_via_ctypes("/opt/axon/libaxon_pjrt.so")
            mod = types.ModuleType("antenv.axon_hooks")
            mod.get_axon_ntff_profile_hook = lambda: hook
            sys.modules["antenv.axon_hooks"] = mod
        tmpdir = os.environ.get("KERNEL_TRACE_DIR") or None
        kwargs = dict(trace=True, tmpdir=tmpdir)
    res = bass_utils.run_bass_kernel_spmd(nc, in_maps,
                                          core_ids=list(range(NCORES)),
                                          **kwargs)
    if trace and res.exec_time_ns is not None:
        print(f"HW exec time: {res.exec_time_ns} ns")
        _CACHE["exec_time_ns"] = res.exec_time_ns

    out = np.zeros((B, S, HID), np.float32)
    for c in range(NCORES):
        b = c // 4
        out[b] += res.results[c]["outT"].T.astype(np.float32)
    return out


if __name__ == "__main__":
    rng = np.random.default_rng(0)
    hs = rng.standard_normal((B, S, HID), dtype=np.float32)
    cos = rng.random((B, S, ROPE), dtype=np.float32)
    sin = rng.random((B, S, ROPE), dtype=np.float32)
    out = kernel(hidden_states=hs, cos=cos, sin=sin,
                 Wq=rng.standard_normal((NH * HD, HID), dtype=np.float32) * 0.02,
                 Wk=rng.standard_normal((NKV * HD, HID), dtype=np.float32) * 0.02,
                 Wv=rng.standard_normal((NKV * HD, HID), dtype=np.float32) * 0.02,
                 Wg=rng.standard_normal((NH * HD, HID), dtype=np.float32) * 0.02,
                 Wo=rng.standard_normal((HID, NH * HD), dtype=np.float32) * 0.02,
                 q_norm_w=np.ones(HD, np.float32),
                 k_norm_w=np.ones(HD, np.float32))
    print(out.shape, out.dtype)


# revision 38
# speedup vs baseline: 1.4296x; 1.0048x over previous
"""GQA attention Trainium2 kernel (8 NeuronCores, SPMD, no collectives).

Sharding: 2-way data parallel (batch) x 4-way tensor parallel (heads).
Core c handles batch b=c//4 and head-group g=c%4 (8 q heads, 2 kv heads).
Each core produces a partial o_proj output (transposed, [HID, S] bf16);
the host sums the 4 partials per batch (f32) and transposes back.

On-device layout is feature-major ("transposed"): hidden is passed as
hT=[HID,S], projections produce qT/kT/gateT=[dim,S], attention scores are
computed as scoresT=[s_k,s_q] so softmax-exp output feeds the PV matmul
directly (lhsT = natural-layout V with an appended ones column that yields
the softmax denominator in psum row 64).

v2 changes vs baseline:
- rstd via ACT Rsqrt + matmul broadcast (sel2 [2,128] lhsT) instead of
  Sqrt + DVE reciprocal + gpsimd partition_broadcast (reciprocal was
  4.8us/instr, 171us total).
- rope math in bf16 (DVE 2x mode).
- gates staged raw into og_sb, sigmoid batched after phase 1 (avoids
  ACT table-set thrash: rsqrt set resident through phase 1).
- attention processed as head PAIRS (kv0 head rows 0-63, kv1 head rows
  64-127) with interleaved K=64 score matmuls -> concurrent row-tiled
  execution on the PE array (2x score throughput).
- causal trimming at 128-col granularity for scores/exp/PV; boundary
  128x128 strip masked by a DVE tril multiply (replaces gpsimd
  affine_select).
- softmax 1/denom via DVE reciprocal_approx_fast + matmul broadcast.
- o_proj split per q-half and emitted between attention halves so the
  PE has work while ACT grinds exp.
- outT in bf16 (host accumulates partials in f32).
"""

import os
import sys
import numpy as np

for _p in ("/opt/trn_rl_repo", "/root/.axon_site/_ro/trn_rl_repo"):
    if os.path.isdir(_p) and _p not in sys.path:
        sys.path.insert(0, _p)

import ml_dtypes

B, S, HID = 2, 2048, 2048
NH, NKV, HD = 32, 8, 64
ROPE = 32
EPS = 1e-6
SCALE = HD ** -0.5
NCORES = 8
QH = NH // 4      # 8 q heads per core
KVH = NKV // 4    # 2 kv heads per core
QD = QH * HD      # 512 per-core q dim
KD = KVH * HD     # 128 per-core kv dim
KC = HID // 128   # 16 contraction chunks
SB = S // 512     # 4 sequence blocks of 512
BF16 = ml_dtypes.bfloat16

_CACHE = {}


def _build_bass(debug_dump=False):
    import concourse.bass as bass
    from concourse import bacc, mybir, tile

    f32 = mybir.dt.float32
    bf16 = mybir.dt.bfloat16

    nc = bacc.Bacc("TRN2", target_bir_lowering=False, debug=False,
                   enable_asserts=False, num_devices=NCORES)

    hT = nc.dram_tensor("hT", [HID, S], bf16, kind="ExternalInput").ap()
    wqT = nc.dram_tensor("wqT", [HID, QD], bf16, kind="ExternalInput").ap()
    wkT = nc.dram_tensor("wkT", [HID, KD], bf16, kind="ExternalInput").ap()
    wvT = nc.dram_tensor("wvT", [HID, KD], bf16, kind="ExternalInput").ap()
    wgT = nc.dram_tensor("wgT", [HID, QD], bf16, kind="ExternalInput").ap()
    woT = nc.dram_tensor("woT", [QD, HID], bf16, kind="ExternalInput").ap()
    csAq = nc.dram_tensor("csAq", [128, S], bf16, kind="ExternalInput").ap()
    csBq = nc.dram_tensor("csBq", [128, S], bf16, kind="ExternalInput").ap()
    csAk = nc.dram_tensor("csAk", [128, S], bf16, kind="ExternalInput").ap()
    csBk = nc.dram_tensor("csBk", [128, S], bf16, kind="ExternalInput").ap()
    sel2d = nc.dram_tensor("sel2", [2, 128], f32, kind="ExternalInput").ap()
    outT = nc.dram_tensor("outT", [HID, S], bf16, kind="ExternalOutput").ap()
    if debug_dump:
        dbg_q = nc.dram_tensor("dbg_q", [128, 4, S], bf16, kind="ExternalOutput").ap()
        dbg_k = nc.dram_tensor("dbg_k", [128, S], bf16, kind="ExternalOutput").ap()
        dbg_v = nc.dram_tensor("dbg_v", [128, KC, KVH, HD + 1], bf16,
                               kind="ExternalOutput").ap()
        dbg_g = nc.dram_tensor("dbg_g", [128, 4, S], bf16, kind="ExternalOutput").ap()
        dbg_og = nc.dram_tensor("dbg_og", [128, 4, S], bf16,
                                kind="ExternalOutput").ap()

    Exp = mybir.ActivationFunctionType.Exp
    Sigmoid = mybir.ActivationFunctionType.Sigmoid
    Square = mybir.ActivationFunctionType.Square
    Sqrt = mybir.ActivationFunctionType.Sqrt
    Copy = mybir.ActivationFunctionType.Copy
    PSUM = bass.MemorySpace.PSUM

    with tile.TileContext(nc) as tc:
        # ---- persistent sbuf ----
        with tc.tile_pool(name="persist", bufs=1) as pp:
            # head h lives at partition rows (h//4)*64 (matching its kv head's
            # rows so matmul operand bases agree), free-dim chunk h%4
            # triu[p, j] = 1.0 where p <= j (causal keep-mask for the
            # scoresT boundary strip), 0 above
            triu = pp.tile([128, 128], bf16)
            qT_sb = pp.tile([128, 4, S], bf16)        # q (roped+normed)
            kT_sb = pp.tile([128, S], bf16)           # k (roped+normed)
            g_sb = pp.tile([128, 4, S], bf16)         # sigmoid(gate)
            v_sb = pp.tile([128, KC, KVH, HD + 1], bf16)  # natural V + ones col
            og_sb = pp.tile([128, 4, S], bf16)        # raw gate, then gated out

            nc.vector.memset(v_sb[:, :, :, HD:HD + 1], 1.0)

            # ================= phase 1: projections =================
            with tc.tile_pool(name="consts", bufs=1) as cp, \
                 tc.tile_pool(name="wts", bufs=1) as wp, \
                 tc.tile_pool(name="hblk", bufs=2) as hp, \
                 tc.tile_pool(name="work", bufs=3) as wk, \
                 tc.tile_pool(name="smallw", bufs=3) as smp, \
                 tc.tile_pool(name="rbpool", bufs=3) as rbp, \
                 tc.tile_pool(name="rwork", bufs=3) as rwk, \
                 tc.tile_pool(name="pps", bufs=2, space=PSUM) as pps, \
                 tc.tile_pool(name="sqps", bufs=2, space=PSUM) as sqps, \
                 tc.tile_pool(name="rbps", bufs=2, space=PSUM) as rbps, \
                 tc.tile_pool(name="trps", bufs=2, space=PSUM) as trps:

                # DMA order matters for the kernel preamble: wq + first hidden
                # block gate the first matmul, so they go first; wo is only
                # needed by o_proj and loads inside the attention scope.
                wq_sb = wp.tile([128, KC, QD], bf16)
                wk_sb = wp.tile([128, KC, KD], bf16)
                wv_sb = wp.tile([128, KC, KD], bf16)
                wg_sb = wp.tile([128, KC, QD], bf16)
                nc.sync.dma_start(out=wq_sb,
                                  in_=wqT.rearrange("(c p) m -> p c m", p=128))
                hblk0 = hp.tile([128, KC, 512], bf16, tag="hblk")
                nc.sync.dma_start(
                    out=hblk0,
                    in_=hT[:, 0:512].rearrange("(c p) s -> p c s", p=128))
                csA_q = cp.tile([128, S], bf16)
                csB_q = cp.tile([128, S], bf16)
                csA_k = cp.tile([128, S], bf16)
                csB_k = cp.tile([128, S], bf16)
                nc.sync.dma_start(out=csA_q, in_=csAq)
                nc.sync.dma_start(out=csB_q, in_=csBq)
                nc.sync.dma_start(out=csA_k, in_=csAk)
                nc.sync.dma_start(out=csB_k, in_=csBk)
                ident = cp.tile([128, 128], bf16)
                from concourse.masks import make_identity, make_upper_triangular
                make_identity(nc, ident)
                make_upper_triangular(nc, triu, val=1.0, diag=True)
                ones2 = cp.tile([128, 2], bf16)
                nc.vector.memset(ones2, 0.0)
                nc.vector.memset(ones2[0:64, 0:1], 1.0)
                nc.vector.memset(ones2[64:128, 1:2], 1.0)
                # sel2.T broadcast: row p of (sel2.T @ rstd) = rstd[p//64]
                sel2 = cp.tile([2, 128], f32)
                nc.sync.dma_start(out=sel2, in_=sel2d)
                eps_t = cp.tile([8, 1], f32)
                nc.vector.memset(eps_t, EPS)

                nc.sync.dma_start(out=wk_sb,
                                  in_=wkT.rearrange("(c p) m -> p c m", p=128))
                nc.sync.dma_start(out=wv_sb,
                                  in_=wvT.rearrange("(c p) m -> p c m", p=128))
                nc.sync.dma_start(out=wg_sb,
                                  in_=wgT.rearrange("(c p) m -> p c m", p=128))
                def rope_norm(ps, csA, csB):
                    """ps: psum [128,512] raw proj. Returns (qa bf16 roped,
                    rb_s bf16 [128,512] rstd broadcast)."""
                    sq_t = wk.tile([128, 512], bf16, tag="sq")
                    nc.scalar.activation(out=sq_t, in_=ps, func=Square)
                    qb = rwk.tile([128, 512], bf16, tag="qb")
                    nc.scalar.activation(out=qb, in_=ps, func=Copy)
                    # sum of squares per 64-row half via matmul
                    sq_ps = sqps.tile([2, 512], f32, tag="sqs")
                    nc.tensor.matmul(sq_ps, ones2, sq_t, start=True, stop=True)
                    sstd = smp.tile([2, 512], f32, tag="sstd")
                    nc.scalar.activation(out=sstd, in_=sq_ps, func=Sqrt,
                                         scale=1.0 / HD, bias=eps_t[0:2])
                    rstd = smp.tile([2, 512], f32, tag="rstd")
                    nc.vector.reciprocal_approx_fast(out=rstd, in_=sstd)
                    # broadcast rstd rows to halves via matmul
                    rb_ps = rbps.tile([128, 512], f32, tag="rb")
                    nc.tensor.matmul(rb_ps, sel2, rstd, start=True, stop=True)
                    rb_s = rbp.tile([128, 512], bf16, tag="rbs")
                    nc.vector.tensor_copy(out=rb_s, in_=rb_ps)
                    # rope: qa = qb*csA + rot(qb)*csB
                    rot = rwk.tile([128, 512], bf16, tag="rot")
                    for hh in (0, 64):
                        # 16-row rotate needs non-32-aligned partition bases:
                        # only DMA can address those
                        nc.gpsimd.dma_start(out=rot[hh + 0:hh + 16],
                                            in_=qb[hh + 16:hh + 32])
                        nc.gpsimd.dma_start(out=rot[hh + 16:hh + 32],
                                            in_=qb[hh + 0:hh + 16])
                        nc.vector.tensor_copy(out=rot[hh + 32:hh + 64],
                                              in_=qb[hh + 32:hh + 64])
                    nc.vector.tensor_mul(rot, rot, csB)
                    qa = rwk.tile([128, 512], bf16, tag="qa")
                    nc.vector.tensor_mul(qa, qb, csA)
                    nc.vector.tensor_add(qa, qa, rot)
                    return qa, rb_s

                for sb in range(SB):
                    s0 = sb * 512
                    if sb == 0:
                        hblk = hblk0
                    else:
                        hblk = hp.tile([128, KC, 512], bf16, tag="hblk")
                        nc.sync.dma_start(
                            out=hblk,
                            in_=hT[:, s0:s0 + 512].rearrange("(c p) s -> p c s",
                                                             p=128))
                    # ---- q projection (4 chunks of 128 rows) ----
                    for m in range(4):
                        ps = pps.tile([128, 512], f32, tag="proj")
                        for kc in range(KC):
                            nc.tensor.matmul(ps, wq_sb[:, kc, m * 128:(m + 1) * 128],
                                             hblk[:, kc, :],
                                             start=(kc == 0), stop=(kc == KC - 1))
                        qa, rb_s = rope_norm(ps, csA_q[:, s0:s0 + 512],
                                             csB_q[:, s0:s0 + 512])
                        # heads 2m, 2m+1 -> row-half r=m//2, chunks 2*(m%2)+{0,1}
                        r = (m // 2) * 64
                        cb = 2 * (m % 2)
                        nc.vector.tensor_mul(
                            qT_sb[r:r + 64, cb, s0:s0 + 512],
                            qa[0:64, :], rb_s[0:64, :])
                        nc.vector.tensor_mul(
                            qT_sb[r:r + 64, cb + 1, s0:s0 + 512],
                            qa[64:128, :], rb_s[64:128, :])
                    # ---- k projection (1 chunk) ----
                    ps = pps.tile([128, 512], f32, tag="proj")
                    for kc in range(KC):
                        nc.tensor.matmul(ps, wk_sb[:, kc, :], hblk[:, kc, :],
                                         start=(kc == 0), stop=(kc == KC - 1))
                    ka, rb_s = rope_norm(ps, csA_k[:, s0:s0 + 512],
                                         csB_k[:, s0:s0 + 512])
                    nc.vector.tensor_mul(kT_sb[:, s0:s0 + 512], ka, rb_s)
                    # ---- v projection + transpose to natural layout ----
                    ps = pps.tile([128, 512], f32, tag="proj")
                    for kc in range(KC):
                        nc.tensor.matmul(ps, wv_sb[:, kc, :], hblk[:, kc, :],
                                         start=(kc == 0), stop=(kc == KC - 1))
                    vt = wk.tile([128, 512], bf16, tag="vt")
                    nc.scalar.activation(out=vt, in_=ps, func=Copy)
                    for ss in range(4):
                        tp = trps.tile([128, 128], bf16, tag="tp")
                        nc.tensor.transpose(tp, vt[:, ss * 128:(ss + 1) * 128],
                                            ident)
                        chunk = sb * 4 + ss
                        nc.vector.tensor_copy(out=v_sb[:, chunk, :, 0:HD],
                                              in_=tp.rearrange("p (kv d) -> p kv d",
                                                               kv=2))
                    # ---- gate projection -> raw staged into og_sb ----
                    for m in range(4):
                        ps = pps.tile([128, 512], f32, tag="proj")
                        for kc in range(KC):
                            nc.tensor.matmul(ps, wg_sb[:, kc, m * 128:(m + 1) * 128],
                                             hblk[:, kc, :],
                                             start=(kc == 0), stop=(kc == KC - 1))
                        nc.scalar.activation(out=og_sb[:, m, s0:s0 + 512],
                                             in_=ps, func=Copy)

                # pre-warm the gpsimd broadcast ext-isa lib (~7us load) after
                # the last gpsimd dma_start, so attention's first
                # partition_broadcast doesn't pay it on the critical path
                warm_b = cp.tile([64, 16], f32)
                warm_s = cp.tile([1, 16], f32)
                nc.vector.memset(warm_s, 1.0)
                nc.gpsimd.partition_broadcast(warm_b, warm_s)

            # batched sigmoid: og_sb (raw gate) -> g_sb; one table switch
            for m in range(4):
                nc.scalar.activation(out=g_sb[:, m, :], in_=og_sb[:, m, :],
                                     func=Sigmoid)

            if debug_dump:
                nc.sync.dma_start(out=dbg_g, in_=g_sb)

            # ============ phase 2: attention + interleaved o_proj ============
            with tc.tile_pool(name="probs", bufs=6) as prp, \
                 tc.tile_pool(name="att_sm", bufs=4) as asm, \
                 tc.tile_pool(name="ostg", bufs=4) as ostg, \
                 tc.tile_pool(name="p2w", bufs=1) as p2w, \
                 tc.tile_pool(name="scps", bufs=2, space=PSUM) as scps, \
                 tc.tile_pool(name="avps", bufs=2, space=PSUM) as avps, \
                 tc.tile_pool(name="pops", bufs=1, space=PSUM) as pops:

                # o_proj weights load in the background during attention Q0
                wo_sb = p2w.tile([128, 4, KC, 128], bf16)
                nc.sync.dma_start(out=wo_sb,
                                  in_=woT.rearrange("(c p) (mb mm) -> p c mb mm",
                                                    p=128, mm=128))

                def emit_pv(av, kc, loc, ptl, qb):
                    last_kc = 4 * qb + 3
                    for hh in range(2):
                        nc.tensor.matmul(
                            av[hh][:, loc:512],
                            v_sb[:, kc, hh, :],
                            ptl[:, hh, loc:512],
                            start=(kc == 0), stop=(kc == last_kc))

                def attn_pair(hp_idx, qb):
                    """Heads hA=hp_idx (kv0, rows 0:64) and hB=hp_idx+4 (kv1,
                    rows 64:128), both free-dim chunk hp_idx; 512-col q block
                    qb (av is one psum bank per head)."""
                    q0 = qb * 512
                    av_a = avps.tile([65, 512], f32, tag="av")
                    av_b = avps.tile([65, 512], f32, tag="av")
                    av = [av_a, av_b]
                    # PV emission is deferred one slot: slot s+1's score MMs
                    # enter the PE FIFO before slot s's PV, so the PE is never
                    # head-of-line blocked on exp(s).
                    pv_pend = None
                    for kc in range(4 * qb + 4):
                        st = max(0, kc * 128 - q0)   # first valid local col
                        sc2 = scps.tile([128, 2, 512], f32, tag="sc")
                        for hh in range(2):  # interleave -> row-tiled pair
                            p0 = hh * 64
                            nc.tensor.matmul(
                                sc2[:, hh, st:512],
                                kT_sb[p0:p0 + 64, kc * 128:(kc + 1) * 128],
                                qT_sb[p0:p0 + 64, hp_idx,
                                      q0 + st:q0 + 512],
                                start=True, stop=True)
                        # probs tile per kc: [*, head, col-in-block]
                        ptl = prp.tile([128, 2, 512], bf16, tag="ptl")
                        if st == 0:
                            # full slot: contiguous 1-region exp
                            nc.scalar.activation(
                                out=ptl.rearrange("p a b -> p (a b)"),
                                in_=sc2.rearrange("p a b -> p (a b)"),
                                func=Exp, scale=SCALE)
                        else:
                            for hh in range(2):
                                nc.scalar.activation(
                                    out=ptl[:, hh, st:512],
                                    in_=sc2[:, hh, st:512],
                                    func=Exp, scale=SCALE)
                        if kc * 128 >= q0:
                            # boundary strip: keep k<=q inside [st, st+128)
                            for hh in range(2):
                                nc.vector.tensor_mul(
                                    ptl[:, hh, st:st + 128],
                                    ptl[:, hh, st:st + 128], triu)
                        if pv_pend is not None:
                            emit_pv(*pv_pend)
                        pv_pend = (av, kc, st, ptl, qb)
                    emit_pv(*pv_pend)
                    # flush av psum -> sbuf (bf16) so the psum slots free
                    # immediately; the normalize/gate tail is emitted LATER
                    # (after the next pair's kc loop) so Tile's position-keyed
                    # slot releases don't serialize pairs on the tail chain.
                    avs = []
                    for hh in range(2):
                        av_sb = asm.tile([65, 512], bf16, tag="avs")
                        nc.vector.tensor_copy(out=av_sb, in_=av[hh])
                        avs.append(av_sb)
                    return (avs, hp_idx, qb)

                def attn_tail(ctx):
                    avs, hp_idx, qb = ctx
                    qsl = slice(qb * 512, (qb + 1) * 512)
                    hc = [hp_idx // 2, hp_idx // 2 + 2]
                    rr = (hp_idx % 2) * 64
                    for hh in range(2):
                        av_sb = avs[hh]
                        # custom DVE op mishandles base_partition 64 and needs
                        # f32 input: stage the denominator row to base 0
                        den = asm.tile([1, 512], f32, tag="den")
                        nc.vector.tensor_copy(out=den, in_=av_sb[64:65, :])
                        recip = asm.tile([1, 512], f32, tag="recip")
                        nc.vector.reciprocal_approx_fast(out=recip, in_=den)
                        rbv = asm.tile([64, 512], f32, tag="rbv")
                        nc.gpsimd.partition_broadcast(rbv, recip)
                        dst = og_sb[rr:rr + 64, hc[hh], qsl]
                        nc.vector.tensor_mul(dst, av_sb[0:64, :], rbv)
                        nc.vector.tensor_mul(
                            dst, dst, g_sb[rr:rr + 64, hc[hh], qsl])

                def oproj_m(m, nbs, use_act):
                    """o_proj chunk m for the two q blocks in nbs. Uses the
                    dedicated po psum banks, so it never steals attention's
                    score-psum slots."""
                    po = pops.tile([128, 2, 512], f32, tag="po")
                    for oc in range(4):
                        for j, nb in enumerate(nbs):
                            nc.tensor.matmul(
                                po[:, j, :], wo_sb[:, oc, m, :],
                                og_sb[:, oc, nb * 512:(nb + 1) * 512],
                                start=(oc == 0), stop=(oc == 3))
                    for j, nb in enumerate(nbs):
                        stg = ostg.tile([128, 512], bf16, tag="stg")
                        if use_act and j == 0:
                            # tail oproj: ACT is idle (exp done) -> share copies
                            nc.scalar.activation(out=stg, in_=po[:, j, :],
                                                 func=Copy)
                        else:
                            nc.vector.tensor_copy(out=stg, in_=po[:, j, :])
                        nc.sync.dma_start(
                            out=outT[m * 128:(m + 1) * 128,
                                     nb * 512:(nb + 1) * 512],
                            in_=stg)

                # software-pipeline: pair p's normalize tail is emitted after
                # pair p+1's kc loop; oproj for q blocks 0/1 is interleaved
                # into q blocks 2/3 where it fills exp-bound PE idle
                pending = None
                op01 = 0
                for qb in range(4):
                    for hp_idx in range(4):
                        ctx = attn_pair(hp_idx, qb)
                        if pending is not None:
                            attn_tail(pending)
                        pending = ctx
                        if qb == 2 and hp_idx >= 1:
                            for m in range(op01, op01 + 2):
                                oproj_m(m, (0, 1), use_act=False)
                            op01 += 2
                        elif qb == 3:
                            for m in range(op01, min(op01 + 3, KC)):
                                oproj_m(m, (0, 1), use_act=False)
                            op01 = min(op01 + 3, KC)
                attn_tail(pending)
                for m in range(op01, KC):
                    oproj_m(m, (0, 1), use_act=False)
                for m in range(KC):
                    oproj_m(m, (2, 3), use_act=True)

            if debug_dump:
                nc.sync.dma_start(out=dbg_q, in_=qT_sb)
                nc.sync.dma_start(out=dbg_k, in_=kT_sb)
                nc.sync.dma_start(out=dbg_v, in_=v_sb)
                nc.sync.dma_start(out=dbg_og, in_=og_sb)

    nc.compile()
    return nc


def _host_prep(hidden_states, cos, sin, Wq, Wk, Wv, Wg, Wo, q_norm_w, k_norm_w):
    """Build per-core input maps."""
    def cs_tables(cos_b, sin_b, w):
        # csA/csB [128, S]: row p -> head-local dim d = p % 64
        A = np.empty((128, S), np.float32)
        Bt = np.empty((128, S), np.float32)
        cosT = cos_b.T  # [32, S]
        sinT = sin_b.T
        for blk in (0, 64):
            A[blk + 0:blk + 32] = cosT * w[0:32, None]
            A[blk + 32:blk + 64] = w[32:64, None]
            Bt[blk + 0:blk + 16] = -sinT[0:16] * w[16:32, None]
            Bt[blk + 16:blk + 32] = sinT[16:32] * w[0:16, None]
            Bt[blk + 32:blk + 64] = 0.0
        return A.astype(BF16), Bt.astype(BF16)

    sel2_host = np.zeros((2, 128), np.float32)
    sel2_host[0, 0:64] = 1.0
    sel2_host[1, 64:128] = 1.0
    in_maps = []
    for c in range(NCORES):
        b, g = c // 4, c % 4
        qs = slice(g * QD, (g + 1) * QD)
        ks = slice(g * KD, (g + 1) * KD)
        csA_q, csB_q = cs_tables(cos[b], sin[b], np.asarray(q_norm_w))
        csA_k, csB_k = cs_tables(cos[b], sin[b], np.asarray(k_norm_w))
        in_maps.append({
            "hT": np.ascontiguousarray(hidden_states[b].T).astype(BF16),
            "wqT": np.ascontiguousarray(Wq[qs].T).astype(BF16),
            "wkT": np.ascontiguousarray(Wk[ks].T).astype(BF16),
            "wvT": np.ascontiguousarray(Wv[ks].T).astype(BF16),
            "wgT": np.ascontiguousarray(Wg[qs].T).astype(BF16),
            "woT": np.ascontiguousarray(Wo[:, qs].T).astype(BF16),
            "csAq": csA_q, "csBq": csB_q, "csAk": csA_k, "csBk": csB_k,
            "sel2": sel2_host,
        })
    return in_maps


def kernel(hidden_states, cos, sin, Wq, Wk, Wv, Wg, Wo, q_norm_w, k_norm_w):
    from concourse import bass_utils

    if "nc" not in _CACHE:
        _CACHE["nc"] = _build_bass()
    nc = _CACHE["nc"]

    in_maps = _host_prep(hidden_states, cos, sin, Wq, Wk, Wv, Wg, Wo,
                         q_norm_w, k_norm_w)

    trace = bool(int(os.environ.get("KERNEL_TRACE", "0")))
    kwargs = {}
    if trace:
        # the agent image's antenv lacks axon_hooks; recreate it from the
        # boot helper so run_bass_kernel_spmd(trace=True) can NTFF-profile
        try:
            import antenv.axon_hooks  # noqa: F401
        except ImportError:
            import types
            sys.path.insert(0, "/root/.axon_site")
            from trn_agent_boot.trn_boot import _ntff_profile_via_ctypes
            hook = _ntff_profile_via_ctypes("/opt/axon/libaxon_pjrt.so")
            mod = types.ModuleType("antenv.axon_hooks")
            mod.get_axon_ntff_profile_hook = lambda: hook
            sys.modules["antenv.axon_hooks"] = mod
        tmpdir = os.environ.get("KERNEL_TRACE_DIR") or None
        kwargs = dict(trace=True, tmpdir=tmpdir)
    res = bass_utils.run_bass_kernel_spmd(nc, in_maps,
                                          core_ids=list(range(NCORES)),
                                          **kwargs)
    if trace and res.exec_time_ns is not None:
        print(f"HW exec time: {res.exec_time_ns} ns")
        _CACHE["exec_time_ns"] = res.exec_time_ns

    out = np.zeros((B, S, HID), np.float32)
    for c in range(NCORES):
        b = c // 4
        out[b] += res.results[c]["outT"].T.astype(np.float32)
    return out


if __name__ == "__main__":
    rng = np.random.default_rng(0)
    hs = rng.standard_normal((B, S, HID), dtype=np.float32)
    cos = rng.random((B, S, ROPE), dtype=np.float32)
    sin = rng.random((B, S, ROPE), dtype=np.float32)
    out = kernel(hidden_states=hs, cos=cos, sin=sin,
                 Wq=rng.standard_normal((NH * HD, HID), dtype=np.float32) * 0.02,
                 Wk=rng.standard_normal((NKV * HD, HID), dtype=np.float32) * 0.02,
                 Wv=rng.standard_normal((NKV * HD, HID), dtype=np.float32) * 0.02,
                 Wg=rng.standard_normal((NH * HD, HID), dtype=np.float32) * 0.02,
                 Wo=rng.standard_normal((HID, NH * HD), dtype=np.float32) * 0.02,
                 q_norm_w=np.ones(HD, np.float32),
                 k_norm_w=np.ones(HD, np.float32))
    print(out.shape, out.dtype)


# revision 41
# speedup vs baseline: 1.4819x; 1.0366x over previous
"""GQA attention Trainium2 kernel (8 NeuronCores, SPMD, no collectives).

Sharding: 2-way data parallel (batch) x 4-way tensor parallel (heads).
Core c handles batch b=c//4 and head-group g=c%4 (8 q heads, 2 kv heads).
Each core produces a partial o_proj output (transposed, [HID, S] bf16);
the host sums the 4 partials per batch (f32) and transposes back.

On-device layout is feature-major ("transposed"): hidden is passed as
hT=[HID,S], projections produce qT/kT/gateT=[dim,S], attention scores are
computed as scoresT=[s_k,s_q] so softmax-exp output feeds the PV matmul
directly (lhsT = natural-layout V with an appended ones column that yields
the softmax denominator in psum row 64).

v2 changes vs baseline:
- rstd via ACT Rsqrt + matmul broadcast (sel2 [2,128] lhsT) instead of
  Sqrt + DVE reciprocal + gpsimd partition_broadcast (reciprocal was
  4.8us/instr, 171us total).
- rope math in bf16 (DVE 2x mode).
- gates staged raw into og_sb, sigmoid batched after phase 1 (avoids
  ACT table-set thrash: rsqrt set resident through phase 1).
- attention processed as head PAIRS (kv0 head rows 0-63, kv1 head rows
  64-127) with interleaved K=64 score matmuls -> concurrent row-tiled
  execution on the PE array (2x score throughput).
- causal trimming at 128-col granularity for scores/exp/PV; boundary
  128x128 strip masked by a DVE tril multiply (replaces gpsimd
  affine_select).
- softmax 1/denom via DVE reciprocal_approx_fast + matmul broadcast.
- o_proj split per q-half and emitted between attention halves so the
  PE has work while ACT grinds exp.
- outT in bf16 (host accumulates partials in f32).
"""

import os
import sys
import numpy as np

for _p in ("/opt/trn_rl_repo", "/root/.axon_site/_ro/trn_rl_repo"):
    if os.path.isdir(_p) and _p not in sys.path:
        sys.path.insert(0, _p)

import ml_dtypes

B, S, HID = 2, 2048, 2048
NH, NKV, HD = 32, 8, 64
ROPE = 32
EPS = 1e-6
SCALE = HD ** -0.5
NCORES = 8
QH = NH // 4      # 8 q heads per core
KVH = NKV // 4    # 2 kv heads per core
QD = QH * HD      # 512 per-core q dim
KD = KVH * HD     # 128 per-core kv dim
KC = HID // 128   # 16 contraction chunks
SB = S // 512     # 4 sequence blocks of 512
BF16 = ml_dtypes.bfloat16

_CACHE = {}


def _build_bass(debug_dump=False):
    import concourse.bass as bass
    from concourse import bacc, mybir, tile

    f32 = mybir.dt.float32
    bf16 = mybir.dt.bfloat16

    nc = bacc.Bacc("TRN2", target_bir_lowering=False, debug=False,
                   enable_asserts=False, num_devices=NCORES)

    hT = nc.dram_tensor("hT", [HID, S], bf16, kind="ExternalInput").ap()
    wqT = nc.dram_tensor("wqT", [HID, QD], bf16, kind="ExternalInput").ap()
    wkT = nc.dram_tensor("wkT", [HID, KD], bf16, kind="ExternalInput").ap()
    wvT = nc.dram_tensor("wvT", [HID, KD], bf16, kind="ExternalInput").ap()
    wgT = nc.dram_tensor("wgT", [HID, QD], bf16, kind="ExternalInput").ap()
    woT = nc.dram_tensor("woT", [QD, HID], bf16, kind="ExternalInput").ap()
    csAq = nc.dram_tensor("csAq", [128, S], bf16, kind="ExternalInput").ap()
    csBq = nc.dram_tensor("csBq", [128, S], bf16, kind="ExternalInput").ap()
    csAk = nc.dram_tensor("csAk", [128, S], bf16, kind="ExternalInput").ap()
    csBk = nc.dram_tensor("csBk", [128, S], bf16, kind="ExternalInput").ap()
    sel2d = nc.dram_tensor("sel2", [2, 128], f32, kind="ExternalInput").ap()
    outT = nc.dram_tensor("outT", [HID, S], bf16, kind="ExternalOutput").ap()
    if debug_dump:
        dbg_q = nc.dram_tensor("dbg_q", [128, 4, S], bf16, kind="ExternalOutput").ap()
        dbg_k = nc.dram_tensor("dbg_k", [128, S], bf16, kind="ExternalOutput").ap()
        dbg_v = nc.dram_tensor("dbg_v", [128, KC, KVH, HD + 1], bf16,
                               kind="ExternalOutput").ap()
        dbg_g = nc.dram_tensor("dbg_g", [128, 4, S], bf16, kind="ExternalOutput").ap()
        dbg_og = nc.dram_tensor("dbg_og", [128, 4, S], bf16,
                                kind="ExternalOutput").ap()

    Exp = mybir.ActivationFunctionType.Exp
    Sigmoid = mybir.ActivationFunctionType.Sigmoid
    Square = mybir.ActivationFunctionType.Square
    Sqrt = mybir.ActivationFunctionType.Sqrt
    Copy = mybir.ActivationFunctionType.Copy
    PSUM = bass.MemorySpace.PSUM

    with tile.TileContext(nc) as tc:
        # ---- persistent sbuf ----
        with tc.tile_pool(name="persist", bufs=1) as pp:
            # head h lives at partition rows (h//4)*64 (matching its kv head's
            # rows so matmul operand bases agree), free-dim chunk h%4
            # triu[p, j] = 1.0 where p <= j (causal keep-mask for the
            # scoresT boundary strip), 0 above
            triu = pp.tile([128, 128], bf16)
            qT_sb = pp.tile([128, 4, S], bf16)        # q (roped+normed)
            kT_sb = pp.tile([128, S], bf16)           # k (roped+normed)
            g_sb = pp.tile([128, 4, S], bf16)         # sigmoid(gate)
            v_sb = pp.tile([128, KC, KVH, HD + 1], bf16)  # natural V + ones col
            og_sb = pp.tile([128, 4, S], bf16)        # raw gate, then gated out

            nc.vector.memset(v_sb[:, :, :, HD:HD + 1], 1.0)

            # ================= phase 1: projections =================
            with tc.tile_pool(name="consts", bufs=1) as cp, \
                 tc.tile_pool(name="wts", bufs=1) as wp, \
                 tc.tile_pool(name="hblk", bufs=2) as hp, \
                 tc.tile_pool(name="work", bufs=3) as wk, \
                 tc.tile_pool(name="smallw", bufs=3) as smp, \
                 tc.tile_pool(name="rbpool", bufs=3) as rbp, \
                 tc.tile_pool(name="rwork", bufs=3) as rwk, \
                 tc.tile_pool(name="pps", bufs=2, space=PSUM) as pps, \
                 tc.tile_pool(name="sqps", bufs=2, space=PSUM) as sqps, \
                 tc.tile_pool(name="rbps", bufs=2, space=PSUM) as rbps, \
                 tc.tile_pool(name="trps", bufs=2, space=PSUM) as trps:

                # DMA order matters for the kernel preamble: wq + first hidden
                # block gate the first matmul, so they go first; wo is only
                # needed by o_proj and loads inside the attention scope.
                wq_sb = wp.tile([128, KC, QD], bf16)
                wk_sb = wp.tile([128, KC, KD], bf16)
                wv_sb = wp.tile([128, KC, KD], bf16)
                wg_sb = wp.tile([128, KC, QD], bf16)
                nc.sync.dma_start(out=wq_sb,
                                  in_=wqT.rearrange("(c p) m -> p c m", p=128))
                hblk0 = hp.tile([128, KC, 512], bf16, tag="hblk")
                nc.sync.dma_start(
                    out=hblk0,
                    in_=hT[:, 0:512].rearrange("(c p) s -> p c s", p=128))
                csA_q = cp.tile([128, S], bf16)
                csB_q = cp.tile([128, S], bf16)
                csA_k = cp.tile([128, S], bf16)
                csB_k = cp.tile([128, S], bf16)
                nc.sync.dma_start(out=csA_q, in_=csAq)
                nc.sync.dma_start(out=csB_q, in_=csBq)
                nc.sync.dma_start(out=csA_k, in_=csAk)
                nc.sync.dma_start(out=csB_k, in_=csBk)
                ident = cp.tile([128, 128], bf16)
                from concourse.masks import make_identity, make_upper_triangular
                make_identity(nc, ident)
                make_upper_triangular(nc, triu, val=1.0, diag=True)
                ones2 = cp.tile([128, 2], bf16)
                nc.vector.memset(ones2, 0.0)
                nc.vector.memset(ones2[0:64, 0:1], 1.0)
                nc.vector.memset(ones2[64:128, 1:2], 1.0)
                # sel2.T broadcast: row p of (sel2.T @ rstd) = rstd[p//64]
                sel2 = cp.tile([2, 128], f32)
                nc.sync.dma_start(out=sel2, in_=sel2d)
                eps_t = cp.tile([8, 1], f32)
                nc.vector.memset(eps_t, EPS)

                nc.sync.dma_start(out=wk_sb,
                                  in_=wkT.rearrange("(c p) m -> p c m", p=128))
                nc.sync.dma_start(out=wv_sb,
                                  in_=wvT.rearrange("(c p) m -> p c m", p=128))
                nc.sync.dma_start(out=wg_sb,
                                  in_=wgT.rearrange("(c p) m -> p c m", p=128))
                def rope_norm(ps, csA, csB):
                    """ps: psum [128,512] raw proj. Returns (qa bf16 roped,
                    rb_s bf16 [128,512] rstd broadcast)."""
                    sq_t = wk.tile([128, 512], bf16, tag="sq")
                    nc.scalar.activation(out=sq_t, in_=ps, func=Square)
                    qb = rwk.tile([128, 512], bf16, tag="qb")
                    nc.scalar.activation(out=qb, in_=ps, func=Copy)
                    # sum of squares per 64-row half via matmul
                    sq_ps = sqps.tile([2, 512], f32, tag="sqs")
                    nc.tensor.matmul(sq_ps, ones2, sq_t, start=True, stop=True)
                    sstd = smp.tile([2, 512], f32, tag="sstd")
                    nc.scalar.activation(out=sstd, in_=sq_ps, func=Sqrt,
                                         scale=1.0 / HD, bias=eps_t[0:2])
                    rstd = smp.tile([2, 512], f32, tag="rstd")
                    nc.vector.reciprocal_approx_fast(out=rstd, in_=sstd)
                    # broadcast rstd rows to halves via matmul
                    rb_ps = rbps.tile([128, 512], f32, tag="rb")
                    nc.tensor.matmul(rb_ps, sel2, rstd, start=True, stop=True)
                    rb_s = rbp.tile([128, 512], bf16, tag="rbs")
                    nc.vector.tensor_copy(out=rb_s, in_=rb_ps)
                    # rope: qa = qb*csA + rot(qb)*csB
                    rot = rwk.tile([128, 512], bf16, tag="rot")
                    for hh in (0, 64):
                        # 16-row rotate needs non-32-aligned partition bases:
                        # only DMA can address those
                        nc.gpsimd.dma_start(out=rot[hh + 0:hh + 16],
                                            in_=qb[hh + 16:hh + 32])
                        nc.gpsimd.dma_start(out=rot[hh + 16:hh + 32],
                                            in_=qb[hh + 0:hh + 16])
                        nc.vector.tensor_copy(out=rot[hh + 32:hh + 64],
                                              in_=qb[hh + 32:hh + 64])
                    nc.vector.tensor_mul(rot, rot, csB)
                    qa = rwk.tile([128, 512], bf16, tag="qa")
                    nc.vector.tensor_mul(qa, qb, csA)
                    nc.vector.tensor_add(qa, qa, rot)
                    return qa, rb_s

                for sb in range(SB):
                    s0 = sb * 512
                    if sb == 0:
                        hblk = hblk0
                    else:
                        hblk = hp.tile([128, KC, 512], bf16, tag="hblk")
                        nc.sync.dma_start(
                            out=hblk,
                            in_=hT[:, s0:s0 + 512].rearrange("(c p) s -> p c s",
                                                             p=128))
                    # ---- q projection (4 chunks of 128 rows) ----
                    for m in range(4):
                        ps = pps.tile([128, 512], f32, tag="proj")
                        for kc in range(KC):
                            nc.tensor.matmul(ps, wq_sb[:, kc, m * 128:(m + 1) * 128],
                                             hblk[:, kc, :],
                                             start=(kc == 0), stop=(kc == KC - 1))
                        qa, rb_s = rope_norm(ps, csA_q[:, s0:s0 + 512],
                                             csB_q[:, s0:s0 + 512])
                        # heads 2m, 2m+1 -> row-half r=m//2, chunks 2*(m%2)+{0,1}
                        r = (m // 2) * 64
                        cb = 2 * (m % 2)
                        nc.vector.tensor_mul(
                            qT_sb[r:r + 64, cb, s0:s0 + 512],
                            qa[0:64, :], rb_s[0:64, :])
                        nc.vector.tensor_mul(
                            qT_sb[r:r + 64, cb + 1, s0:s0 + 512],
                            qa[64:128, :], rb_s[64:128, :])
                    # ---- k projection (1 chunk) ----
                    ps = pps.tile([128, 512], f32, tag="proj")
                    for kc in range(KC):
                        nc.tensor.matmul(ps, wk_sb[:, kc, :], hblk[:, kc, :],
                                         start=(kc == 0), stop=(kc == KC - 1))
                    ka, rb_s = rope_norm(ps, csA_k[:, s0:s0 + 512],
                                         csB_k[:, s0:s0 + 512])
                    nc.vector.tensor_mul(kT_sb[:, s0:s0 + 512], ka, rb_s)
                    # ---- v projection + transpose to natural layout ----
                    ps = pps.tile([128, 512], f32, tag="proj")
                    for kc in range(KC):
                        nc.tensor.matmul(ps, wv_sb[:, kc, :], hblk[:, kc, :],
                                         start=(kc == 0), stop=(kc == KC - 1))
                    vt = wk.tile([128, 512], bf16, tag="vt")
                    nc.scalar.activation(out=vt, in_=ps, func=Copy)
                    for ss in range(4):
                        tp = trps.tile([128, 128], bf16, tag="tp")
                        nc.tensor.transpose(tp, vt[:, ss * 128:(ss + 1) * 128],
                                            ident)
                        chunk = sb * 4 + ss
                        nc.vector.tensor_copy(out=v_sb[:, chunk, :, 0:HD],
                                              in_=tp.rearrange("p (kv d) -> p kv d",
                                                               kv=2))
                    # ---- gate projection -> raw staged into og_sb ----
                    for m in range(4):
                        ps = pps.tile([128, 512], f32, tag="proj")
                        for kc in range(KC):
                            nc.tensor.matmul(ps, wg_sb[:, kc, m * 128:(m + 1) * 128],
                                             hblk[:, kc, :],
                                             start=(kc == 0), stop=(kc == KC - 1))
                        nc.scalar.activation(out=og_sb[:, m, s0:s0 + 512],
                                             in_=ps, func=Copy)

                # pre-warm the gpsimd broadcast ext-isa lib (~7us load) after
                # the last gpsimd dma_start, so attention's first
                # partition_broadcast doesn't pay it on the critical path
                warm_b = cp.tile([64, 16], f32)
                warm_s = cp.tile([1, 16], f32)
                nc.vector.memset(warm_s, 1.0)
                nc.gpsimd.partition_broadcast(warm_b, warm_s)

            # batched sigmoid: og_sb (raw gate) -> g_sb; one table switch
            for m in range(4):
                nc.scalar.activation(out=g_sb[:, m, :], in_=og_sb[:, m, :],
                                     func=Sigmoid)

            if debug_dump:
                nc.sync.dma_start(out=dbg_g, in_=g_sb)

            # ============ phase 2: attention + interleaved o_proj ============
            with tc.tile_pool(name="probs", bufs=8) as prp, \
                 tc.tile_pool(name="att_sm", bufs=4) as asm, \
                 tc.tile_pool(name="ostg", bufs=4) as ostg, \
                 tc.tile_pool(name="p2w", bufs=1) as p2w, \
                 tc.tile_pool(name="scps", bufs=2, space=PSUM) as scps, \
                 tc.tile_pool(name="avps", bufs=2, space=PSUM) as avps, \
                 tc.tile_pool(name="pops", bufs=1, space=PSUM) as pops:

                # o_proj weights load in the background during attention Q0
                wo_sb = p2w.tile([128, 4, KC, 128], bf16)
                nc.sync.dma_start(out=wo_sb,
                                  in_=woT.rearrange("(c p) (mb mm) -> p c mb mm",
                                                    p=128, mm=128))

                def emit_pv(av, kc, loc, ptl, qb):
                    last_kc = 4 * qb + 3
                    for hh in range(2):
                        nc.tensor.matmul(
                            av[hh][:, loc:512],
                            v_sb[:, kc, hh, :],
                            ptl[:, hh, loc:512],
                            start=(kc == 0), stop=(kc == last_kc))

                def attn_pair(hp_idx, qb):
                    """Heads hA=hp_idx (kv0, rows 0:64) and hB=hp_idx+4 (kv1,
                    rows 64:128), both free-dim chunk hp_idx; 512-col q block
                    qb (av is one psum bank per head)."""
                    q0 = qb * 512
                    av_a = avps.tile([65, 512], f32, tag="av")
                    av_b = avps.tile([65, 512], f32, tag="av")
                    av = [av_a, av_b]
                    # PV emission is deferred one slot: slot s+1's score MMs
                    # enter the PE FIFO before slot s's PV, so the PE is never
                    # head-of-line blocked on exp(s).
                    pv_pend = None
                    for kc in range(4 * qb + 4):
                        st = max(0, kc * 128 - q0)   # first valid local col
                        sc2 = scps.tile([128, 2, 512], f32, tag="sc")
                        for hh in range(2):  # interleave -> row-tiled pair
                            p0 = hh * 64
                            nc.tensor.matmul(
                                sc2[:, hh, st:512],
                                kT_sb[p0:p0 + 64, kc * 128:(kc + 1) * 128],
                                qT_sb[p0:p0 + 64, hp_idx,
                                      q0 + st:q0 + 512],
                                start=True, stop=True)
                        # probs tile per kc: [*, head, col-in-block]
                        ptl = prp.tile([128, 2, 512], bf16, tag="ptl")
                        if st == 0:
                            # full slot: contiguous 1-region exp
                            nc.scalar.activation(
                                out=ptl.rearrange("p a b -> p (a b)"),
                                in_=sc2.rearrange("p a b -> p (a b)"),
                                func=Exp, scale=SCALE)
                        else:
                            for hh in range(2):
                                nc.scalar.activation(
                                    out=ptl[:, hh, st:512],
                                    in_=sc2[:, hh, st:512],
                                    func=Exp, scale=SCALE)
                        if kc * 128 >= q0:
                            # boundary strip: keep k<=q inside [st, st+128)
                            for hh in range(2):
                                nc.vector.tensor_mul(
                                    ptl[:, hh, st:st + 128],
                                    ptl[:, hh, st:st + 128], triu)
                        if pv_pend is not None:
                            emit_pv(*pv_pend)
                        pv_pend = (av, kc, st, ptl, qb)
                    emit_pv(*pv_pend)
                    # flush av psum -> sbuf (bf16) so the psum slots free
                    # immediately; the normalize/gate tail is emitted LATER
                    # (after the next pair's kc loop) so Tile's position-keyed
                    # slot releases don't serialize pairs on the tail chain.
                    avs = []
                    for hh in range(2):
                        av_sb = asm.tile([65, 512], bf16, tag="avs")
                        nc.vector.tensor_copy(out=av_sb, in_=av[hh])
                        avs.append(av_sb)
                    return (avs, hp_idx, qb)

                def attn_tail(ctx):
                    avs, hp_idx, qb = ctx
                    qsl = slice(qb * 512, (qb + 1) * 512)
                    hc = [hp_idx // 2, hp_idx // 2 + 2]
                    rr = (hp_idx % 2) * 64
                    for hh in range(2):
                        av_sb = avs[hh]
                        # custom DVE op mishandles base_partition 64 and needs
                        # f32 input: stage the denominator row to base 0
                        den = asm.tile([1, 512], f32, tag="den")
                        nc.vector.tensor_copy(out=den, in_=av_sb[64:65, :])
                        recip = asm.tile([1, 512], f32, tag="recip")
                        nc.vector.reciprocal_approx_fast(out=recip, in_=den)
                        rbv = asm.tile([64, 512], f32, tag="rbv")
                        nc.gpsimd.partition_broadcast(rbv, recip)
                        dst = og_sb[rr:rr + 64, hc[hh], qsl]
                        nc.vector.tensor_mul(dst, av_sb[0:64, :], rbv)
                        nc.vector.tensor_mul(
                            dst, dst, g_sb[rr:rr + 64, hc[hh], qsl])

                def oproj_m(units, use_act):
                    """o_proj for two (m, nb) units sharing one po tile. Uses
                    the dedicated po psum banks, so it never steals
                    attention's score-psum slots."""
                    po = pops.tile([128, 2, 512], f32, tag="po")
                    for oc in range(4):
                        for j, (m, nb) in enumerate(units):
                            nc.tensor.matmul(
                                po[:, j, :], wo_sb[:, oc, m, :],
                                og_sb[:, oc, nb * 512:(nb + 1) * 512],
                                start=(oc == 0), stop=(oc == 3))
                    for j, (m, nb) in enumerate(units):
                        stg = ostg.tile([128, 512], bf16, tag="stg")
                        if use_act and j == 0:
                            # tail oproj: ACT is idle (exp done) -> share copies
                            nc.scalar.activation(out=stg, in_=po[:, j, :],
                                                 func=Copy)
                        else:
                            nc.vector.tensor_copy(out=stg, in_=po[:, j, :])
                        nc.sync.dma_start(
                            out=outT[m * 128:(m + 1) * 128,
                                     nb * 512:(nb + 1) * 512],
                            in_=stg)

                # software-pipeline: pair p's normalize tail is emitted after
                # pair p+1's kc loop. oproj is interleaved into later q
                # blocks as each og block completes: nb0/1 units into
                # qb2/qb3, nb2 units into qb3; only nb3 (+ leftovers) tails.
                work01 = [(m, nb) for m in range(KC) for nb in (0, 1)]
                work2 = [(m, 2) for m in range(KC)]
                queue = []          # chunk-units awaiting emission
                pending = None
                for qb in range(4):
                    if qb == 2:
                        queue = work01
                    elif qb == 3:
                        queue = queue + work2
                    for hp_idx in range(4):
                        ctx = attn_pair(hp_idx, qb)
                        if pending is not None:
                            attn_tail(pending)
                        pending = ctx
                        if qb == 2 and hp_idx >= 1:
                            take, queue = queue[:6], queue[6:]
                        elif qb == 3:
                            take, queue = queue[:8], queue[8:]
                        else:
                            take = []
                        for u in range(0, len(take) - 1, 2):
                            oproj_m(take[u:u + 2], use_act=False)
                attn_tail(pending)
                queue = queue + [(m, 3) for m in range(KC)]
                for u in range(0, len(queue) - 1, 2):
                    oproj_m(queue[u:u + 2], use_act=True)

            if debug_dump:
                nc.sync.dma_start(out=dbg_q, in_=qT_sb)
                nc.sync.dma_start(out=dbg_k, in_=kT_sb)
                nc.sync.dma_start(out=dbg_v, in_=v_sb)
                nc.sync.dma_start(out=dbg_og, in_=og_sb)

    nc.compile()
    return nc


def _host_prep(hidden_states, cos, sin, Wq, Wk, Wv, Wg, Wo, q_norm_w, k_norm_w):
    """Build per-core input maps."""
    def cs_tables(cos_b, sin_b, w):
        # csA/csB [128, S]: row p -> head-local dim d = p % 64
        A = np.empty((128, S), np.float32)
        Bt = np.empty((128, S), np.float32)
        cosT = cos_b.T  # [32, S]
        sinT = sin_b.T
        for blk in (0, 64):
            A[blk + 0:blk + 32] = cosT * w[0:32, None]
            A[blk + 32:blk + 64] = w[32:64, None]
            Bt[blk + 0:blk + 16] = -sinT[0:16] * w[16:32, None]
            Bt[blk + 16:blk + 32] = sinT[16:32] * w[0:16, None]
            Bt[blk + 32:blk + 64] = 0.0
        return A.astype(BF16), Bt.astype(BF16)

    sel2_host = np.zeros((2, 128), np.float32)
    sel2_host[0, 0:64] = 1.0
    sel2_host[1, 64:128] = 1.0
    in_maps = []
    for c in range(NCORES):
        b, g = c // 4, c % 4
        qs = slice(g * QD, (g + 1) * QD)
        ks = slice(g * KD, (g + 1) * KD)
        csA_q, csB_q = cs_tables(cos[b], sin[b], np.asarray(q_norm_w))
        csA_k, csB_k = cs_tables(cos[b], sin[b], np.asarray(k_norm_w))
        in_maps.append({
            "hT": np.ascontiguousarray(hidden_states[b].T).astype(BF16),
            "wqT": np.ascontiguousarray(Wq[qs].T).astype(BF16),
            "wkT": np.ascontiguousarray(Wk[ks].T).astype(BF16),
            "wvT": np.ascontiguousarray(Wv[ks].T).astype(BF16),
            "wgT": np.ascontiguousarray(Wg[qs].T).astype(BF16),
            "woT": np.ascontiguousarray(Wo[:, qs].T).astype(BF16),
            "csAq": csA_q, "csBq": csB_q, "csAk": csA_k, "csBk": csB_k,
            "sel2": sel2_host,
        })
    return in_maps


def kernel(hidden_states, cos, sin, Wq, Wk, Wv, Wg, Wo, q_norm_w, k_norm_w):
    from concourse import bass_utils

    if "nc" not in _CACHE:
        _CACHE["nc"] = _build_bass()
    nc = _CACHE["nc"]

    in_maps = _host_prep(hidden_states, cos, sin, Wq, Wk, Wv, Wg, Wo,
                         q_norm_w, k_norm_w)

    trace = bool(int(os.environ.get("KERNEL_TRACE", "0")))
    kwargs = {}
    if trace:
        # the agent image's antenv lacks axon_hooks; recreate it from the
        # boot helper so run_bass_kernel_spmd(trace=True) can NTFF-profile
        try:
            import antenv.axon_hooks  # noqa: F401
        except ImportError:
            import types
            sys.path.insert(0, "/root/.axon_site")
            from trn_agent_boot.trn_boot import _ntff_profile_via_ctypes
            hook = _ntff_profile_via_ctypes("/opt/axon/libaxon_pjrt.so")
            mod = types.ModuleType("antenv.axon_hooks")
            mod.get_axon_ntff_profile_hook = lambda: hook
            sys.modules["antenv.axon_hooks"] = mod
        tmpdir = os.environ.get("KERNEL_TRACE_DIR") or None
        kwargs = dict(trace=True, tmpdir=tmpdir)
    res = bass_utils.run_bass_kernel_spmd(nc, in_maps,
                                          core_ids=list(range(NCORES)),
                                          **kwargs)
    if trace and res.exec_time_ns is not None:
        print(f"HW exec time: {res.exec_time_ns} ns")
        _CACHE["exec_time_ns"] = res.exec_time_ns

    out = np.zeros((B, S, HID), np.float32)
    for c in range(NCORES):
        b = c // 4
        out[b] += res.results[c]["outT"].T.astype(np.float32)
    return out


if __name__ == "__main__":
    rng = np.random.default_rng(0)
    hs = rng.standard_normal((B, S, HID), dtype=np.float32)
    cos = rng.random((B, S, ROPE), dtype=np.float32)
    sin = rng.random((B, S, ROPE), dtype=np.float32)
    out = kernel(hidden_states=hs, cos=cos, sin=sin,
                 Wq=rng.standard_normal((NH * HD, HID), dtype=np.float32) * 0.02,
                 Wk=rng.standard_normal((NKV * HD, HID), dtype=np.float32) * 0.02,
                 Wv=rng.standard_normal((NKV * HD, HID), dtype=np.float32) * 0.02,
                 Wg=rng.standard_normal((NH * HD, HID), dtype=np.float32) * 0.02,
                 Wo=rng.standard_normal((HID, NH * HD), dtype=np.float32) * 0.02,
                 q_norm_w=np.ones(HD, np.float32),
                 k_norm_w=np.ones(HD, np.float32))
    print(out.shape, out.dtype)
